# revision 1
# baseline (speedup 1.0000x reference)
"""Grouped gated DeltaNet (KDA-style) on 8 TRN2 NeuronCores.

Sharding: core c -> (batch b = c//4, head-group hg = c%4 of 4 heads).
Each core: column-sharded projections, short-conv+silu, l2norm, chunked
gated delta-rule recurrence (chunk C=128, group-factorized per-channel
decay, triangular solve by Neumann doubling on TensorE), gated RMSNorm,
row-shard of the output projection. Host sums 4 partials per batch.

Self-contained: B=2, T=1024, D=2048, H=16, DK=DV=128 hardcoded.
"""
import sys
sys.path.insert(0, '/opt/trn_rl_repo')
import numpy as np
import ml_dtypes
from contextlib import ExitStack

B, T, D = 2, 1024, 2048
H, DK, DV, GG = 16, 128, 128, 16
NG = DK // GG          # 8 gate groups per head
NH = 4                 # heads per core
C = 128                # chunk length
NCH = T // C
SCALE = DK ** -0.5
EPS = 1e-5

BF = ml_dtypes.bfloat16
_CACHE = {}


def _build():
    import concourse.tile as tile
    from concourse import bacc, mybir

    fp32 = mybir.dt.float32
    bf16 = mybir.dt.bfloat16
    Alu = mybir.AluOpType
    Act = mybir.ActivationFunctionType

    nc = bacc.Bacc("TRN2", target_bir_lowering=False, debug=False, num_devices=8)
    dp = lambda n, sh, dt: nc.dram_tensor(n, sh, dt, kind="ExternalInput").ap()
    hT = dp("hT", [D, T], bf16)
    wq = dp("wq", [D, NH * DK], bf16)
    wk = dp("wk", [D, NH * DK], bf16)
    wv = dp("wv", [D, NH * DV], bf16)
    wg = dp("wg", [D, NH * DV], bf16)
    wo = dp("wo", [NH * DV, D], bf16)
    wf1 = dp("wf1", [D, DV], bf16)
    wf2 = dp("wf2", [DV, NH * NG], bf16)
    wb = dp("wb", [D, NH], bf16)
    cw = dp("cw", [NH * DK, 12], fp32)
    nega = dp("nega", [NH * NG, 1], fp32)
    dtb = dp("dtb", [NH * NG, 1], fp32)
    bgc = dp("bgc", [DV, NH], fp32)
    normw = dp("normw", [DV, 1], fp32)
    repl = dp("repl", [NG, DK], fp32)
    self8f = dp("self8f", [NG, NG * C], fp32)
    ones1b = dp("ones1b", [1, C], bf16)
    onescol = dp("onescol", [DK, 1], bf16)
    oh8 = dp("oh8", [DK, 64], bf16)
    sel8b = dp("sel8b", [8, 8 * 128], bf16)
    gmc = dp("gmc", [DK, NG], fp32)
    sc8 = dp("sc8", [8, 1], fp32)
    eps8 = dp("eps8", [8, 1], fp32)
    epsn = dp("epsn", [1, 1], fp32)
    maskM = dp("maskM", [C, C], bf16)
    maskG = dp("maskG", [C, C], bf16)
    idbf = dp("idbf", [128, 128], bf16)
    idf32 = dp("idf32", [128, 128], fp32)
    outT = nc.dram_tensor("outT", [D, T], fp32, kind="ExternalOutput").ap()

    with tile.TileContext(nc) as tc, ExitStack() as ctx:
        pool = lambda name, bufs, space="SBUF": ctx.enter_context(
            tc.tile_pool(name=name, bufs=bufs, space=space))

        cons = pool("cons", 1)
        htp = pool("htp", 1)
        wst = pool("wst", 1)
        wsm = pool("wsm", 1)
        pers = pool("pers", 1)
        convp = pool("convp", 1)
        sqp = pool("sqp", 1)
        chk = pool("chk", 2)
        big = pool("big", 1)
        st = pool("st", 1)
        # PSUM: 8 banks total.  proj(2) + big(2) + sm1(2) + gp(1) + otp(1)
        pps = pool("pps", 2, "PSUM")
        pbig = pool("pbig", 1, "PSUM")
        psm = pool("psm", 1, "PSUM")

        def sm1(shape, dt=fp32):
            return psm.tile(shape, dt, tag="sm1", bufs=2, name="sm1t")

        dma = nc.sync.dma_start

        # ---- consts ----
        cwt = []
        for m in range(4):
            t = cons.tile([128, 12], fp32, tag=f"cw{m}", name=f"cw{m}")
            dma(t[:], cw[m * 128:(m + 1) * 128, :])
            cwt.append(t)

        def ctile(shape, dt, src, nm):
            t = cons.tile(shape, dt, tag=nm, name=nm)
            dma(t[:], src[:])
            return t
        negat = ctile([32, 1], fp32, nega, "negat")
        dtbt = ctile([32, 1], fp32, dtb, "dtbt")
        bgt = ctile([128, 4], fp32, bgc, "bgt")
        nwt = ctile([128, 1], fp32, normw, "nwt")
        replt = ctile([8, 128], fp32, repl, "replt")
        s8f = ctile([NG, NG * C], fp32, self8f, "s8f")
        o1b = ctile([1, C], bf16, ones1b, "o1b")
        oct_ = ctile([128, 1], bf16, onescol, "oct")
        oh8t = ctile([128, 64], bf16, oh8, "oh8t")
        s8b = ctile([8, 8 * 128], bf16, sel8b, "s8b")
        gmct = ctile([128, NG], fp32, gmc, "gmct")
        sc8t = ctile([8, 1], fp32, sc8, "sc8t")
        eps8t = ctile([8, 1], fp32, eps8, "eps8t")
        epsnt = ctile([1, 1], fp32, epsn, "epsnt")
        mMt = ctile([128, 128], bf16, maskM, "mMt")
        mGt = ctile([128, 128], bf16, maskG, "mGt")
        idb = ctile([128, 128], bf16, idbf, "idb")
        idf = ctile([128, 128], fp32, idf32, "idf")
        ones32 = cons.tile([32, C], fp32, tag="ones32", name="ones32")
        nc.vector.memset(ones32[:], 1.0)

        # ---- hidden^T resident ----
        ht = []
        for k in range(16):
            t = htp.tile([128, T], bf16, tag=f"ht{k}", name=f"ht{k}")
            dma(t[:], hT[k * 128:(k + 1) * 128, :])
            ht.append(t)

        # ---- projections ----
        mk = lambda p, nm, dt=bf16, sh=None: [
            p.tile(sh or [128, T], dt, tag=f"{nm}{m}", name=f"{nm}{m}") for m in range(4)]
        qb, kb, vb = mk(pers, "qb"), mk(pers, "kb"), mk(pers, "vb")
        gateb, yb = mk(pers, "gateb"), mk(pers, "yb")
        f1b = pers.tile([128, T], bf16, tag="f1b", name="f1b")
        qs = {}
        ssqsb = cons.tile([8, T], fp32, tag="ssqsb", name="ssqsb")
        nc.vector.memset(ssqsb[:], 0.0)

        def project(w_ap, m, dst_bf16=None, conv_slot=None, pair=None, gate_bias=None):
            wt = [wst.tile([128, 512], bf16, tag=f"w{k}", name=f"wt{k}")
                  for k in range(16)]
            for k in range(16):
                dma(wt[k][:], w_ap[k * 128:(k + 1) * 128, :])
            xpad = None
            if conv_slot is not None:
                xpad = convp.tile([128, T + 3], fp32, tag="xpad", name="xpad", bufs=2)
                nc.vector.memset(xpad[:, 0:3], 0.0)
            for half in range(2):
                ps = pps.tile([128, 512], fp32, tag="proj", name="projps")
                for k in range(16):
                    nc.tensor.matmul(ps[:], wt[k][:, m * 128:(m + 1) * 128],
                                     ht[k][:, half * 512:(half + 1) * 512],
                                     start=(k == 0), stop=(k == 15))
                if xpad is not None:
                    nc.scalar.copy(xpad[:, 3 + half * 512: 3 + (half + 1) * 512], ps[:])
                elif gate_bias is not None:
                    nc.scalar.activation(dst_bf16[:, half * 512:(half + 1) * 512],
                                         ps[:], Act.Silu, bias=gate_bias)
                else:
                    nc.scalar.copy(dst_bf16[:, half * 512:(half + 1) * 512], ps[:])
            if xpad is None:
                return
            cwm = cwt[m]
            s = conv_slot * 4
            a = convp.tile([128, T], fp32, tag="acca", name="acca")
            bt = convp.tile([128, T], fp32, tag="accb", name="accb")
            nc.vector.tensor_scalar(a[:], xpad[:, 3:3 + T], cwm[:, s + 3:s + 4], None,
                                    op0=Alu.mult)
            cur, nxt = a, bt
            for kk in (2, 1, 0):
                nc.vector.scalar_tensor_tensor(nxt[:], xpad[:, kk:kk + T],
                                               cwm[:, s + kk:s + kk + 1], cur[:],
                                               op0=Alu.mult, op1=Alu.add)
                cur, nxt = nxt, cur
            if pair is None:
                nc.scalar.activation(dst_bf16[:], cur[:], Act.Silu)
            else:
                qsil = qb[pair] if pair < 4 else kb[pair - 4]
                qs[pair] = qsil
                nc.scalar.activation(qsil[:], cur[:], Act.Silu)
                sq = sqp.tile([128, T], bf16, tag="sq", name="sq")
                nc.scalar.activation(sq[:], qsil[:], Act.Square)
                for half in range(2):
                    pss = sm1([8, 512])
                    nc.tensor.matmul(pss[:], oh8t[:, pair * 8:pair * 8 + 8],
                                     sq[:, half * 512:(half + 1) * 512],
                                     start=True, stop=True)
                    nc.vector.tensor_tensor(ssqsb[:, half * 512:(half + 1) * 512],
                                            ssqsb[:, half * 512:(half + 1) * 512],
                                            pss[:], op=Alu.add)

        for m in range(4):
            project(wq, m, conv_slot=0, pair=m)
        for m in range(4):
            project(wk, m, conv_slot=1, pair=4 + m)
        for m in range(4):
            project(wv, m, dst_bf16=vb[m], conv_slot=2)
        for m in range(4):
            project(wg, m, dst_bf16=gateb[m], gate_bias=bgt[:, m:m + 1])

        # l2 normalizers
        nrm = cons.tile([8, T], fp32, tag="nrm", name="nrm")
        rec = cons.tile([8, T], fp32, tag="ssqsb", name="rec")
        recb = cons.tile([8, T], bf16, tag="recb", name="recb")
        nc.scalar.activation(nrm[:], ssqsb[:], Act.Ln, scale=sc8t[:, 0:1],
                             bias=eps8t[:, 0:1])
        nc.scalar.activation(recb[:], nrm[:], Act.Exp, scale=-0.5)
        for pair in range(8):
            dst = qb[pair] if pair < 4 else kb[pair - 4]
            for half in range(2):
                nb = sm1([128, 512])
                nc.tensor.matmul(nb[:], s8b[:, pair * 128:(pair + 1) * 128],
                                 recb[:, half * 512:(half + 1) * 512],
                                 start=True, stop=True)
                nc.vector.tensor_tensor(dst[:, half * 512:(half + 1) * 512],
                                        qs[pair][:, half * 512:(half + 1) * 512],
                                        nb[:], op=Alu.mult)

        # ---- f / beta ----
        wt1 = [wsm.tile([128, 128], bf16, tag=f"wf1_{k}", name=f"wf1_{k}")
               for k in range(16)]
        for k in range(16):
            dma(wt1[k][:], wf1[k * 128:(k + 1) * 128, :])
        for half in range(2):
            ps = pps.tile([128, 512], fp32, tag="proj", name="f1ps")
            for k in range(16):
                nc.tensor.matmul(ps[:], wt1[k][:], ht[k][:, half * 512:(half + 1) * 512],
                                 start=(k == 0), stop=(k == 15))
            nc.scalar.copy(f1b[:, half * 512:(half + 1) * 512], ps[:])
        wf2t = wsm.tile([128, 32], bf16, tag="wf2t", name="wf2t")
        dma(wf2t[:], wf2[:])
        wbt = [wsm.tile([128, 4], bf16, tag=f"wb{k}", name=f"wbt{k}")
               for k in range(16)]
        for k in range(16):
            dma(wbt[k][:], wb[k * 128:(k + 1) * 128, :])
        gna = cons.tile([32, T], fp32, tag="gna", name="gna")
        bsg = cons.tile([4, T], fp32, tag="bsg", name="bsg")
        for half in range(2):
            gps = sm1([32, 512])
            nc.tensor.matmul(gps[:], wf2t[:], f1b[:, half * 512:(half + 1) * 512],
                             start=True, stop=True)
            spe = chk.tile([32, 512], fp32, tag="spe", name="spe")
            nc.scalar.activation(spe[:], gps[:], Act.Exp, bias=dtbt[:, 0:1])
            sp1 = chk.tile([32, 512], fp32, tag="sp", name="sp1")
            nc.vector.tensor_scalar(sp1[:], spe[:], 1.0, None, op0=Alu.add)
            sp = chk.tile([32, 512], fp32, tag="spe", name="sp")
            nc.scalar.activation(sp[:], sp1[:], Act.Ln)
            nc.vector.tensor_scalar(gna[:, half * 512:(half + 1) * 512], sp[:],
                                    negat[:, 0:1], None, op0=Alu.mult)
            bps = sm1([4, 512])
            for k in range(16):
                nc.tensor.matmul(bps[:], wbt[k][:], ht[k][:, half * 512:(half + 1) * 512],
                                 start=(k == 0), stop=(k == 15))
            nc.scalar.activation(bsg[:, half * 512:(half + 1) * 512], bps[:], Act.Sigmoid)

        # ---- recurrence ----
        Sf = [st.tile([128, 128], fp32, tag=f"Sf{h}", name=f"Sf{h}") for h in range(4)]
        Sb = [st.tile([128, 128], bf16, tag=f"Sb{h}", name=f"Sb{h}") for h in range(4)]
        for h in range(4):
            nc.vector.memset(Sf[h][:], 0.0)
            nc.vector.memset(Sb[h][:], 0.0)

        for ci in range(NCH):
            ts = slice(ci * C, (ci + 1) * C)
            cN32 = chk.tile([32, C], fp32, tag="cN32", name="cN32")
            nc.vector.tensor_tensor_scan(cN32[:], ones32[:], gna[:, ts], 0.0,
                                         op0=Alu.mult, op1=Alu.add)
            cntp = sm1([128, 32])
            nc.tensor.transpose(cntp[:], cN32[:], idf[0:32, 0:32])
            cNt = chk.tile([128, 32], fp32, tag="cNt", name="cNt")
            nc.scalar.copy(cNt[:], cntp[:])
            cN8s = []
            for h4 in range(4):
                c8p = sm1([8, C])
                nc.tensor.transpose(c8p[:], cNt[:, h4 * 8:(h4 + 1) * 8], idf[:])
                cN8 = chk.tile([8, C], fp32, tag=f"cN8_{h4}", name=f"cN8_{h4}")
                nc.scalar.copy(cN8[:], c8p[:])
                cN8s.append(cN8)
            b2p = sm1([128, 4])
            nc.tensor.transpose(b2p[:], bsg[:, ts], idf[0:4, 0:4])
            beta2 = chk.tile([128, 4], fp32, tag="beta2", name="beta2")
            nc.scalar.copy(beta2[:], b2p[:])

            for h in range(4):
                cfp = sm1([128, C])
                nc.tensor.matmul(cfp[:], replt[:], cN8s[h][:],
                                 start=True, stop=True)
                clast = chk.tile([128, 1], fp32, tag="clast", name="clast")
                nc.scalar.copy(clast[:], cfp[:, C - 1:C])
                bful = chk.tile([128, C], bf16, tag="bful", name="bful")
                nc.scalar.activation(bful[:], cfp[:], Act.Exp)
                bC = chk.tile([128, 1], fp32, tag="bC", name="bC")
                nc.scalar.activation(bC[:], cfp[:, C - 1:C], Act.Exp)
                kendf = chk.tile([128, C], bf16, tag="kendf", name="kendf")
                nc.scalar.activation(kendf[:], cfp[:], Act.Exp, scale=-1.0,
                                     bias=clast[:, 0:1])
                Wt = chk.tile([128, C], bf16, tag="Wt", name="Wt")
                nc.vector.tensor_tensor(Wt[:], kb[h][:, ts], bful[:], op=Alu.mult)
                qtT = chk.tile([128, C], bf16, tag="qtT", name="qtT")
                nc.vector.tensor_tensor(qtT[:], qb[h][:, ts], bful[:], op=Alu.mult)
                kend = chk.tile([128, C], bf16, tag="kend", name="kend")
                nc.vector.tensor_tensor(kend[:], kb[h][:, ts], kendf[:], op=Alu.mult)

                bca = pbig.tile([128, 8 * C], fp32, tag="big", name="bca")
                for n in range(8):
                    nc.tensor.matmul(bca[:, n * C:(n + 1) * C],
                                     s8f[:, n * 128:(n + 1) * 128],
                                     cN8s[h][:], start=True, stop=True)
                eall = big.tile([128, 8 * C], bf16, tag="eall", name="eall")
                for n in range(8):
                    dtn = chk.tile([128, C], fp32, tag="dtn", name="dtn")
                    nc.vector.tensor_scalar(dtn[:],
                                            bca[:, n * C:(n + 1) * C],
                                            cNt[:, h * 8 + n:h * 8 + n + 1], 0.0,
                                            op0=Alu.subtract, op1=Alu.min)
                    nc.scalar.activation(eall[:, n * C:(n + 1) * C], dtn[:], Act.Exp)
                kmsk = []
                for n in range(8):
                    km = chk.tile([128, C], bf16, tag=f"km{n}", name=f"km{n}")
                    nc.scalar.mul(km[:], kb[h][:, ts], gmct[:, n:n + 1])
                    kmsk.append(km)

                def corr_mat(src, maskt, nm):
                    pall = pbig.tile([128, 8 * C], fp32, tag="big", name=f"p{nm}")
                    for n in range(8):
                        nc.tensor.matmul(pall[:, n * C:(n + 1) * C],
                                         kmsk[n][:], src[:, ts],
                                         start=True, stop=True)
                    prod = big.tile([128, 8 * C], bf16, tag="prod", name=f"pr{nm}")
                    nc.vector.tensor_tensor(prod[:], eall[:], pall[:], op=Alu.mult)
                    t4 = chk.tile([128, 4 * C], bf16, tag=f"{nm}4", name=f"{nm}4")
                    nc.vector.tensor_tensor(t4[:], prod[:, :4 * C], prod[:, 4 * C:],
                                            op=Alu.add)
                    t2 = chk.tile([128, 2 * C], bf16, tag=f"{nm}2", name=f"{nm}2")
                    nc.vector.tensor_tensor(t2[:], t4[:, :2 * C], t4[:, 2 * C:],
                                            op=Alu.add)
                    t1 = chk.tile([128, C], bf16, tag=f"{nm}1", name=f"{nm}1")
                    nc.vector.tensor_tensor(t1[:], t2[:, :C], t2[:, C:], op=Alu.add)
                    tm = chk.tile([128, C], bf16, tag=f"{nm}m", name=f"{nm}m")
                    nc.vector.tensor_tensor(tm[:], t1[:], maskt[:], op=Alu.mult)
                    return tm
                MtM = corr_mat(kb[h], mMt, "M")
                GtM = corr_mat(qb[h], mGt, "G")
                Nt0 = chk.tile([128, C], bf16, tag="Nt0", name="Nt0")
                nc.vector.tensor_scalar(Nt0[:], MtM[:], beta2[:, h:h + 1], None,
                                        op0=Alu.mult)

                vtp = sm1([128, C], bf16)
                nc.tensor.transpose(vtp[:], vb[h][:, ts], idb[:])
                vt = chk.tile([128, C], bf16, tag="vt", name="vt")
                nc.scalar.copy(vt[:], vtp[:])
                ws0 = sm1([128, C])
                nc.tensor.matmul(ws0[:], Wt[:], Sb[h][:], start=True, stop=True)
                xf = chk.tile([128, C], fp32, tag="xf", name="xf")
                nc.vector.tensor_tensor(xf[:], vt[:], ws0[:], op=Alu.subtract)
                xb = chk.tile([128, C], bf16, tag="xb", name="xb")
                nc.scalar.copy(xb[:], xf[:])

                Hs = [Nt0]
                g0p = psm.tile([128, C], bf16, tag="gp", bufs=1, name="g0p")
                nc.tensor.transpose(g0p[:], Nt0[:], idb[:])
                gsb = chk.tile([128, C], bf16, tag="gsb", name="gsb")
                nc.scalar.copy(gsb[:], g0p[:])
                for lev in range(6):
                    sqps = psm.tile([128, C], fp32, tag="gp", bufs=1, name="sqps")
                    nc.tensor.matmul(sqps[:], Hs[-1][:], gsb[:], start=True, stop=True)
                    gnew = chk.tile([128, C], bf16, tag="gsb", name="gnew")
                    nc.scalar.copy(gnew[:], sqps[:])
                    htp_ = psm.tile([128, C], bf16, tag="gp", bufs=1, name="htp_")
                    nc.tensor.transpose(htp_[:], gnew[:], idb[:])
                    hnew = chk.tile([128, C], bf16, tag=f"H{lev + 1}", name=f"H{lev + 1}")
                    nc.scalar.copy(hnew[:], htp_[:])
                    Hs.append(hnew)
                    gsb = gnew
                for lev in range(6, -1, -1):
                    mx = psm.tile([128, C], fp32, tag="gp", bufs=1, name="mx")
                    nc.tensor.matmul(mx[:], Hs[lev][:], xb[:], start=True, stop=True)
                    xf2 = chk.tile([128, C], fp32, tag="xf", name="xf2")
                    nc.vector.tensor_tensor(xf2[:], xf[:], mx[:],
                                            op=(Alu.add if lev > 0 else Alu.subtract))
                    xf = xf2
                    xb = chk.tile([128, C], bf16, tag="xb", name="xb2")
                    nc.scalar.copy(xb[:], xf[:])
                u = chk.tile([128, C], fp32, tag="u", name="u")
                nc.vector.tensor_scalar(u[:], xf[:], beta2[:, h:h + 1], None,
                                        op0=Alu.mult)
                ub = chk.tile([128, C], bf16, tag="ub", name="ub")
                nc.scalar.copy(ub[:], u[:])

                otp = psm.tile([128, C], fp32, tag="otp", bufs=1, name="otp")
                nc.tensor.matmul(otp[:], Sb[h][:], qtT[:], start=True, stop=False)
                nc.tensor.matmul(otp[:], ub[:], GtM[:], start=False, stop=True)

                ktp = sm1([128, C], bf16)
                nc.tensor.transpose(ktp[:], kend[:], idb[:])
                kts = chk.tile([128, C], bf16, tag="kts", name="kts")
                nc.scalar.copy(kts[:], ktp[:])
                sup = sm1([128, C])
                nc.tensor.matmul(sup[:], kts[:], ub[:], start=True, stop=True)
                nc.vector.scalar_tensor_tensor(Sf[h][:], Sf[h][:], bC[:, 0:1],
                                               sup[:], op0=Alu.mult, op1=Alu.add)
                nc.scalar.copy(Sb[h][:], Sf[h][:])

                yf = chk.tile([128, C], fp32, tag="yf", name="yf")
                nc.vector.tensor_tensor(yf[:], gateb[h][:, ts], otp[:], op=Alu.mult)
                ysq = chk.tile([128, C], bf16, tag="ysq", name="ysq")
                nc.scalar.activation(ysq[:], yf[:], Act.Square)
                ssp = sm1([1, C])
                nc.tensor.matmul(ssp[:], oct_[:], ysq[:], start=True, stop=True)
                nrc = chk.tile([1, C], fp32, tag="nrc", name="nrc")
                nc.scalar.activation(nrc[:], ssp[:], Act.Ln, scale=1.0 / DV,
                                     bias=epsnt[:, 0:1])
                rcb = chk.tile([1, C], bf16, tag="rcb", name="rcb")
                nc.scalar.activation(rcb[:], nrc[:], Act.Exp, scale=-0.5)
                rbc = sm1([128, C])
                nc.tensor.matmul(rbc[:], o1b[:], rcb[:], start=True, stop=True)
                nc.vector.scalar_tensor_tensor(yb[h][:, ts], yf[:], nwt[:, 0:1],
                                               rbc[:], op0=Alu.mult, op1=Alu.mult)

        # ---- output projection ----
        wot = [pers.tile([128, D], bf16, tag=f"wo{k}", name=f"wo{k}") for k in range(4)]
        for k in range(4):
            dma(wot[k][:], wo[k * 128:(k + 1) * 128, :])
        for m in range(16):
            for half in range(2):
                ps = pps.tile([128, 512], fp32, tag="proj", name="ops")
                for k in range(4):
                    nc.tensor.matmul(ps[:], wot[k][:, m * 128:(m + 1) * 128],
                                     yb[k][:, half * 512:(half + 1) * 512],
                                     start=(k == 0), stop=(k == 3))
                osb = convp.tile([128, 512], fp32, tag="osb", name="osb", bufs=2)
                nc.vector.tensor_copy(osb[:], ps[:])
                dma(outT[m * 128:(m + 1) * 128, half * 512:(half + 1) * 512], osb[:])

    nc.compile()
    return nc


def _prep_inputs(inputs):
    f32 = np.float32
    hs = np.asarray(inputs['hidden_states'], f32)
    maps = []
    tri = np.tril(np.ones((C, C), f32))
    maskM = (1.0 - tri).astype(BF)
    maskG = (1.0 - tri + np.eye(C, dtype=f32)).astype(BF)
    repl = np.zeros((NG, DK), f32)
    for n in range(NG):
        repl[n, n * GG:(n + 1) * GG] = 1.0
    sel8 = np.zeros((NG, NG * 128), f32)
    for n in range(NG):
        sel8[n, n * 128:(n + 1) * 128] = 1.0
    oh8 = np.zeros((DK, 64), f32)
    for i in range(8):
        oh8[:, i * 8 + i] = 1.0
    ident = np.eye(128, dtype=f32)
    for c in range(8):
        b, hg = c // 4, c % 4
        cols = slice(hg * NH * DK, (hg + 1) * NH * DK)
        gcols = slice(hg * NH * NG, (hg + 1) * NH * NG)
        hcols = slice(hg * NH, (hg + 1) * NH)
        nega = -np.exp(np.repeat(np.asarray(inputs['A_log'], f32)[hcols], NG))
        m = {
            'hT': np.ascontiguousarray(hs[b].T).astype(BF),
            'wq': np.asarray(inputs['Wq'], f32)[:, cols].astype(BF),
            'wk': np.asarray(inputs['Wk'], f32)[:, cols].astype(BF),
            'wv': np.asarray(inputs['Wv'], f32)[:, cols].astype(BF),
            'wg': np.asarray(inputs['Wg'], f32)[:, cols].astype(BF),
            'wo': np.asarray(inputs['Wo'], f32)[cols, :].astype(BF),
            'wf1': np.asarray(inputs['Wf1'], f32).astype(BF),
            'wf2': np.asarray(inputs['Wf2'], f32)[:, gcols].astype(BF),
            'wb': np.asarray(inputs['Wb'], f32)[:, hcols].astype(BF),
            'cw': np.ascontiguousarray(np.concatenate(
                [np.asarray(inputs['conv_q'], f32)[cols],
                 np.asarray(inputs['conv_k'], f32)[cols],
                 np.asarray(inputs['conv_v'], f32)[cols]], 1)),
            'nega': np.ascontiguousarray(nega[:, None]).astype(f32),
            'dtb': np.ascontiguousarray(
                np.asarray(inputs['dt_bias'], f32)[gcols][:, None]),
            'bgc': np.ascontiguousarray(
                np.asarray(inputs['bg'], f32)[cols].reshape(NH, DV).T),
            'normw': np.ascontiguousarray(
                np.asarray(inputs['norm_w'], f32)[:, None]),
            'repl': repl,
            'self8f': sel8,
            'sel8b': sel8.astype(BF),
            'gmc': np.ascontiguousarray(repl.T),
            'ones1b': np.ones((1, C), f32).astype(BF),
            'onescol': np.ones((DK, 1), f32).astype(BF),
            'oh8': oh8.astype(BF),
            'sc8': np.array([[1.0 / SCALE ** 2]] * 4 + [[1.0]] * 4, f32),
            'eps8': np.array([[1e-6 / SCALE ** 2]] * 4 + [[1e-6]] * 4, f32),
            'epsn': np.array([[EPS]], f32),
            'maskM': maskM,
            'maskG': maskG,
            'idbf': ident.astype(BF),
            'idf32': ident,
        }
        maps.append(m)
    return maps


def kernel(**inputs):
    from concourse.bass_utils import run_bass_kernel_spmd
    if 'nc' not in _CACHE:
        _CACHE['nc'] = _build()
    nc = _CACHE['nc']
    maps = _prep_inputs(inputs)
    res = run_bass_kernel_spmd(nc, maps, list(range(8))).results
    out = np.zeros((B, T, D), np.float32)
    for c in range(8):
        out[c // 4] += res[c]['outT'].T.astype(np.float32)
    return out



# revision 17
# speedup vs baseline: 1.5606x; 1.5606x over previous
"""Grouped gated DeltaNet (KDA-style) on 8 TRN2 NeuronCores.

Sharding: core c -> (batch b = c//4, head-group hg = c%4 of 4 heads).
Per core: column-sharded projections (weights resident, loaded once),
short-conv+silu, l2norm, chunked gated delta-rule recurrence (chunk
C=128, group decay via 1-partition f32r broadcast matmuls + fused
sub/clamp, 16-partition group correlation matmuls, transpose-free A/B
doubling with interleaved triangular-solve application), deferred gated
RMSNorm (batched over T), row-shard output projection. Host sums 4
partials per batch.

Self-contained: B=2, T=1024, D=2048, H=16, DK=DV=128 hardcoded.
"""
import sys
sys.path.insert(0, '/opt/trn_rl_repo')
import numpy as np
import ml_dtypes
from contextlib import ExitStack

B, T, D = 2, 1024, 2048
H, DK, DV, GG = 16, 128, 128, 16
NG = DK // GG          # 8 gate groups per head
NH = 4                 # heads per core
C = 128                # chunk length
NCH = T // C
SCALE = DK ** -0.5
EPS = 1e-5

# packf fp32 column offsets
PF_CW = 0        # 4 blocks x 12
PF_BG = 48
PF_NW = 52
PF_NEGA = 53     # [8,4] (n,h)
PF_DTB = 57      # [8,4]
PF_SC8 = 61
PF_EPS8 = 62
PF_EPSN = 63
PF_REPL = 64     # [8,128]
PF_IDF = 192     # [128,128]
PF_SEL = 320     # [8,1024] group-selector
PF_GMC = 1344    # [128,8] group row-mask cols
NF = 1352
# packb bf16 column offsets
PB_OH8 = 0       # [128,64]
PB_S8B = 64      # [8,1024]
PB_MM = 1088     # [128,128]
PB_MG = 1216
PB_IDB = 1344
PB_OCT = 1472    # [128,1]
PB_O1B = 1473    # [1,128]
NB = 1601

WQ0, WK0, WV0, WG0, WF10, WB0 = 0, 512, 1024, 1536, 2048, 2176
WALLC = 2180

BF = ml_dtypes.bfloat16
_CACHE = {}

FP32_CHAIN = False   # fp32 x-chain fallback (precision)


def _build():
    import concourse.tile as tile
    from concourse import bacc, mybir

    fp32 = mybir.dt.float32
    f32r = mybir.dt.float32r
    bf16 = mybir.dt.bfloat16
    Alu = mybir.AluOpType
    Act = mybir.ActivationFunctionType

    nc = bacc.Bacc("TRN2", target_bir_lowering=False, debug=False, num_devices=8)
    dp = lambda n, sh, dt: nc.dram_tensor(n, sh, dt, kind="ExternalInput").ap()
    hT = dp("hT", [D, T], bf16)
    wall = dp("wall", [D, WALLC], bf16)
    wo = dp("wo", [NH * DV, D], bf16)
    wf2 = dp("wf2", [DV, NH * NG], bf16)
    packf = dp("packf", [128, NF], fp32)
    packb = dp("packb", [128, NB], bf16)
    outT = nc.dram_tensor("outT", [D, T], fp32, kind="ExternalOutput").ap()

    with tile.TileContext(nc) as tc, ExitStack() as ctx:
        pool = lambda name, bufs, space="SBUF": ctx.enter_context(
            tc.tile_pool(name=name, bufs=bufs, space=space))

        cons = pool("cons", 1)
        pers = pool("pers", 1)
        stp = pool("st", 1)

        dma = nc.sync.dma_start

        pf = cons.tile([128, NF], fp32, tag="packf", name="packf")
        dma(pf[:], packf[:])
        pb = cons.tile([128, NB], bf16, tag="packb", name="packb")
        dma(pb[:], packb[:])
        wf2t = cons.tile([128, 32], bf16, tag="wf2t", name="wf2t")
        dma(wf2t[:], wf2[:])

        cwt = lambda m: pf[:, PF_CW + m * 12: PF_CW + (m + 1) * 12]
        bgt = pf[:, PF_BG:PF_BG + 4]
        nwt = pf[:, PF_NW:PF_NW + 1]
        negat8 = lambda h: pf[0:8, PF_NEGA + h:PF_NEGA + h + 1]
        dtbt = pf[0:8, PF_DTB:PF_DTB + 4]
        sc8t = pf[0:8, PF_SC8:PF_SC8 + 1]
        eps8t = pf[0:8, PF_EPS8:PF_EPS8 + 1]
        epsnt = pf[0:1, PF_EPSN:PF_EPSN + 1]
        replt = pf[0:8, PF_REPL:PF_REPL + 128]
        idf = pf[:, PF_IDF:PF_IDF + 128]
        sel8 = pf[0:8, PF_SEL:PF_SEL + 1024]
        oh8t = pb[:, PB_OH8:PB_OH8 + 64]
        s8b = pb[0:8, PB_S8B:PB_S8B + 1024]
        mMt = pb[:, PB_MM:PB_MM + 128]
        mGt = pb[:, PB_MG:PB_MG + 128]
        idb = pb[:, PB_IDB:PB_IDB + 128]
        octb = pb[:, PB_OCT:PB_OCT + 1]
        o1b = pb[0:1, PB_O1B:PB_O1B + 128]
        gmct = pf[:, PF_GMC:PF_GMC + 8]

        ones32 = cons.tile([32, C], fp32, tag="ones32", name="ones32")
        nc.vector.memset(ones32[:], 1.0)

        # ---- persistent activations ----
        mk = lambda nm: [pers.tile([128, T], bf16, tag=f"{nm}{m}", name=f"{nm}{m}")
                         for m in range(4)]
        qb, kb, vb = mk("qb"), mk("kb"), mk("vb")
        gateb = mk("gateb")
        f1b = pers.tile([128, T], bf16, tag="f1b", name="f1b")
        gna8 = [pers.tile([8, T], bf16, tag=f"gna{h}", name=f"gna{h}")
                for h in range(4)]
        bsg = pers.tile([4, T], fp32, tag="bsg", name="bsg")

        # ---- state tiles (parity pairs) ----
        Sf = [[stp.tile([128, DV], fp32, tag=f"Sf{h}_{p}", name=f"Sf{h}_{p}")
               for p in range(2)] for h in range(4)]
        Sb = [[stp.tile([128, DV], bf16, tag=f"Sb{h}_{p}", name=f"Sb{h}_{p}")
               for p in range(2)] for h in range(4)]
        for h in range(4):
            nc.vector.memset(Sf[h][0][:], 0.0)
            nc.vector.memset(Sb[h][0][:], 0.0)

        # ================= projections =================
        with tc.tile_pool(name="htp", bufs=1) as htp, \
             tc.tile_pool(name="wallp", bufs=1) as wallp, \
             tc.tile_pool(name="convp", bufs=2) as convp, \
             tc.tile_pool(name="sqp", bufs=2) as sqp, \
             tc.tile_pool(name="smt", bufs=2) as smt, \
             tc.tile_pool(name="pps", bufs=1, space="PSUM") as pr:
            # PSUM tags: proj(2) sqs(1) bps(1) gps(1) nb(2) = 7 banks
            ssqsb = smt.tile([8, T], fp32, tag="ssqsb", name="ssqsb", bufs=1)
            nc.vector.memset(ssqsb[:], 0.0)
            ht, wt = [], []
            for k in range(16):
                t = htp.tile([128, T], bf16, tag=f"ht{k}", name=f"ht{k}")
                dma(t[:], hT[k * 128:(k + 1) * 128, :])
                ht.append(t)
                w = wallp.tile([128, WALLC], bf16, tag=f"wl{k}", name=f"wl{k}")
                dma(w[:], wall[k * 128:(k + 1) * 128, :])
                wt.append(w)

            qs = {}

            def project(col0, m, dst_bf16=None, conv_slot=None, pair=None,
                        gate_bias=None):
                xpad = None
                if conv_slot is not None:
                    xpad = convp.tile([128, T + 3], fp32, tag="xpad", name="xpad")
                    nc.vector.memset(xpad[:, 0:3], 0.0)
                for half in range(2):
                    ps = pr.tile([128, 512], fp32, tag="proj", name="projps", bufs=2)
                    for k in range(16):
                        nc.tensor.matmul(ps[:], wt[k][:, col0 + m * 128:col0 + (m + 1) * 128],
                                         ht[k][:, half * 512:(half + 1) * 512],
                                         start=(k == 0), stop=(k == 15))
                    if xpad is not None:
                        nc.scalar.copy(xpad[:, 3 + half * 512: 3 + (half + 1) * 512], ps[:])
                    elif gate_bias is not None:
                        nc.scalar.activation(dst_bf16[:, half * 512:(half + 1) * 512],
                                             ps[:], Act.Silu, bias=gate_bias)
                    else:
                        nc.scalar.copy(dst_bf16[:, half * 512:(half + 1) * 512], ps[:])
                if xpad is None:
                    return
                cwm = cwt(m)
                s = conv_slot * 4
                a = convp.tile([128, T], fp32, tag="acca", name="acca", bufs=1)
                bt = convp.tile([128, T], fp32, tag="accb", name="accb", bufs=1)
                nc.vector.tensor_scalar(a[:], xpad[:, 3:3 + T], cwm[:, s + 3:s + 4],
                                        None, op0=Alu.mult)
                cur, nxt = a, bt
                for kk in (2, 1, 0):
                    nc.vector.scalar_tensor_tensor(nxt[:], xpad[:, kk:kk + T],
                                                   cwm[:, s + kk:s + kk + 1], cur[:],
                                                   op0=Alu.mult, op1=Alu.add)
                    cur, nxt = nxt, cur
                if pair is None:
                    nc.scalar.activation(dst_bf16[:], cur[:], Act.Silu)
                else:
                    qsil = qb[pair] if pair < 4 else kb[pair - 4]
                    qs[pair] = qsil
                    nc.scalar.activation(qsil[:], cur[:], Act.Silu)
                    sq = sqp.tile([128, T], bf16, tag="sq", name="sq", bufs=1)
                    nc.scalar.activation(sq[:], qsil[:], Act.Square)
                    for half in range(2):
                        pss = pr.tile([8, 512], fp32, tag="sqs", name="sqs")
                        nc.tensor.matmul(pss[:], oh8t[:, pair * 8:pair * 8 + 8],
                                         sq[:, half * 512:(half + 1) * 512],
                                         start=True, stop=True)
                        nc.vector.tensor_tensor(ssqsb[:, half * 512:(half + 1) * 512],
                                                ssqsb[:, half * 512:(half + 1) * 512],
                                                pss[:], op=Alu.add)

            for m in range(4):
                project(WQ0, m, conv_slot=0, pair=m)
            for m in range(4):
                project(WK0, m, conv_slot=1, pair=4 + m)
            for m in range(4):
                project(WV0, m, dst_bf16=vb[m], conv_slot=2)
            for m in range(4):
                project(WG0, m, dst_bf16=gateb[m], gate_bias=bgt[:, m:m + 1])

            # f1 projection
            for half in range(2):
                ps = pr.tile([128, 512], fp32, tag="proj", name="f1ps", bufs=2)
                for k in range(16):
                    nc.tensor.matmul(ps[:], wt[k][:, WF10:WF10 + 128],
                                     ht[k][:, half * 512:(half + 1) * 512],
                                     start=(k == 0), stop=(k == 15))
                nc.scalar.copy(f1b[:, half * 512:(half + 1) * 512], ps[:])

            # beta (sigmoid) then gate-softplus chain, table-load friendly order
            for half in range(2):
                bps = pr.tile([4, 512], fp32, tag="bps", name="bps")
                for k in range(16):
                    nc.tensor.matmul(bps[:], wt[k][:, WB0:WB0 + 4],
                                     ht[k][:, half * 512:(half + 1) * 512],
                                     start=(k == 0), stop=(k == 15))
                nc.scalar.activation(bsg[:, half * 512:(half + 1) * 512], bps[:],
                                     Act.Sigmoid)
            sp1s = []
            for half in range(2):
                for h in range(4):
                    gps = pr.tile([8, 512], fp32, tag="gps", name="gps", bufs=2)
                    nc.tensor.matmul(gps[:], wf2t[:, h * 8:(h + 1) * 8],
                                     f1b[:, half * 512:(half + 1) * 512],
                                     start=True, stop=True)
                    spe = smt.tile([8, 512], bf16, tag="spe", name="spe", bufs=2)
                    nc.scalar.activation(spe[:], gps[:], Act.Exp,
                                         bias=dtbt[:, h:h + 1])
                    sp1 = smt.tile([8, 512], bf16, tag="sp1", name="sp1", bufs=8)
                    nc.vector.tensor_scalar(sp1[:], spe[:], 1.0, None, op0=Alu.add)
                    sp1s.append((half, h, sp1))
            # all Ln together: l2 normalizer + softplus logs
            nrm = smt.tile([8, T], fp32, tag="nrm", name="nrm", bufs=1)
            nc.scalar.activation(nrm[:], ssqsb[:], Act.Ln, scale=sc8t[:, 0:1],
                                 bias=eps8t[:, 0:1])
            for half, h, sp1 in sp1s:
                sp = smt.tile([8, 512], bf16, tag="sp", name="sp", bufs=2)
                nc.scalar.activation(sp[:], sp1[:], Act.Ln)
                nc.vector.tensor_scalar(gna8[h][:, half * 512:(half + 1) * 512],
                                        sp[:], negat8(h), None, op0=Alu.mult)
            recb = smt.tile([8, T], bf16, tag="recb", name="recb", bufs=1)
            nc.scalar.activation(recb[:], nrm[:], Act.Exp, scale=-0.5)
            for pair in range(8):
                dst = qb[pair] if pair < 4 else kb[pair - 4]
                for half in range(2):
                    nb = pr.tile([128, 512], fp32, tag="nb", name="nb", bufs=2)
                    nc.tensor.matmul(nb[:], s8b[:, pair * 128:(pair + 1) * 128],
                                     recb[:, half * 512:(half + 1) * 512],
                                     start=True, stop=True)
                    nc.vector.tensor_tensor(dst[:, half * 512:(half + 1) * 512],
                                            qs[pair][:, half * 512:(half + 1) * 512],
                                            nb[:], op=Alu.mult)

        # weights/hT pools closed; load wo for the tail now (overlaps recurrence)
        pers2 = ctx.enter_context(tc.tile_pool(name="pers2", bufs=1))
        yb = [pers2.tile([128, T], bf16, tag=f"yb{m}", name=f"yb{m}")
              for m in range(4)]
        wotp = ctx.enter_context(tc.tile_pool(name="wotp", bufs=1))
        wot = [wotp.tile([128, D], bf16, tag=f"wo{k}", name=f"wo{k}") for k in range(4)]
        for k in range(4):
            dma(wot[k][:], wo[k * 128:(k + 1) * 128, :])

        # ================= recurrence =================
        rc = ctx.enter_context(tc.tile_pool(name="rc", bufs=2))
        rr = ctx.enter_context(tc.tile_pool(name="rr", bufs=4))
        rctx = ExitStack()
        prc = rctx.enter_context(tc.tile_pool(name="prc", bufs=1, space="PSUM"))
        # PSUM tags: tp(2) big(3) dblx(2) = 7 banks

        hdt = lambda nm, h, sh, dt=bf16, bufs=2: rc.tile(
            sh, dt, tag=f"{nm}{h}", name=f"{nm}{h}", bufs=bufs)

        for ci in range(NCH):
            ts = slice(ci * C, (ci + 1) * C)
            po, pn = ci % 2, (ci + 1) % 2
            # ---- chunk prologue ----
            cn8s = []
            for h in range(4):
                cn8 = hdt("cn8", h, [8, C], fp32)
                nc.vector.tensor_tensor_scan(cn8[:], ones32[0:8, :],
                                             gna8[h][:, ts], 0.0,
                                             op0=Alu.mult, op1=Alu.add)
                cn8s.append(cn8)
            cnt8s = []
            for h in range(4):
                cNtp = prc.tile([128, 8], fp32, tag="tp", name="cNtp", bufs=2)
                nc.tensor.transpose(cNtp[:], cn8s[h][:], idf[0:8, 0:8])
                cnt8 = hdt("cnt8", h, [128, 8], fp32)
                nc.scalar.copy(cnt8[:], cNtp[:])
                cnt8s.append(cnt8)
            b2p = prc.tile([128, 4], fp32, tag="tp", name="b2p", bufs=2)
            nc.tensor.transpose(b2p[:], bsg[:, ts], idf[0:4, 0:4])
            beta2 = rc.tile([128, 4], fp32, tag="beta2", name="beta2")
            nc.scalar.copy(beta2[:], b2p[:])
            # masked keys for group correlations (per-partition 0/1 scale)
            kmsks = []
            for h in range(4):
                kmsk = rr.tile([128, 8 * C], bf16, tag="kmsk", name="kmsk")
                for n in range(8):
                    dst = kmsk[:, n * C:(n + 1) * C]
                    if n % 2 == 0:
                        nc.scalar.mul(dst, kb[h][:, ts], gmct[:, n:n + 1])
                    else:
                        nc.vector.tensor_scalar(dst, kb[h][:, ts], gmct[:, n:n + 1],
                                                None, op0=Alu.mult)
                kmsks.append(kmsk)

            # ---- per-head prologue (step-major) ----
            exp8s, exp8ks = [], []
            for h in range(4):
                e8 = hdt("exp8", h, [8, C], fp32)
                nc.scalar.activation(e8[:], cn8s[h][:], Act.Exp)
                exp8s.append(e8)
            for h in range(4):
                e8k = hdt("exp8k", h, [8, C], fp32)
                nc.scalar.activation(e8k[:], cn8s[h][:], Act.Exp,
                                     scale=-1.0,
                                     bias=cn8s[h][:, C - 1:C])
                exp8ks.append(e8k)
            bfks = []
            for h in range(4):
                bfk = prc.tile([128, 256], fp32, tag="tp", name="bfk", bufs=2)
                nc.tensor.matmul(bfk[:, 0:128], replt,
                                 exp8s[h][:], start=True, stop=True)
                nc.tensor.matmul(bfk[:, 128:256], replt,
                                 exp8ks[h][:], start=True, stop=True)
                bfks.append(bfk)
            bCs, Wts, qtTs, kends = [], [], [], []
            for h in range(4):
                bC = hdt("bC", h, [128, 1], fp32)
                nc.scalar.copy(bC[:], bfks[h][:, 127:128])
                bCs.append(bC)
            for h in range(4):
                Wth = hdt("Wt", h, [128, C])
                nc.vector.tensor_tensor(Wth[:], kb[h][:, ts], bfks[h][:, 0:128],
                                        op=Alu.mult)
                Wts.append(Wth)
                qtTh = hdt("qtT", h, [128, C])
                nc.vector.tensor_tensor(qtTh[:], qb[h][:, ts], bfks[h][:, 0:128],
                                        op=Alu.mult)
                qtTs.append(qtTh)
                kendh = hdt("kend", h, [128, C])
                nc.vector.tensor_tensor(kendh[:], kb[h][:, ts], bfks[h][:, 128:256],
                                        op=Alu.mult)
                kends.append(kendh)
            # decay matrices: bca broadcast (f32r) + fused sub/clamp + one exp
            ealls = []
            for h in range(4):
                bcaL = prc.tile([128, 512], fp32, tag="big", name="bcaL", bufs=3)
                bcaH = prc.tile([128, 512], fp32, tag="big", name="bcaH", bufs=3)
                for n in range(8):
                    dst = bcaL if n < 4 else bcaH
                    nc.tensor.matmul(dst[:, (n % 4) * C:(n % 4 + 1) * C],
                                     sel8[:, n * 128:(n + 1) * 128],
                                     cn8s[h][:],
                                     start=True, stop=True)
                eallin = rr.tile([128, 8 * C], bf16, tag="eallin", name="eallin")
                for n in range(8):
                    src = bcaL if n < 4 else bcaH
                    eng = nc.vector
                    eng.tensor_scalar(eallin[:, n * C:(n + 1) * C],
                                      src[:, (n % 4) * C:(n % 4 + 1) * C],
                                      cnt8s[h][:, n:n + 1], 0.0,
                                      op0=Alu.subtract, op1=Alu.min)
                eall = rr.tile([128, 8 * C], bf16, tag="eall", name="eall")
                nc.scalar.activation(eall[:], eallin[:], Act.Exp)
                ealls.append(eall)
            # group correlation matmuls (16-partition contraction)
            A0s, GtMs = [], []
            for h in range(4):
                pls = []
                for src in (kb[h], qb[h]):
                    pl = prc.tile([128, 512], fp32, tag="big", name="pall", bufs=3)
                    ph = prc.tile([128, 512], fp32, tag="big", name="pallh", bufs=3)
                    for n in range(8):
                        dst = pl if n < 4 else ph
                        nc.tensor.matmul(dst[:, (n % 4) * C:(n % 4 + 1) * C],
                                         kmsks[h][:, n * C:(n + 1) * C],
                                         src[:, ts],
                                         start=True, stop=True)
                    pls.append((pl, ph))
                prods = []
                for x, (pl, ph) in enumerate(pls):
                    prod = rr.tile([128, 8 * C], bf16, tag="prod", name=f"prod{x}")
                    nc.vector.tensor_tensor(prod[:, 0:4 * C], ealls[h][:, 0:4 * C],
                                            pl[:], op=Alu.mult)
                    nc.vector.tensor_tensor(prod[:, 4 * C:], ealls[h][:, 4 * C:],
                                            ph[:], op=Alu.mult)
                    prods.append(prod)
                for x, prod in enumerate(prods):
                    t4 = rr.tile([128, 4 * C], bf16, tag="t4", name="t4")
                    nc.vector.tensor_tensor(t4[:], prod[:, :4 * C], prod[:, 4 * C:],
                                            op=Alu.add)
                    t2 = rr.tile([128, 2 * C], bf16, tag="t2", name="t2")
                    nc.vector.tensor_tensor(t2[:], t4[:, :2 * C], t4[:, 2 * C:],
                                            op=Alu.add)
                    t1 = rr.tile([128, C], bf16, tag="t1", name="t1")
                    nc.vector.tensor_tensor(t1[:], t2[:, :C], t2[:, C:], op=Alu.add)
                    # masked/beta-folded (Pool, SBUF-only)
                    if x == 0:
                        A0 = hdt("A0", h, [128, C])
                        nc.vector.scalar_tensor_tensor(A0[:], t1[:], beta2[:, h:h + 1],
                                                       mMt[:], op0=Alu.mult,
                                                       op1=Alu.mult)
                        A0s.append(A0)
                    else:
                        GtM = hdt("GtM", h, [128, C])
                        nc.vector.scalar_tensor_tensor(GtM[:], t1[:], beta2[:, h:h + 1],
                                                       mGt[:], op0=Alu.mult,
                                                       op1=Alu.mult)
                        GtMs.append(GtM)
            # vt / kts
            vts, ktss = [], []
            for h in range(4):
                vtp = prc.tile([128, C], bf16, tag="tp", name="vtp", bufs=2)
                nc.tensor.transpose(vtp[:], vb[h][:, ts], idb[:])
                vt = hdt("vt", h, [128, C])
                nc.scalar.copy(vt[:], vtp[:])
                vts.append(vt)
                ktp = prc.tile([128, C], bf16, tag="tp", name="ktp", bufs=2)
                nc.tensor.transpose(ktp[:], kends[h][:], idb[:])
                kts = hdt("kts", h, [128, C])
                nc.vector.tensor_scalar(kts[:], ktp[:], beta2[:, h:h + 1], None,
                                        op0=Alu.mult)
                ktss.append(kts)
            # B0 transpose
            Bs = [[None] * 6 for _ in range(4)]
            As = [[None] * 7 for _ in range(4)]
            for h in range(4):
                As[h][0] = A0s[h]
                b0p = prc.tile([128, C], bf16, tag="tp", name="b0p", bufs=2)
                nc.tensor.transpose(b0p[:], A0s[h][:], idb[:])
                B0 = rc.tile([128, C], bf16, tag=f"B{h}", name=f"B{h}_0", bufs=3)
                nc.scalar.copy(B0[:], b0p[:])
                Bs[h][0] = B0
            # ---- spine: ws0 / x-chain with interleaved doubling ----
            xbs = []
            for h in range(4):
                ws0 = prc.tile([128, C], fp32, tag="tp", name="ws0", bufs=2)
                nc.tensor.matmul(ws0[:], Wts[h][:], Sb[h][po][:], start=True, stop=True)
                xb = rc.tile([128, C], bf16, tag=f"xb{h}", name=f"xb{h}", bufs=3)
                nc.vector.tensor_tensor(xb[:], vts[h][:], ws0[:], op=Alu.subtract)
                xbs.append(xb)
            # apply level 0 (subtract), then levels 1..6 with doubling
            for lev in range(7):
                if lev >= 1:
                    for h in range(4):
                        dbl = prc.tile([128, 256], fp32, tag="dblx", name="dbl", bufs=2)
                        nc.tensor.matmul(dbl[:, 0:128], Bs[h][lev - 1][:],
                                         As[h][lev - 1][:], start=True, stop=True)
                        if lev < 6:
                            nc.tensor.matmul(dbl[:, 128:256], As[h][lev - 1][:],
                                             Bs[h][lev - 1][:], start=True, stop=True)
                        An = rc.tile([128, C], bf16, tag=f"A{h}", name=f"A{h}_{lev}",
                                     bufs=3)
                        nc.scalar.copy(An[:], dbl[:, 0:128])
                        As[h][lev] = An
                        if lev < 6:
                            Bn = rc.tile([128, C], bf16, tag=f"B{h}",
                                         name=f"B{h}_{lev}", bufs=3)
                            nc.vector.tensor_copy(Bn[:], dbl[:, 128:256])
                            Bs[h][lev] = Bn
                for h in range(4):
                    mx = prc.tile([128, C], fp32, tag="dblx", name="mx", bufs=2)
                    nc.tensor.matmul(mx[:], As[h][lev][:], xbs[h][:],
                                     start=True, stop=True)
                    xn = rc.tile([128, C], bf16, tag=f"xb{h}", name=f"xb{h}_{lev}",
                                 bufs=3)
                    nc.vector.tensor_tensor(xn[:], xbs[h][:], mx[:],
                                            op=(Alu.subtract if lev == 0 else Alu.add))
                    xbs[h] = xn
            # state update + output
            for h in range(4):
                sup = prc.tile([128, DV], fp32, tag="tp", name="sup", bufs=2)
                nc.tensor.matmul(sup[:], ktss[h][:], xbs[h][:], start=True, stop=True)
                nc.vector.scalar_tensor_tensor(Sf[h][pn][:], Sf[h][po][:],
                                               bCs[h][:, 0:1], sup[:],
                                               op0=Alu.mult, op1=Alu.add)
                nc.vector.scalar_tensor_tensor(Sb[h][pn][:], Sf[h][po][:],
                                               bCs[h][:, 0:1], sup[:],
                                               op0=Alu.mult, op1=Alu.add)
                otp = prc.tile([128, C], fp32, tag="tp", name="otp", bufs=2)
                nc.tensor.matmul(otp[:], Sb[h][po][:], qtTs[h][:],
                                 start=True, stop=False)
                nc.tensor.matmul(otp[:], xbs[h][:], GtMs[h][:],
                                 start=False, stop=True)
                nc.vector.tensor_tensor(yb[h][:, ts], gateb[h][:, ts], otp[:],
                                        op=Alu.mult)

        rctx.close()
        # ================= deferred RMSNorm + output projection =================
        with tc.tile_pool(name="post", bufs=2) as post, \
             tc.tile_pool(name="ppc", bufs=2, space="PSUM") as ppc:
            # PSUM tags: ssp(2) rbc(2) proj(2) = 6 banks
            ysqs = []
            for h in range(4):
                ysq = post.tile([128, T], bf16, tag="ysq", name="ysq", bufs=4)
                nc.scalar.activation(ysq[:], yb[h][:], Act.Square)
                ysqs.append(ysq)
            nrcs = []
            for h in range(4):
                nrc = post.tile([1, T], fp32, tag="nrc", name="nrc", bufs=4)
                for half in range(2):
                    ssp = ppc.tile([1, 512], fp32, tag="ssp", name="ssp")
                    nc.tensor.matmul(ssp[:], octb[:],
                                     ysqs[h][:, half * 512:(half + 1) * 512],
                                     start=True, stop=True)
                    nc.scalar.activation(nrc[:, half * 512:(half + 1) * 512],
                                         ssp[:], Act.Ln, scale=1.0 / DV,
                                         bias=epsnt[:, 0:1])
                nrcs.append(nrc)
            for h in range(4):
                rcb = post.tile([1, T], bf16, tag="rcb", name="rcb", bufs=4)
                nc.scalar.activation(rcb[:], nrcs[h][:], Act.Exp, scale=-0.5)
                for half in range(2):
                    rbc = ppc.tile([128, 512], fp32, tag="rbc", name="rbc")
                    nc.tensor.matmul(rbc[:], o1b[:], rcb[:, half * 512:(half + 1) * 512],
                                     start=True, stop=True)
                    nc.vector.scalar_tensor_tensor(yb[h][:, half * 512:(half + 1) * 512],
                                                   yb[h][:, half * 512:(half + 1) * 512],
                                                   nwt[:, 0:1], rbc[:],
                                                   op0=Alu.mult, op1=Alu.mult)
            # output projection
            for m in range(16):
                osb = post.tile([128, T], fp32, tag="osb", name="osb")
                for half in range(2):
                    ps = ppc.tile([128, 512], fp32, tag="proj", name="ops")
                    for k in range(4):
                        nc.tensor.matmul(ps[:], wot[k][:, m * 128:(m + 1) * 128],
                                         yb[k][:, half * 512:(half + 1) * 512],
                                         start=(k == 0), stop=(k == 3))
                    if half == 0:
                        nc.vector.tensor_copy(osb[:, 0:512], ps[:])
                    else:
                        nc.scalar.copy(osb[:, 512:1024], ps[:])
                dma(outT[m * 128:(m + 1) * 128, :], osb[:])

    nc.compile()
    return nc


def _prep_inputs(inputs):
    f32 = np.float32
    hs = np.asarray(inputs['hidden_states'], f32)
    tri = np.tril(np.ones((C, C), f32))
    maskM = (1.0 - tri).astype(f32)
    maskG = (1.0 - tri + np.eye(C, dtype=f32)).astype(f32)
    repl = np.zeros((NG, DK), f32)
    for n in range(NG):
        repl[n, n * GG:(n + 1) * GG] = 1.0
    sel8 = np.zeros((NG, NG * 128), f32)
    for n in range(NG):
        sel8[n, n * 128:(n + 1) * 128] = 1.0
    oh8 = np.zeros((DK, 64), f32)
    for i in range(8):
        oh8[:, i * 8 + i] = 1.0
    ident = np.eye(128, dtype=f32)

    maps = []
    for c in range(8):
        b, hg = c // 4, c % 4
        cols = slice(hg * NH * DK, (hg + 1) * NH * DK)
        gcols = slice(hg * NH * NG, (hg + 1) * NH * NG)
        hcols = slice(hg * NH, (hg + 1) * NH)
        nega = -np.exp(np.repeat(np.asarray(inputs['A_log'], f32)[hcols], NG))

        packf = np.zeros((128, NF), f32)
        cw = np.concatenate(
            [np.asarray(inputs['conv_q'], f32)[cols],
             np.asarray(inputs['conv_k'], f32)[cols],
             np.asarray(inputs['conv_v'], f32)[cols]], 1)  # [512, 12]
        for m in range(4):
            packf[:, PF_CW + m * 12:PF_CW + (m + 1) * 12] = cw[m * 128:(m + 1) * 128]
        packf[:, PF_BG:PF_BG + 4] = np.asarray(inputs['bg'], f32)[cols].reshape(NH, DV).T
        packf[:, PF_NW] = np.asarray(inputs['norm_w'], f32)
        packf[0:8, PF_NEGA:PF_NEGA + 4] = nega.reshape(NH, NG).T
        packf[0:8, PF_DTB:PF_DTB + 4] = (
            np.asarray(inputs['dt_bias'], f32)[gcols].reshape(NH, NG).T)
        packf[0:8, PF_SC8] = [1.0 / SCALE ** 2] * 4 + [1.0] * 4
        packf[0:8, PF_EPS8] = [1e-6 / SCALE ** 2] * 4 + [1e-6] * 4
        packf[0:1, PF_EPSN] = EPS
        packf[0:8, PF_REPL:PF_REPL + 128] = repl
        packf[:, PF_IDF:PF_IDF + 128] = ident
        packf[0:8, PF_SEL:PF_SEL + 1024] = sel8
        packf[:, PF_GMC:PF_GMC + 8] = repl.T

        packb = np.zeros((128, NB), f32)
        packb[:, PB_OH8:PB_OH8 + 64] = oh8
        packb[0:8, PB_S8B:PB_S8B + 1024] = sel8
        packb[:, PB_MM:PB_MM + 128] = maskM
        packb[:, PB_MG:PB_MG + 128] = maskG
        packb[:, PB_IDB:PB_IDB + 128] = ident
        packb[:, PB_OCT] = 1.0
        packb[0:1, PB_O1B:PB_O1B + 128] = 1.0

        wallm = np.concatenate(
            [np.asarray(inputs['Wq'], f32)[:, cols],
             np.asarray(inputs['Wk'], f32)[:, cols],
             np.asarray(inputs['Wv'], f32)[:, cols],
             np.asarray(inputs['Wg'], f32)[:, cols],
             np.asarray(inputs['Wf1'], f32),
             np.asarray(inputs['Wb'], f32)[:, hcols]], 1)

        m = {
            'hT': np.ascontiguousarray(hs[b].T).astype(BF),
            'wall': np.ascontiguousarray(wallm).astype(BF),
            'wo': np.ascontiguousarray(np.asarray(inputs['Wo'], f32)[cols, :]).astype(BF),
            'wf2': np.ascontiguousarray(np.asarray(inputs['Wf2'], f32)[:, gcols]).astype(BF),
            'packf': packf,
            'packb': packb.astype(BF),
        }
        maps.append(m)
    return maps


def kernel(**inputs):
    from concourse.bass_utils import run_bass_kernel_spmd
    if 'nc' not in _CACHE:
        _CACHE['nc'] = _build()
    nc = _CACHE['nc']
    maps = _prep_inputs(inputs)
    res = run_bass_kernel_spmd(nc, maps, list(range(8))).results
    out = np.zeros((B, T, D), np.float32)
    for c in range(8):
        out[c // 4] += res[c]['outT'].T.astype(np.float32)
    return out


# revision 19
# speedup vs baseline: 1.6014x; 1.0261x over previous
"""Grouped gated DeltaNet (KDA-style) on 8 TRN2 NeuronCores.

Sharding: core c -> (batch b = c//4, head-group hg = c%4 of 4 heads).
Per core: column-sharded projections (weights resident, loaded once),
short-conv+silu, l2norm, chunked gated delta-rule recurrence (chunk
C=128, group decay via 1-partition f32r broadcast matmuls + fused
sub/clamp, 16-partition group correlation matmuls, transpose-free A/B
doubling with interleaved triangular-solve application), deferred gated
RMSNorm (batched over T), row-shard output projection. Host sums 4
partials per batch.

Self-contained: B=2, T=1024, D=2048, H=16, DK=DV=128 hardcoded.
"""
import sys
sys.path.insert(0, '/opt/trn_rl_repo')
import numpy as np
import ml_dtypes
from contextlib import ExitStack

B, T, D = 2, 1024, 2048
H, DK, DV, GG = 16, 128, 128, 16
NG = DK // GG          # 8 gate groups per head
NH = 4                 # heads per core
C = 128                # chunk length
NCH = T // C
SCALE = DK ** -0.5
EPS = 1e-5

# packf fp32 column offsets
PF_CW = 0        # 4 blocks x 12
PF_BG = 48
PF_NW = 52
PF_NEGA = 53     # [8,4] (n,h)
PF_DTB = 57      # [8,4]
PF_SC8 = 61
PF_EPS8 = 62
PF_EPSN = 63
PF_REPL = 64     # [8,128]
PF_IDF = 192     # [128,128]
PF_SEL = 320     # [8,1024] group-selector
PF_GMC = 1344    # [128,8] group row-mask cols
NF = 1352
# packb bf16 column offsets
PB_OH8 = 0       # [128,64]
PB_S8B = 64      # [8,1024]
PB_MM = 1088     # [128,128]
PB_MG = 1216
PB_IDB = 1344
PB_OCT = 1472    # [128,1]
PB_O1B = 1473    # [1,128]
NB = 1601

WQ0, WK0, WV0, WG0, WF10, WB0 = 0, 512, 1024, 1536, 2048, 2176
WALLC = 2180

BF = ml_dtypes.bfloat16
_CACHE = {}

FP32_CHAIN = False   # fp32 x-chain fallback (precision)


def _build():
    import concourse.tile as tile
    from concourse import bacc, mybir

    fp32 = mybir.dt.float32
    f32r = mybir.dt.float32r
    bf16 = mybir.dt.bfloat16
    Alu = mybir.AluOpType
    Act = mybir.ActivationFunctionType

    nc = bacc.Bacc("TRN2", target_bir_lowering=False, debug=False, num_devices=8)
    dp = lambda n, sh, dt: nc.dram_tensor(n, sh, dt, kind="ExternalInput").ap()
    hT = dp("hT", [D, T], bf16)
    wall = dp("wall", [D, WALLC], bf16)
    wo = dp("wo", [NH * DV, D], bf16)
    wf2 = dp("wf2", [DV, NH * NG], bf16)
    packf = dp("packf", [128, NF], fp32)
    packb = dp("packb", [128, NB], bf16)
    outT = nc.dram_tensor("outT", [D, T], fp32, kind="ExternalOutput").ap()

    with tile.TileContext(nc) as tc, ExitStack() as ctx:
        pool = lambda name, bufs, space="SBUF": ctx.enter_context(
            tc.tile_pool(name=name, bufs=bufs, space=space))

        cons = pool("cons", 1)
        pers = pool("pers", 1)
        stp = pool("st", 1)

        dma = nc.sync.dma_start

        pf = cons.tile([128, NF], fp32, tag="packf", name="packf")
        dma(pf[:], packf[:])
        pb = cons.tile([128, NB], bf16, tag="packb", name="packb")
        dma(pb[:], packb[:])
        wf2t = cons.tile([128, 32], bf16, tag="wf2t", name="wf2t")
        dma(wf2t[:], wf2[:])

        cwt = lambda m: pf[:, PF_CW + m * 12: PF_CW + (m + 1) * 12]
        bgt = pf[:, PF_BG:PF_BG + 4]
        nwt = pf[:, PF_NW:PF_NW + 1]
        negat8 = lambda h: pf[0:8, PF_NEGA + h:PF_NEGA + h + 1]
        dtbt = pf[0:8, PF_DTB:PF_DTB + 4]
        sc8t = pf[0:8, PF_SC8:PF_SC8 + 1]
        eps8t = pf[0:8, PF_EPS8:PF_EPS8 + 1]
        epsnt = pf[0:1, PF_EPSN:PF_EPSN + 1]
        replt = pf[0:8, PF_REPL:PF_REPL + 128]
        idf = pf[:, PF_IDF:PF_IDF + 128]
        sel8 = pf[0:8, PF_SEL:PF_SEL + 1024]
        oh8t = pb[:, PB_OH8:PB_OH8 + 64]
        s8b = pb[0:8, PB_S8B:PB_S8B + 1024]
        mMt = pb[:, PB_MM:PB_MM + 128]
        mGt = pb[:, PB_MG:PB_MG + 128]
        idb = pb[:, PB_IDB:PB_IDB + 128]
        octb = pb[:, PB_OCT:PB_OCT + 1]
        o1b = pb[0:1, PB_O1B:PB_O1B + 128]
        gmct = pf[:, PF_GMC:PF_GMC + 8]

        ones32 = cons.tile([32, C], fp32, tag="ones32", name="ones32")
        nc.vector.memset(ones32[:], 1.0)

        # ---- persistent activations ----
        mk = lambda nm: [pers.tile([128, T], bf16, tag=f"{nm}{m}", name=f"{nm}{m}")
                         for m in range(4)]
        qb, kb, vb = mk("qb"), mk("kb"), mk("vb")
        gateb = mk("gateb")
        f1b = pers.tile([128, T], bf16, tag="f1b", name="f1b")
        gna8 = [pers.tile([8, T], bf16, tag=f"gna{h}", name=f"gna{h}")
                for h in range(4)]
        bsg = pers.tile([4, T], fp32, tag="bsg", name="bsg")

        # ---- state tiles (parity pairs) ----
        Sf = [[stp.tile([128, DV], fp32, tag=f"Sf{h}_{p}", name=f"Sf{h}_{p}")
               for p in range(2)] for h in range(4)]
        Sb = [[stp.tile([128, DV], bf16, tag=f"Sb{h}_{p}", name=f"Sb{h}_{p}")
               for p in range(2)] for h in range(4)]
        for h in range(4):
            nc.vector.memset(Sf[h][0][:], 0.0)
            nc.vector.memset(Sb[h][0][:], 0.0)

        # ================= projections =================
        with tc.tile_pool(name="htp", bufs=1) as htp, \
             tc.tile_pool(name="wallp", bufs=1) as wallp, \
             tc.tile_pool(name="convp", bufs=2) as convp, \
             tc.tile_pool(name="sqp", bufs=2) as sqp, \
             tc.tile_pool(name="smt", bufs=2) as smt, \
             tc.tile_pool(name="pps", bufs=1, space="PSUM") as pr:
            # PSUM tags: proj(2) sqs(1) bps(1) gps(1) nb(2) = 7 banks
            ssqsb = smt.tile([8, T], fp32, tag="ssqsb", name="ssqsb", bufs=1)
            nc.vector.memset(ssqsb[:], 0.0)
            ht, wt = [], []
            for k in range(16):
                t = htp.tile([128, T], bf16, tag=f"ht{k}", name=f"ht{k}")
                dma(t[:], hT[k * 128:(k + 1) * 128, :])
                ht.append(t)
                w = wallp.tile([128, WALLC], bf16, tag=f"wl{k}", name=f"wl{k}")
                dma(w[:], wall[k * 128:(k + 1) * 128, :])
                wt.append(w)

            qs = {}

            def project(col0, m, dst_bf16=None, conv_slot=None, pair=None,
                        gate_bias=None):
                xpad = None
                if conv_slot is not None:
                    xpad = convp.tile([128, T + 3], fp32, tag="xpad", name="xpad")
                    nc.vector.memset(xpad[:, 0:3], 0.0)
                for half in range(2):
                    ps = pr.tile([128, 512], fp32, tag="proj", name="projps", bufs=2)
                    for k in range(16):
                        nc.tensor.matmul(ps[:], wt[k][:, col0 + m * 128:col0 + (m + 1) * 128],
                                         ht[k][:, half * 512:(half + 1) * 512],
                                         start=(k == 0), stop=(k == 15))
                    if xpad is not None:
                        nc.scalar.copy(xpad[:, 3 + half * 512: 3 + (half + 1) * 512], ps[:])
                    elif gate_bias is not None:
                        nc.scalar.activation(dst_bf16[:, half * 512:(half + 1) * 512],
                                             ps[:], Act.Silu, bias=gate_bias)
                    else:
                        nc.scalar.copy(dst_bf16[:, half * 512:(half + 1) * 512], ps[:])
                if xpad is None:
                    return
                cwm = cwt(m)
                s = conv_slot * 4
                a = convp.tile([128, T], fp32, tag="acca", name="acca", bufs=1)
                bt = convp.tile([128, T], fp32, tag="accb", name="accb", bufs=1)
                nc.vector.tensor_scalar(a[:], xpad[:, 3:3 + T], cwm[:, s + 3:s + 4],
                                        None, op0=Alu.mult)
                cur, nxt = a, bt
                for kk in (2, 1, 0):
                    nc.vector.scalar_tensor_tensor(nxt[:], xpad[:, kk:kk + T],
                                                   cwm[:, s + kk:s + kk + 1], cur[:],
                                                   op0=Alu.mult, op1=Alu.add)
                    cur, nxt = nxt, cur
                if pair is None:
                    nc.scalar.activation(dst_bf16[:], cur[:], Act.Silu)
                else:
                    qsil = qb[pair] if pair < 4 else kb[pair - 4]
                    qs[pair] = qsil
                    nc.scalar.activation(qsil[:], cur[:], Act.Silu)
                    sq = sqp.tile([128, T], bf16, tag="sq", name="sq", bufs=1)
                    nc.scalar.activation(sq[:], qsil[:], Act.Square)
                    for half in range(2):
                        pss = pr.tile([8, 512], fp32, tag="sqs", name="sqs")
                        nc.tensor.matmul(pss[:], oh8t[:, pair * 8:pair * 8 + 8],
                                         sq[:, half * 512:(half + 1) * 512],
                                         start=True, stop=True)
                        nc.vector.tensor_tensor(ssqsb[:, half * 512:(half + 1) * 512],
                                                ssqsb[:, half * 512:(half + 1) * 512],
                                                pss[:], op=Alu.add)

            for m in range(4):
                project(WQ0, m, conv_slot=0, pair=m)
            for m in range(4):
                project(WK0, m, conv_slot=1, pair=4 + m)
            for m in range(4):
                project(WV0, m, dst_bf16=vb[m], conv_slot=2)
            for m in range(4):
                project(WG0, m, dst_bf16=gateb[m], gate_bias=bgt[:, m:m + 1])

            # f1 projection
            for half in range(2):
                ps = pr.tile([128, 512], fp32, tag="proj", name="f1ps", bufs=2)
                for k in range(16):
                    nc.tensor.matmul(ps[:], wt[k][:, WF10:WF10 + 128],
                                     ht[k][:, half * 512:(half + 1) * 512],
                                     start=(k == 0), stop=(k == 15))
                nc.scalar.copy(f1b[:, half * 512:(half + 1) * 512], ps[:])

            # beta (sigmoid) then gate-softplus chain, table-load friendly order
            for half in range(2):
                bps = pr.tile([4, 512], fp32, tag="bps", name="bps")
                for k in range(16):
                    nc.tensor.matmul(bps[:], wt[k][:, WB0:WB0 + 4],
                                     ht[k][:, half * 512:(half + 1) * 512],
                                     start=(k == 0), stop=(k == 15))
                nc.scalar.activation(bsg[:, half * 512:(half + 1) * 512], bps[:],
                                     Act.Sigmoid)
            sp1s = []
            for half in range(2):
                for h in range(4):
                    gps = pr.tile([8, 512], fp32, tag="gps", name="gps", bufs=2)
                    nc.tensor.matmul(gps[:], wf2t[:, h * 8:(h + 1) * 8],
                                     f1b[:, half * 512:(half + 1) * 512],
                                     start=True, stop=True)
                    spe = smt.tile([8, 512], bf16, tag="spe", name="spe", bufs=2)
                    nc.scalar.activation(spe[:], gps[:], Act.Exp,
                                         bias=dtbt[:, h:h + 1])
                    sp1 = smt.tile([8, 512], bf16, tag="sp1", name="sp1", bufs=8)
                    nc.vector.tensor_scalar(sp1[:], spe[:], 1.0, None, op0=Alu.add)
                    sp1s.append((half, h, sp1))
            # all Ln together: l2 normalizer + softplus logs
            nrm = smt.tile([8, T], fp32, tag="nrm", name="nrm", bufs=1)
            nc.scalar.activation(nrm[:], ssqsb[:], Act.Ln, scale=sc8t[:, 0:1],
                                 bias=eps8t[:, 0:1])
            for half, h, sp1 in sp1s:
                sp = smt.tile([8, 512], bf16, tag="sp", name="sp", bufs=2)
                nc.scalar.activation(sp[:], sp1[:], Act.Ln)
                nc.vector.tensor_scalar(gna8[h][:, half * 512:(half + 1) * 512],
                                        sp[:], negat8(h), None, op0=Alu.mult)
            recb = smt.tile([8, T], bf16, tag="recb", name="recb", bufs=1)
            nc.scalar.activation(recb[:], nrm[:], Act.Exp, scale=-0.5)
            for pair in range(8):
                dst = qb[pair] if pair < 4 else kb[pair - 4]
                for half in range(2):
                    nb = pr.tile([128, 512], fp32, tag="nb", name="nb", bufs=2)
                    nc.tensor.matmul(nb[:], s8b[:, pair * 128:(pair + 1) * 128],
                                     recb[:, half * 512:(half + 1) * 512],
                                     start=True, stop=True)
                    nc.vector.tensor_tensor(dst[:, half * 512:(half + 1) * 512],
                                            qs[pair][:, half * 512:(half + 1) * 512],
                                            nb[:], op=Alu.mult)

        # weights/hT pools closed; load wo for the tail now (overlaps recurrence)
        pers2 = ctx.enter_context(tc.tile_pool(name="pers2", bufs=1))
        yb = [pers2.tile([128, T], bf16, tag=f"yb{m}", name=f"yb{m}")
              for m in range(4)]
        wotp = ctx.enter_context(tc.tile_pool(name="wotp", bufs=1))
        wot = [wotp.tile([128, D], bf16, tag=f"wo{k}", name=f"wo{k}") for k in range(4)]
        for k in range(4):
            dma(wot[k][:], wo[k * 128:(k + 1) * 128, :])

        # ================= recurrence =================
        rc = ctx.enter_context(tc.tile_pool(name="rc", bufs=2))
        rr = ctx.enter_context(tc.tile_pool(name="rr", bufs=4))
        rctx = ExitStack()
        prc = rctx.enter_context(tc.tile_pool(name="prc", bufs=1, space="PSUM"))
        # PSUM tags: tp(2) big(3) dblx(2) = 7 banks

        hdt = lambda nm, h, sh, dt=bf16, bufs=2: rc.tile(
            sh, dt, tag=f"{nm}{h}", name=f"{nm}{h}", bufs=bufs)

        for ci in range(NCH):
            ts = slice(ci * C, (ci + 1) * C)
            po, pn = ci % 2, (ci + 1) % 2
            # ---- chunk prologue ----
            cn8s = []
            for h in range(4):
                cn8 = hdt("cn8", h, [8, C], fp32)
                nc.vector.tensor_tensor_scan(cn8[:], ones32[0:8, :],
                                             gna8[h][:, ts], 0.0,
                                             op0=Alu.mult, op1=Alu.add)
                cn8s.append(cn8)
            cnt8s = []
            for h in range(4):
                cNtp = prc.tile([128, 8], fp32, tag="tp", name="cNtp", bufs=2)
                nc.tensor.transpose(cNtp[:], cn8s[h][:], idf[0:8, 0:8])
                cnt8 = hdt("cnt8", h, [128, 8], fp32)
                nc.scalar.copy(cnt8[:], cNtp[:])
                cnt8s.append(cnt8)
            b2p = prc.tile([128, 4], fp32, tag="tp", name="b2p", bufs=2)
            nc.tensor.transpose(b2p[:], bsg[:, ts], idf[0:4, 0:4])
            beta2 = rc.tile([128, 4], fp32, tag="beta2", name="beta2")
            nc.scalar.copy(beta2[:], b2p[:])
            # masked keys for groups whose base partition is illegal (16/48/80/96/112)
            MSKN = {1: 0, 3: 1, 5: 2, 6: 3, 7: 4}
            kmsks = []
            for h in range(4):
                kmsk = rr.tile([128, 5 * C], bf16, tag="kmsk", name="kmsk")
                for n, j in MSKN.items():
                    dst = kmsk[:, j * C:(j + 1) * C]
                    if j % 2 == 0:
                        nc.scalar.mul(dst, kb[h][:, ts], gmct[:, n:n + 1])
                    else:
                        nc.vector.tensor_scalar(dst, kb[h][:, ts], gmct[:, n:n + 1],
                                                None, op0=Alu.mult)
                kmsks.append(kmsk)

            # ---- per-head prologue (step-major) ----
            exp8s, exp8ks = [], []
            for h in range(4):
                e8 = hdt("exp8", h, [8, C], fp32)
                nc.scalar.activation(e8[:], cn8s[h][:], Act.Exp)
                exp8s.append(e8)
            for h in range(4):
                e8k = hdt("exp8k", h, [8, C], fp32)
                nc.scalar.activation(e8k[:], cn8s[h][:], Act.Exp,
                                     scale=-1.0,
                                     bias=cn8s[h][:, C - 1:C])
                exp8ks.append(e8k)
            bfks = []
            for h in range(4):
                bfk = prc.tile([128, 256], fp32, tag="tp", name="bfk", bufs=2)
                nc.tensor.matmul(bfk[:, 0:128], replt,
                                 exp8s[h][:], start=True, stop=True)
                nc.tensor.matmul(bfk[:, 128:256], replt,
                                 exp8ks[h][:], start=True, stop=True)
                bfks.append(bfk)
            bCs, Wts, qtTs, kends = [], [], [], []
            for h in range(4):
                bC = hdt("bC", h, [128, 1], fp32)
                nc.scalar.copy(bC[:], bfks[h][:, 127:128])
                bCs.append(bC)
            for h in range(4):
                Wth = hdt("Wt", h, [128, C])
                nc.vector.tensor_tensor(Wth[:], kb[h][:, ts], bfks[h][:, 0:128],
                                        op=Alu.mult)
                Wts.append(Wth)
                qtTh = hdt("qtT", h, [128, C])
                nc.vector.tensor_tensor(qtTh[:], qb[h][:, ts], bfks[h][:, 0:128],
                                        op=Alu.mult)
                qtTs.append(qtTh)
                kendh = hdt("kend", h, [128, C])
                nc.vector.tensor_tensor(kendh[:], kb[h][:, ts], bfks[h][:, 128:256],
                                        op=Alu.mult)
                kends.append(kendh)
            # decay matrices: bca broadcast (f32r) + fused sub/clamp + one exp
            ealls = []
            for h in range(4):
                bcaL = prc.tile([128, 512], fp32, tag="big", name="bcaL", bufs=3)
                bcaH = prc.tile([128, 512], fp32, tag="big", name="bcaH", bufs=3)
                for n in range(8):
                    dst = bcaL if n < 4 else bcaH
                    nc.tensor.matmul(dst[:, (n % 4) * C:(n % 4 + 1) * C],
                                     sel8[:, n * 128:(n + 1) * 128],
                                     cn8s[h][:],
                                     start=True, stop=True)
                eallin = rr.tile([128, 8 * C], bf16, tag="eallin", name="eallin")
                for n in range(8):
                    src = bcaL if n < 4 else bcaH
                    # relu(cnt - bca) = -min(bca - cnt, 0); exp uses scale=-1
                    nc.scalar.activation(eallin[:, n * C:(n + 1) * C],
                                         src[:, (n % 4) * C:(n % 4 + 1) * C],
                                         Act.Relu, scale=-1.0,
                                         bias=cnt8s[h][:, n:n + 1])
                eall = rr.tile([128, 8 * C], bf16, tag="eall", name="eall")
                nc.scalar.activation(eall[:], eallin[:], Act.Exp, scale=-1.0)
                ealls.append(eall)
            # group correlation matmuls (16-partition contraction)
            A0s, GtMs = [], []
            for h in range(4):
                pls = []
                for src in (kb[h], qb[h]):
                    pl = prc.tile([128, 512], fp32, tag="big", name="pall", bufs=3)
                    ph = prc.tile([128, 512], fp32, tag="big", name="pallh", bufs=3)
                    for n in range(8):
                        dst = pl if n < 4 else ph
                        if n in (0, 2, 4):
                            nc.tensor.matmul(dst[:, (n % 4) * C:(n % 4 + 1) * C],
                                             kb[h][16 * n:16 * (n + 1), ts],
                                             src[16 * n:16 * (n + 1), ts],
                                             start=True, stop=True)
                        else:
                            j = MSKN[n]
                            nc.tensor.matmul(dst[:, (n % 4) * C:(n % 4 + 1) * C],
                                             kmsks[h][:, j * C:(j + 1) * C],
                                             src[:, ts],
                                             start=True, stop=True)
                    pls.append((pl, ph))
                prods = []
                for x, (pl, ph) in enumerate(pls):
                    prod = rr.tile([128, 8 * C], bf16, tag="prod", name=f"prod{x}")
                    nc.vector.tensor_tensor(prod[:, 0:4 * C], ealls[h][:, 0:4 * C],
                                            pl[:], op=Alu.mult)
                    nc.vector.tensor_tensor(prod[:, 4 * C:], ealls[h][:, 4 * C:],
                                            ph[:], op=Alu.mult)
                    prods.append(prod)
                for x, prod in enumerate(prods):
                    t4 = rr.tile([128, 4 * C], bf16, tag="t4", name="t4")
                    nc.vector.tensor_tensor(t4[:], prod[:, :4 * C], prod[:, 4 * C:],
                                            op=Alu.add)
                    t2 = rr.tile([128, 2 * C], bf16, tag="t2", name="t2")
                    nc.vector.tensor_tensor(t2[:], t4[:, :2 * C], t4[:, 2 * C:],
                                            op=Alu.add)
                    t1 = rr.tile([128, C], bf16, tag="t1", name="t1")
                    nc.vector.tensor_tensor(t1[:], t2[:, :C], t2[:, C:], op=Alu.add)
                    # masked/beta-folded (Pool, SBUF-only)
                    if x == 0:
                        A0 = hdt("A0", h, [128, C])
                        nc.vector.scalar_tensor_tensor(A0[:], t1[:], beta2[:, h:h + 1],
                                                       mMt[:], op0=Alu.mult,
                                                       op1=Alu.mult)
                        A0s.append(A0)
                    else:
                        GtM = hdt("GtM", h, [128, C])
                        nc.vector.scalar_tensor_tensor(GtM[:], t1[:], beta2[:, h:h + 1],
                                                       mGt[:], op0=Alu.mult,
                                                       op1=Alu.mult)
                        GtMs.append(GtM)
            # vt / kts
            vts, ktss = [], []
            for h in range(4):
                vtp = prc.tile([128, C], bf16, tag="tp", name="vtp", bufs=2)
                nc.tensor.transpose(vtp[:], vb[h][:, ts], idb[:])
                vt = hdt("vt", h, [128, C])
                nc.vector.tensor_copy(vt[:], vtp[:])
                vts.append(vt)
                ktp = prc.tile([128, C], bf16, tag="tp", name="ktp", bufs=2)
                nc.tensor.transpose(ktp[:], kends[h][:], idb[:])
                kts = hdt("kts", h, [128, C])
                nc.vector.tensor_scalar(kts[:], ktp[:], beta2[:, h:h + 1], None,
                                        op0=Alu.mult)
                ktss.append(kts)
            # B0 transpose
            Bs = [[None] * 6 for _ in range(4)]
            As = [[None] * 7 for _ in range(4)]
            for h in range(4):
                As[h][0] = A0s[h]
                b0p = prc.tile([128, C], bf16, tag="tp", name="b0p", bufs=2)
                nc.tensor.transpose(b0p[:], A0s[h][:], idb[:])
                B0 = rc.tile([128, C], bf16, tag=f"B{h}", name=f"B{h}_0", bufs=3)
                nc.scalar.copy(B0[:], b0p[:])
                Bs[h][0] = B0
            # ---- spine: ws0 / x-chain with interleaved doubling ----
            xbs = []
            for h in range(4):
                ws0 = prc.tile([128, C], fp32, tag="tp", name="ws0", bufs=2)
                nc.tensor.matmul(ws0[:], Wts[h][:], Sb[h][po][:], start=True, stop=True)
                xb = rc.tile([128, C], bf16, tag=f"xb{h}", name=f"xb{h}", bufs=3)
                nc.vector.tensor_tensor(xb[:], vts[h][:], ws0[:], op=Alu.subtract)
                xbs.append(xb)
            # apply level 0 (subtract), then levels 1..6 with doubling
            for lev in range(7):
                if lev >= 1:
                    for h in range(4):
                        dbl = prc.tile([128, 256], fp32, tag="dblx", name="dbl", bufs=2)
                        nc.tensor.matmul(dbl[:, 0:128], Bs[h][lev - 1][:],
                                         As[h][lev - 1][:], start=True, stop=True)
                        if lev < 6:
                            nc.tensor.matmul(dbl[:, 128:256], As[h][lev - 1][:],
                                             Bs[h][lev - 1][:], start=True, stop=True)
                        An = rc.tile([128, C], bf16, tag=f"A{h}", name=f"A{h}_{lev}",
                                     bufs=3)
                        nc.scalar.copy(An[:], dbl[:, 0:128])
                        As[h][lev] = An
                        if lev < 6:
                            Bn = rc.tile([128, C], bf16, tag=f"B{h}",
                                         name=f"B{h}_{lev}", bufs=3)
                            nc.vector.tensor_copy(Bn[:], dbl[:, 128:256])
                            Bs[h][lev] = Bn
                for h in range(4):
                    mx = prc.tile([128, C], fp32, tag="dblx", name="mx", bufs=2)
                    nc.tensor.matmul(mx[:], As[h][lev][:], xbs[h][:],
                                     start=True, stop=True)
                    xn = rc.tile([128, C], bf16, tag=f"xb{h}", name=f"xb{h}_{lev}",
                                 bufs=3)
                    nc.vector.tensor_tensor(xn[:], xbs[h][:], mx[:],
                                            op=(Alu.subtract if lev == 0 else Alu.add))
                    xbs[h] = xn
            # state update + output
            for h in range(4):
                sup = prc.tile([128, DV], fp32, tag="tp", name="sup", bufs=2)
                nc.tensor.matmul(sup[:], ktss[h][:], xbs[h][:], start=True, stop=True)
                nc.vector.scalar_tensor_tensor(Sf[h][pn][:], Sf[h][po][:],
                                               bCs[h][:, 0:1], sup[:],
                                               op0=Alu.mult, op1=Alu.add)
                nc.vector.scalar_tensor_tensor(Sb[h][pn][:], Sf[h][po][:],
                                               bCs[h][:, 0:1], sup[:],
                                               op0=Alu.mult, op1=Alu.add)
                otp = prc.tile([128, C], fp32, tag="tp", name="otp", bufs=2)
                nc.tensor.matmul(otp[:], Sb[h][po][:], qtTs[h][:],
                                 start=True, stop=False)
                nc.tensor.matmul(otp[:], xbs[h][:], GtMs[h][:],
                                 start=False, stop=True)
                nc.vector.tensor_tensor(yb[h][:, ts], gateb[h][:, ts], otp[:],
                                        op=Alu.mult)

        rctx.close()
        # ================= deferred RMSNorm + output projection =================
        with tc.tile_pool(name="post", bufs=2) as post, \
             tc.tile_pool(name="ppc", bufs=2, space="PSUM") as ppc:
            # PSUM tags: ssp(2) rbc(2) proj(2) = 6 banks
            ysqs = []
            for h in range(4):
                ysq = post.tile([128, T], bf16, tag="ysq", name="ysq", bufs=4)
                nc.scalar.activation(ysq[:], yb[h][:], Act.Square)
                ysqs.append(ysq)
            nrcs = []
            for h in range(4):
                nrc = post.tile([1, T], fp32, tag="nrc", name="nrc", bufs=4)
                for half in range(2):
                    ssp = ppc.tile([1, 512], fp32, tag="ssp", name="ssp")
                    nc.tensor.matmul(ssp[:], octb[:],
                                     ysqs[h][:, half * 512:(half + 1) * 512],
                                     start=True, stop=True)
                    nc.scalar.activation(nrc[:, half * 512:(half + 1) * 512],
                                         ssp[:], Act.Ln, scale=1.0 / DV,
                                         bias=epsnt[:, 0:1])
                nrcs.append(nrc)
            for h in range(4):
                rcb = post.tile([1, T], bf16, tag="rcb", name="rcb", bufs=4)
                nc.scalar.activation(rcb[:], nrcs[h][:], Act.Exp, scale=-0.5)
                for half in range(2):
                    rbc = ppc.tile([128, 512], fp32, tag="rbc", name="rbc")
                    nc.tensor.matmul(rbc[:], o1b[:], rcb[:, half * 512:(half + 1) * 512],
                                     start=True, stop=True)
                    nc.vector.scalar_tensor_tensor(yb[h][:, half * 512:(half + 1) * 512],
                                                   yb[h][:, half * 512:(half + 1) * 512],
                                                   nwt[:, 0:1], rbc[:],
                                                   op0=Alu.mult, op1=Alu.mult)
            # output projection
            for m in range(16):
                osb = post.tile([128, T], fp32, tag="osb", name="osb")
                for half in range(2):
                    ps = ppc.tile([128, 512], fp32, tag="proj", name="ops")
                    for k in range(4):
                        nc.tensor.matmul(ps[:], wot[k][:, m * 128:(m + 1) * 128],
                                         yb[k][:, half * 512:(half + 1) * 512],
                                         start=(k == 0), stop=(k == 3))
                    if half == 0:
                        nc.vector.tensor_copy(osb[:, 0:512], ps[:])
                    else:
                        nc.scalar.copy(osb[:, 512:1024], ps[:])
                dma(outT[m * 128:(m + 1) * 128, :], osb[:])

    nc.compile()
    return nc


def _prep_inputs(inputs):
    f32 = np.float32
    hs = np.asarray(inputs['hidden_states'], f32)
    tri = np.tril(np.ones((C, C), f32))
    maskM = (1.0 - tri).astype(f32)
    maskG = (1.0 - tri + np.eye(C, dtype=f32)).astype(f32)
    repl = np.zeros((NG, DK), f32)
    for n in range(NG):
        repl[n, n * GG:(n + 1) * GG] = 1.0
    sel8 = np.zeros((NG, NG * 128), f32)
    for n in range(NG):
        sel8[n, n * 128:(n + 1) * 128] = 1.0
    oh8 = np.zeros((DK, 64), f32)
    for i in range(8):
        oh8[:, i * 8 + i] = 1.0
    ident = np.eye(128, dtype=f32)

    maps = []
    for c in range(8):
        b, hg = c // 4, c % 4
        cols = slice(hg * NH * DK, (hg + 1) * NH * DK)
        gcols = slice(hg * NH * NG, (hg + 1) * NH * NG)
        hcols = slice(hg * NH, (hg + 1) * NH)
        nega = -np.exp(np.repeat(np.asarray(inputs['A_log'], f32)[hcols], NG))

        packf = np.zeros((128, NF), f32)
        cw = np.concatenate(
            [np.asarray(inputs['conv_q'], f32)[cols],
             np.asarray(inputs['conv_k'], f32)[cols],
             np.asarray(inputs['conv_v'], f32)[cols]], 1)  # [512, 12]
        for m in range(4):
            packf[:, PF_CW + m * 12:PF_CW + (m + 1) * 12] = cw[m * 128:(m + 1) * 128]
        packf[:, PF_BG:PF_BG + 4] = np.asarray(inputs['bg'], f32)[cols].reshape(NH, DV).T
        packf[:, PF_NW] = np.asarray(inputs['norm_w'], f32)
        packf[0:8, PF_NEGA:PF_NEGA + 4] = nega.reshape(NH, NG).T
        packf[0:8, PF_DTB:PF_DTB + 4] = (
            np.asarray(inputs['dt_bias'], f32)[gcols].reshape(NH, NG).T)
        packf[0:8, PF_SC8] = [1.0 / SCALE ** 2] * 4 + [1.0] * 4
        packf[0:8, PF_EPS8] = [1e-6 / SCALE ** 2] * 4 + [1e-6] * 4
        packf[0:1, PF_EPSN] = EPS
        packf[0:8, PF_REPL:PF_REPL + 128] = repl
        packf[:, PF_IDF:PF_IDF + 128] = ident
        packf[0:8, PF_SEL:PF_SEL + 1024] = sel8
        packf[:, PF_GMC:PF_GMC + 8] = repl.T

        packb = np.zeros((128, NB), f32)
        packb[:, PB_OH8:PB_OH8 + 64] = oh8
        packb[0:8, PB_S8B:PB_S8B + 1024] = sel8
        packb[:, PB_MM:PB_MM + 128] = maskM
        packb[:, PB_MG:PB_MG + 128] = maskG
        packb[:, PB_IDB:PB_IDB + 128] = ident
        packb[:, PB_OCT] = 1.0
        packb[0:1, PB_O1B:PB_O1B + 128] = 1.0

        wallm = np.concatenate(
            [np.asarray(inputs['Wq'], f32)[:, cols],
             np.asarray(inputs['Wk'], f32)[:, cols],
             np.asarray(inputs['Wv'], f32)[:, cols],
             np.asarray(inputs['Wg'], f32)[:, cols],
             np.asarray(inputs['Wf1'], f32),
             np.asarray(inputs['Wb'], f32)[:, hcols]], 1)

        m = {
            'hT': np.ascontiguousarray(hs[b].T).astype(BF),
            'wall': np.ascontiguousarray(wallm).astype(BF),
            'wo': np.ascontiguousarray(np.asarray(inputs['Wo'], f32)[cols, :]).astype(BF),
            'wf2': np.ascontiguousarray(np.asarray(inputs['Wf2'], f32)[:, gcols]).astype(BF),
            'packf': packf,
            'packb': packb.astype(BF),
        }
        maps.append(m)
    return maps


def kernel(**inputs):
    from concourse.bass_utils import run_bass_kernel_spmd
    if 'nc' not in _CACHE:
        _CACHE['nc'] = _build()
    nc = _CACHE['nc']
    maps = _prep_inputs(inputs)
    res = run_bass_kernel_spmd(nc, maps, list(range(8))).results
    out = np.zeros((B, T, D), np.float32)
    for c in range(8):
        out[c // 4] += res[c]['outT'].T.astype(np.float32)
    return out


# revision 20
# speedup vs baseline: 1.6631x; 1.0386x over previous
"""Grouped gated DeltaNet (KDA-style) on 8 TRN2 NeuronCores.

Sharding: core c -> (batch b = c//4, head-group hg = c%4 of 4 heads).
Per core: column-sharded projections (weights resident, loaded once),
short-conv+silu, l2norm, chunked gated delta-rule recurrence (chunk
C=128, group decay via 1-partition f32r broadcast matmuls + fused
sub/clamp, 16-partition group correlation matmuls, transpose-free A/B
doubling with interleaved triangular-solve application), deferred gated
RMSNorm (batched over T), row-shard output projection. Host sums 4
partials per batch.

Self-contained: B=2, T=1024, D=2048, H=16, DK=DV=128 hardcoded.
"""
import sys
sys.path.insert(0, '/opt/trn_rl_repo')
import numpy as np
import ml_dtypes
from contextlib import ExitStack

B, T, D = 2, 1024, 2048
H, DK, DV, GG = 16, 128, 128, 16
NG = DK // GG          # 8 gate groups per head
NH = 4                 # heads per core
C = 128                # chunk length
NCH = T // C
SCALE = DK ** -0.5
EPS = 1e-5

# packf fp32 column offsets
PF_CW = 0        # 4 blocks x 12
PF_BG = 48
PF_NW = 52
PF_NEGA = 53     # [8,4] (n,h)
PF_DTB = 57      # [8,4]
PF_SC8 = 61
PF_EPS8 = 62
PF_EPSN = 63
PF_REPL = 64     # [8,128]
PF_IDF = 192     # [128,128]
PF_SEL = 320     # [8,1024] group-selector
PF_GMC = 1344    # [128,8] group row-mask cols
NF = 1352
# packb bf16 column offsets
PB_OH8 = 0       # [128,64]
PB_S8B = 64      # [8,1024]
PB_MM = 1088     # [128,128]
PB_MG = 1216
PB_IDB = 1344
PB_OCT = 1472    # [128,1]
PB_O1B = 1473    # [1,128]
NB = 1601

WQ0, WK0, WV0, WG0, WF10, WB0 = 0, 512, 1024, 1536, 2048, 2176
WALLC = 2180

BF = ml_dtypes.bfloat16
_CACHE = {}

FP32_CHAIN = False   # fp32 x-chain fallback (precision)


def _build():
    import concourse.tile as tile
    from concourse import bacc, mybir

    fp32 = mybir.dt.float32
    f32r = mybir.dt.float32r
    bf16 = mybir.dt.bfloat16
    Alu = mybir.AluOpType
    Act = mybir.ActivationFunctionType

    nc = bacc.Bacc("TRN2", target_bir_lowering=False, debug=False, num_devices=8)
    dp = lambda n, sh, dt: nc.dram_tensor(n, sh, dt, kind="ExternalInput").ap()
    hT = dp("hT", [D, T], bf16)
    wall = dp("wall", [D, WALLC], bf16)
    wo = dp("wo", [NH * DV, D], bf16)
    wf2 = dp("wf2", [DV, NH * NG], bf16)
    packf = dp("packf", [128, NF], fp32)
    packb = dp("packb", [128, NB], bf16)
    outT = nc.dram_tensor("outT", [D, T], fp32, kind="ExternalOutput").ap()

    with tile.TileContext(nc) as tc, ExitStack() as ctx:
        pool = lambda name, bufs, space="SBUF": ctx.enter_context(
            tc.tile_pool(name=name, bufs=bufs, space=space))

        cons = pool("cons", 1)
        pers = pool("pers", 1)
        stp = pool("st", 1)

        dma = nc.sync.dma_start

        pf = cons.tile([128, NF], fp32, tag="packf", name="packf")
        dma(pf[:], packf[:])
        pb = cons.tile([128, NB], bf16, tag="packb", name="packb")
        dma(pb[:], packb[:])
        wf2t = cons.tile([128, 32], bf16, tag="wf2t", name="wf2t")
        dma(wf2t[:], wf2[:])

        cwt = lambda m: pf[:, PF_CW + m * 12: PF_CW + (m + 1) * 12]
        bgt = pf[:, PF_BG:PF_BG + 4]
        nwt = pf[:, PF_NW:PF_NW + 1]
        negat8 = lambda h: pf[0:8, PF_NEGA + h:PF_NEGA + h + 1]
        dtbt = pf[0:8, PF_DTB:PF_DTB + 4]
        sc8t = pf[0:8, PF_SC8:PF_SC8 + 1]
        eps8t = pf[0:8, PF_EPS8:PF_EPS8 + 1]
        epsnt = pf[0:1, PF_EPSN:PF_EPSN + 1]
        replt = pf[0:8, PF_REPL:PF_REPL + 128]
        idf = pf[:, PF_IDF:PF_IDF + 128]
        sel8 = pf[0:8, PF_SEL:PF_SEL + 1024]
        oh8t = pb[:, PB_OH8:PB_OH8 + 64]
        s8b = pb[0:8, PB_S8B:PB_S8B + 1024]
        mMt = pb[:, PB_MM:PB_MM + 128]
        mGt = pb[:, PB_MG:PB_MG + 128]
        idb = pb[:, PB_IDB:PB_IDB + 128]
        octb = pb[:, PB_OCT:PB_OCT + 1]
        o1b = pb[0:1, PB_O1B:PB_O1B + 128]
        gmct = pf[:, PF_GMC:PF_GMC + 8]

        ones32 = cons.tile([32, C], fp32, tag="ones32", name="ones32")
        nc.vector.memset(ones32[:], 1.0)

        # ---- persistent activations ----
        mk = lambda nm: [pers.tile([128, T], bf16, tag=f"{nm}{m}", name=f"{nm}{m}")
                         for m in range(4)]
        qb, kb, vb = mk("qb"), mk("kb"), mk("vb")
        gateb = mk("gateb")
        f1b = pers.tile([128, T], bf16, tag="f1b", name="f1b")
        gna8 = [pers.tile([8, T], bf16, tag=f"gna{h}", name=f"gna{h}")
                for h in range(4)]
        bsg = pers.tile([4, T], fp32, tag="bsg", name="bsg")

        # ---- state tiles (parity pairs) ----
        Sf = [[stp.tile([128, DV], fp32, tag=f"Sf{h}_{p}", name=f"Sf{h}_{p}")
               for p in range(2)] for h in range(4)]
        Sb = [[stp.tile([128, DV], bf16, tag=f"Sb{h}_{p}", name=f"Sb{h}_{p}")
               for p in range(2)] for h in range(4)]
        for h in range(4):
            nc.vector.memset(Sf[h][0][:], 0.0)
            nc.vector.memset(Sb[h][0][:], 0.0)

        # ================= projections =================
        with tc.tile_pool(name="htp", bufs=1) as htp, \
             tc.tile_pool(name="wallp", bufs=1) as wallp, \
             tc.tile_pool(name="convp", bufs=2) as convp, \
             tc.tile_pool(name="sqp", bufs=2) as sqp, \
             tc.tile_pool(name="smt", bufs=2) as smt, \
             tc.tile_pool(name="pps", bufs=1, space="PSUM") as pr:
            # PSUM tags: proj(2) sqs(1) bps(1) gps(1) nb(2) = 7 banks
            ssqsb = smt.tile([8, T], fp32, tag="ssqsb", name="ssqsb", bufs=1)
            nc.vector.memset(ssqsb[:], 0.0)
            ht, wt = [], []
            for k in range(16):
                t = htp.tile([128, T], bf16, tag=f"ht{k}", name=f"ht{k}")
                dma(t[:], hT[k * 128:(k + 1) * 128, :])
                ht.append(t)
                w = wallp.tile([128, WALLC], bf16, tag=f"wl{k}", name=f"wl{k}")
                dma(w[:], wall[k * 128:(k + 1) * 128, :])
                wt.append(w)

            qs = {}

            def project(col0, m, dst_bf16=None, conv_slot=None, pair=None,
                        gate_bias=None):
                xpad = None
                if conv_slot is not None:
                    xpad = convp.tile([128, T + 3], fp32, tag="xpad", name="xpad")
                    nc.vector.memset(xpad[:, 0:3], 0.0)
                for half in range(2):
                    ps = pr.tile([128, 512], fp32, tag="proj", name="projps", bufs=2)
                    for k in range(16):
                        nc.tensor.matmul(ps[:], wt[k][:, col0 + m * 128:col0 + (m + 1) * 128],
                                         ht[k][:, half * 512:(half + 1) * 512],
                                         start=(k == 0), stop=(k == 15))
                    if xpad is not None:
                        nc.scalar.copy(xpad[:, 3 + half * 512: 3 + (half + 1) * 512], ps[:])
                    elif gate_bias is not None:
                        nc.scalar.activation(dst_bf16[:, half * 512:(half + 1) * 512],
                                             ps[:], Act.Silu, bias=gate_bias)
                    else:
                        nc.scalar.copy(dst_bf16[:, half * 512:(half + 1) * 512], ps[:])
                if xpad is None:
                    return
                cwm = cwt(m)
                s = conv_slot * 4
                a = convp.tile([128, T], fp32, tag="acca", name="acca", bufs=1)
                bt = convp.tile([128, T], fp32, tag="accb", name="accb", bufs=1)
                nc.vector.tensor_scalar(a[:], xpad[:, 3:3 + T], cwm[:, s + 3:s + 4],
                                        None, op0=Alu.mult)
                cur, nxt = a, bt
                for kk in (2, 1, 0):
                    nc.vector.scalar_tensor_tensor(nxt[:], xpad[:, kk:kk + T],
                                                   cwm[:, s + kk:s + kk + 1], cur[:],
                                                   op0=Alu.mult, op1=Alu.add)
                    cur, nxt = nxt, cur
                if pair is None:
                    nc.scalar.activation(dst_bf16[:], cur[:], Act.Silu)
                else:
                    qsil = qb[pair] if pair < 4 else kb[pair - 4]
                    qs[pair] = qsil
                    nc.scalar.activation(qsil[:], cur[:], Act.Silu)
                    sq = sqp.tile([128, T], bf16, tag="sq", name="sq", bufs=1)
                    nc.scalar.activation(sq[:], qsil[:], Act.Square)
                    for half in range(2):
                        pss = pr.tile([8, 512], fp32, tag="sqs", name="sqs")
                        nc.tensor.matmul(pss[:], oh8t[:, pair * 8:pair * 8 + 8],
                                         sq[:, half * 512:(half + 1) * 512],
                                         start=True, stop=True)
                        nc.vector.tensor_tensor(ssqsb[:, half * 512:(half + 1) * 512],
                                                ssqsb[:, half * 512:(half + 1) * 512],
                                                pss[:], op=Alu.add)

            for m in range(4):
                project(WQ0, m, conv_slot=0, pair=m)
            for m in range(4):
                project(WK0, m, conv_slot=1, pair=4 + m)
            for m in range(4):
                project(WV0, m, dst_bf16=vb[m], conv_slot=2)
            for m in range(4):
                project(WG0, m, dst_bf16=gateb[m], gate_bias=bgt[:, m:m + 1])

            # f1 projection
            for half in range(2):
                ps = pr.tile([128, 512], fp32, tag="proj", name="f1ps", bufs=2)
                for k in range(16):
                    nc.tensor.matmul(ps[:], wt[k][:, WF10:WF10 + 128],
                                     ht[k][:, half * 512:(half + 1) * 512],
                                     start=(k == 0), stop=(k == 15))
                nc.scalar.copy(f1b[:, half * 512:(half + 1) * 512], ps[:])

            # beta (sigmoid) then gate-softplus chain, table-load friendly order
            for half in range(2):
                bps = pr.tile([4, 512], fp32, tag="bps", name="bps")
                for k in range(16):
                    nc.tensor.matmul(bps[:], wt[k][:, WB0:WB0 + 4],
                                     ht[k][:, half * 512:(half + 1) * 512],
                                     start=(k == 0), stop=(k == 15))
                nc.scalar.activation(bsg[:, half * 512:(half + 1) * 512], bps[:],
                                     Act.Sigmoid)
            sp1s = []
            for half in range(2):
                for h in range(4):
                    gps = pr.tile([8, 512], fp32, tag="gps", name="gps", bufs=2)
                    nc.tensor.matmul(gps[:], wf2t[:, h * 8:(h + 1) * 8],
                                     f1b[:, half * 512:(half + 1) * 512],
                                     start=True, stop=True)
                    spe = smt.tile([8, 512], bf16, tag="spe", name="spe", bufs=2)
                    nc.scalar.activation(spe[:], gps[:], Act.Exp,
                                         bias=dtbt[:, h:h + 1])
                    sp1 = smt.tile([8, 512], bf16, tag="sp1", name="sp1", bufs=8)
                    nc.vector.tensor_scalar(sp1[:], spe[:], 1.0, None, op0=Alu.add)
                    sp1s.append((half, h, sp1))
            # all Ln together: l2 normalizer + softplus logs
            nrm = smt.tile([8, T], fp32, tag="nrm", name="nrm", bufs=1)
            nc.scalar.activation(nrm[:], ssqsb[:], Act.Ln, scale=sc8t[:, 0:1],
                                 bias=eps8t[:, 0:1])
            for half, h, sp1 in sp1s:
                sp = smt.tile([8, 512], bf16, tag="sp", name="sp", bufs=2)
                nc.scalar.activation(sp[:], sp1[:], Act.Ln)
                nc.vector.tensor_scalar(gna8[h][:, half * 512:(half + 1) * 512],
                                        sp[:], negat8(h), None, op0=Alu.mult)
            recb = smt.tile([8, T], bf16, tag="recb", name="recb", bufs=1)
            nc.scalar.activation(recb[:], nrm[:], Act.Exp, scale=-0.5)
            for pair in range(8):
                dst = qb[pair] if pair < 4 else kb[pair - 4]
                for half in range(2):
                    nb = pr.tile([128, 512], fp32, tag="nb", name="nb", bufs=2)
                    nc.tensor.matmul(nb[:], s8b[:, pair * 128:(pair + 1) * 128],
                                     recb[:, half * 512:(half + 1) * 512],
                                     start=True, stop=True)
                    nc.vector.tensor_tensor(dst[:, half * 512:(half + 1) * 512],
                                            qs[pair][:, half * 512:(half + 1) * 512],
                                            nb[:], op=Alu.mult)

        # weights/hT pools closed; load wo for the tail now (overlaps recurrence)
        pers2 = ctx.enter_context(tc.tile_pool(name="pers2", bufs=1))
        yb = [pers2.tile([128, T], bf16, tag=f"yb{m}", name=f"yb{m}")
              for m in range(4)]
        wotp = ctx.enter_context(tc.tile_pool(name="wotp", bufs=1))
        wot = [wotp.tile([128, D], bf16, tag=f"wo{k}", name=f"wo{k}") for k in range(4)]
        for k in range(4):
            dma(wot[k][:], wo[k * 128:(k + 1) * 128, :])

        # ================= recurrence =================
        rc = ctx.enter_context(tc.tile_pool(name="rc", bufs=2))
        rr = ctx.enter_context(tc.tile_pool(name="rr", bufs=4))
        rctx = ExitStack()
        prc = rctx.enter_context(tc.tile_pool(name="prc", bufs=1, space="PSUM"))
        # PSUM tags: tp(2) big(3) dblx(2) = 7 banks

        hdt = lambda nm, h, sh, dt=bf16, bufs=2: rc.tile(
            sh, dt, tag=f"{nm}{h}", name=f"{nm}{h}", bufs=bufs)

        MSKN = {1: 0, 3: 1, 5: 2, 6: 3, 7: 4}

        def prologue(ci):
            ts = slice(ci * C, (ci + 1) * C)
            P = {'ts': ts}
            cn8s = []
            for h in range(4):
                cn8 = hdt("cn8", h, [8, C], fp32)
                nc.vector.tensor_tensor_scan(cn8[:], ones32[0:8, :],
                                             gna8[h][:, ts], 0.0,
                                             op0=Alu.mult, op1=Alu.add)
                cn8s.append(cn8)
            cnt8s = []
            for h in range(4):
                cNtp = prc.tile([128, 8], fp32, tag="tp", name="cNtp", bufs=2)
                nc.tensor.transpose(cNtp[:], cn8s[h][:], idf[0:8, 0:8])
                cnt8 = hdt("cnt8", h, [128, 8], fp32)
                nc.scalar.copy(cnt8[:], cNtp[:])
                cnt8s.append(cnt8)
            b2p = prc.tile([128, 4], fp32, tag="tp", name="b2p", bufs=2)
            nc.tensor.transpose(b2p[:], bsg[:, ts], idf[0:4, 0:4])
            beta2 = rc.tile([128, 4], fp32, tag="beta2", name="beta2")
            nc.scalar.copy(beta2[:], b2p[:])
            kmsks = []
            for h in range(4):
                kmsk = rr.tile([128, 5 * C], bf16, tag="kmsk", name="kmsk")
                for n, j in MSKN.items():
                    dst = kmsk[:, j * C:(j + 1) * C]
                    if j % 2 == 0:
                        nc.scalar.mul(dst, kb[h][:, ts], gmct[:, n:n + 1])
                    else:
                        nc.vector.tensor_scalar(dst, kb[h][:, ts], gmct[:, n:n + 1],
                                                None, op0=Alu.mult)
                kmsks.append(kmsk)
            exp8s, exp8ks = [], []
            for h in range(4):
                e8 = hdt("exp8", h, [8, C], fp32)
                nc.scalar.activation(e8[:], cn8s[h][:], Act.Exp)
                exp8s.append(e8)
            for h in range(4):
                e8k = hdt("exp8k", h, [8, C], fp32)
                nc.scalar.activation(e8k[:], cn8s[h][:], Act.Exp, scale=-1.0,
                                     bias=cn8s[h][:, C - 1:C])
                exp8ks.append(e8k)
            bfks = []
            for h in range(4):
                bfk = prc.tile([128, 256], fp32, tag="tp", name="bfk", bufs=2)
                nc.tensor.matmul(bfk[:, 0:128], replt, exp8s[h][:],
                                 start=True, stop=True)
                nc.tensor.matmul(bfk[:, 128:256], replt, exp8ks[h][:],
                                 start=True, stop=True)
                bfks.append(bfk)
            P['bC'] = []
            for h in range(4):
                bC = hdt("bC", h, [128, 1], fp32)
                nc.scalar.copy(bC[:], bfks[h][:, 127:128])
                P['bC'].append(bC)
            P['Wt'], P['qtT'], kends = [], [], []
            for h in range(4):
                Wth = hdt("Wt", h, [128, C])
                nc.vector.tensor_tensor(Wth[:], kb[h][:, ts], bfks[h][:, 0:128],
                                        op=Alu.mult)
                P['Wt'].append(Wth)
                qtTh = hdt("qtT", h, [128, C])
                nc.vector.tensor_tensor(qtTh[:], qb[h][:, ts], bfks[h][:, 0:128],
                                        op=Alu.mult)
                P['qtT'].append(qtTh)
                kendh = hdt("kend", h, [128, C])
                nc.vector.tensor_tensor(kendh[:], kb[h][:, ts], bfks[h][:, 128:256],
                                        op=Alu.mult)
                kends.append(kendh)
            ealls = []
            for h in range(4):
                bcaL = prc.tile([128, 512], fp32, tag="big", name="bcaL", bufs=3)
                bcaH = prc.tile([128, 512], fp32, tag="big", name="bcaH", bufs=3)
                for n in range(8):
                    dst = bcaL if n < 4 else bcaH
                    nc.tensor.matmul(dst[:, (n % 4) * C:(n % 4 + 1) * C],
                                     sel8[:, n * 128:(n + 1) * 128], cn8s[h][:],
                                     start=True, stop=True)
                eallin = rr.tile([128, 8 * C], bf16, tag="eallin", name="eallin")
                for n in range(8):
                    bsrc = bcaL if n < 4 else bcaH
                    nc.scalar.activation(eallin[:, n * C:(n + 1) * C],
                                         bsrc[:, (n % 4) * C:(n % 4 + 1) * C],
                                         Act.Relu, scale=-1.0,
                                         bias=cnt8s[h][:, n:n + 1])
                eall = rr.tile([128, 8 * C], bf16, tag="eall", name="eall")
                nc.scalar.activation(eall[:], eallin[:], Act.Exp, scale=-1.0)
                ealls.append(eall)
            P['A'] = [[None] * 7 for _ in range(4)]
            P['GtM'] = []
            for h in range(4):
                pls = []
                for csrc in (kb[h], qb[h]):
                    pl = prc.tile([128, 512], fp32, tag="big", name="pall", bufs=3)
                    ph = prc.tile([128, 512], fp32, tag="big", name="pallh", bufs=3)
                    for n in range(8):
                        dst = pl if n < 4 else ph
                        if n in (0, 2, 4):
                            nc.tensor.matmul(dst[:, (n % 4) * C:(n % 4 + 1) * C],
                                             kb[h][16 * n:16 * (n + 1), ts],
                                             csrc[16 * n:16 * (n + 1), ts],
                                             start=True, stop=True)
                        else:
                            j = MSKN[n]
                            nc.tensor.matmul(dst[:, (n % 4) * C:(n % 4 + 1) * C],
                                             kmsks[h][:, j * C:(j + 1) * C],
                                             csrc[:, ts],
                                             start=True, stop=True)
                    pls.append((pl, ph))
                prods = []
                for x, (pl, ph) in enumerate(pls):
                    prod = rr.tile([128, 8 * C], bf16, tag="prod", name=f"prod{x}")
                    nc.vector.tensor_tensor(prod[:, 0:4 * C], ealls[h][:, 0:4 * C],
                                            pl[:], op=Alu.mult)
                    nc.vector.tensor_tensor(prod[:, 4 * C:], ealls[h][:, 4 * C:],
                                            ph[:], op=Alu.mult)
                    prods.append(prod)
                for x, prod in enumerate(prods):
                    t4 = rr.tile([128, 4 * C], bf16, tag="t4", name="t4")
                    nc.vector.tensor_tensor(t4[:], prod[:, :4 * C], prod[:, 4 * C:],
                                            op=Alu.add)
                    t2 = rr.tile([128, 2 * C], bf16, tag="t2", name="t2")
                    nc.vector.tensor_tensor(t2[:], t4[:, :2 * C], t4[:, 2 * C:],
                                            op=Alu.add)
                    t1 = rr.tile([128, C], bf16, tag="t1", name="t1")
                    nc.vector.tensor_tensor(t1[:], t2[:, :C], t2[:, C:], op=Alu.add)
                    if x == 0:
                        A0 = rc.tile([128, C], bf16, tag=f"A{h}", name=f"A{h}_0",
                                     bufs=14)
                        nc.vector.scalar_tensor_tensor(A0[:], t1[:],
                                                       beta2[:, h:h + 1], mMt[:],
                                                       op0=Alu.mult, op1=Alu.mult)
                        P['A'][h][0] = A0
                    else:
                        GtM = hdt("GtM", h, [128, C])
                        nc.vector.scalar_tensor_tensor(GtM[:], t1[:],
                                                       beta2[:, h:h + 1], mGt[:],
                                                       op0=Alu.mult, op1=Alu.mult)
                        P['GtM'].append(GtM)
            P['vt'], P['kts'] = [], []
            for h in range(4):
                vtp = prc.tile([128, C], bf16, tag="tp", name="vtp", bufs=2)
                nc.tensor.transpose(vtp[:], vb[h][:, ts], idb[:])
                vt = hdt("vt", h, [128, C])
                nc.vector.tensor_copy(vt[:], vtp[:])
                P['vt'].append(vt)
                ktp = prc.tile([128, C], bf16, tag="tp", name="ktp", bufs=2)
                nc.tensor.transpose(ktp[:], kends[h][:], idb[:])
                kts = hdt("kts", h, [128, C])
                nc.vector.tensor_scalar(kts[:], ktp[:], beta2[:, h:h + 1], None,
                                        op0=Alu.mult)
                P['kts'].append(kts)
            # A/B doubling chains (state-independent)
            Bs = [[None] * 6 for _ in range(4)]
            for h in range(4):
                b0p = prc.tile([128, C], bf16, tag="tp", name="b0p", bufs=2)
                nc.tensor.transpose(b0p[:], P['A'][h][0][:], idb[:])
                B0 = rc.tile([128, C], bf16, tag=f"B{h}", name=f"B{h}_0", bufs=3)
                nc.scalar.copy(B0[:], b0p[:])
                Bs[h][0] = B0
            for lev in range(1, 7):
                for h in range(4):
                    dbl = prc.tile([128, 256], fp32, tag="dblx", name="dbl", bufs=2)
                    nc.tensor.matmul(dbl[:, 0:128], Bs[h][lev - 1][:],
                                     P['A'][h][lev - 1][:], start=True, stop=True)
                    if lev < 6:
                        nc.tensor.matmul(dbl[:, 128:256], P['A'][h][lev - 1][:],
                                         Bs[h][lev - 1][:], start=True, stop=True)
                    An = rc.tile([128, C], bf16, tag=f"A{h}", name=f"A{h}_{lev}",
                                 bufs=14)
                    nc.scalar.copy(An[:], dbl[:, 0:128])
                    P['A'][h][lev] = An
                    if lev < 6:
                        Bn = rc.tile([128, C], bf16, tag=f"B{h}",
                                     name=f"B{h}_{lev}", bufs=3)
                        nc.vector.tensor_copy(Bn[:], dbl[:, 128:256])
                        Bs[h][lev] = Bn
            return P

        def spine(ci, P):
            ts = P['ts']
            po, pn = ci % 2, (ci + 1) % 2
            xbs = []
            for h in range(4):
                ws0 = prc.tile([128, C], fp32, tag="tp", name="ws0", bufs=2)
                nc.tensor.matmul(ws0[:], P['Wt'][h][:], Sb[h][po][:],
                                 start=True, stop=True)
                xb = rc.tile([128, C], bf16, tag=f"xb{h}", name=f"xb{h}", bufs=3)
                nc.vector.tensor_tensor(xb[:], P['vt'][h][:], ws0[:],
                                        op=Alu.subtract)
                xbs.append(xb)
            for lev in range(7):
                for h in range(4):
                    mx = prc.tile([128, C], fp32, tag="dblx", name="mx", bufs=2)
                    nc.tensor.matmul(mx[:], P['A'][h][lev][:], xbs[h][:],
                                     start=True, stop=True)
                    xn = rc.tile([128, C], bf16, tag=f"xb{h}", name=f"xb{h}_{lev}",
                                 bufs=3)
                    nc.vector.tensor_tensor(xn[:], xbs[h][:], mx[:],
                                            op=(Alu.subtract if lev == 0 else Alu.add))
                    xbs[h] = xn
            for h in range(4):
                sup = prc.tile([128, DV], fp32, tag="tp", name="sup", bufs=2)
                nc.tensor.matmul(sup[:], P['kts'][h][:], xbs[h][:],
                                 start=True, stop=True)
                nc.vector.scalar_tensor_tensor(Sf[h][pn][:], Sf[h][po][:],
                                               P['bC'][h][:, 0:1], sup[:],
                                               op0=Alu.mult, op1=Alu.add)
                nc.vector.scalar_tensor_tensor(Sb[h][pn][:], Sf[h][po][:],
                                               P['bC'][h][:, 0:1], sup[:],
                                               op0=Alu.mult, op1=Alu.add)
                otp = prc.tile([128, C], fp32, tag="tp", name="otp", bufs=2)
                nc.tensor.matmul(otp[:], Sb[h][po][:], P['qtT'][h][:],
                                 start=True, stop=False)
                nc.tensor.matmul(otp[:], xbs[h][:], P['GtM'][h][:],
                                 start=False, stop=True)
                nc.vector.tensor_tensor(yb[h][:, ts], gateb[h][:, ts], otp[:],
                                        op=Alu.mult)

        pros = [None, None]
        for ci in range(NCH + 1):
            if ci < NCH:
                pros[ci % 2] = prologue(ci)
            if ci >= 1:
                spine(ci - 1, pros[(ci - 1) % 2])

        rctx.close()
        # ================= deferred RMSNorm + output projection =================
        with tc.tile_pool(name="post", bufs=2) as post, \
             tc.tile_pool(name="ppc", bufs=2, space="PSUM") as ppc:
            # PSUM tags: ssp(2) rbc(2) proj(2) = 6 banks
            ysqs = []
            for h in range(4):
                ysq = post.tile([128, T], bf16, tag="ysq", name="ysq", bufs=4)
                nc.scalar.activation(ysq[:], yb[h][:], Act.Square)
                ysqs.append(ysq)
            nrcs = []
            for h in range(4):
                nrc = post.tile([1, T], fp32, tag="nrc", name="nrc", bufs=4)
                for half in range(2):
                    ssp = ppc.tile([1, 512], fp32, tag="ssp", name="ssp")
                    nc.tensor.matmul(ssp[:], octb[:],
                                     ysqs[h][:, half * 512:(half + 1) * 512],
                                     start=True, stop=True)
                    nc.scalar.activation(nrc[:, half * 512:(half + 1) * 512],
                                         ssp[:], Act.Ln, scale=1.0 / DV,
                                         bias=epsnt[:, 0:1])
                nrcs.append(nrc)
            for h in range(4):
                rcb = post.tile([1, T], bf16, tag="rcb", name="rcb", bufs=4)
                nc.scalar.activation(rcb[:], nrcs[h][:], Act.Exp, scale=-0.5)
                for half in range(2):
                    rbc = ppc.tile([128, 512], fp32, tag="rbc", name="rbc")
                    nc.tensor.matmul(rbc[:], o1b[:], rcb[:, half * 512:(half + 1) * 512],
                                     start=True, stop=True)
                    nc.vector.scalar_tensor_tensor(yb[h][:, half * 512:(half + 1) * 512],
                                                   yb[h][:, half * 512:(half + 1) * 512],
                                                   nwt[:, 0:1], rbc[:],
                                                   op0=Alu.mult, op1=Alu.mult)
            # output projection
            for m in range(16):
                osb = post.tile([128, T], fp32, tag="osb", name="osb")
                for half in range(2):
                    ps = ppc.tile([128, 512], fp32, tag="proj", name="ops")
                    for k in range(4):
                        nc.tensor.matmul(ps[:], wot[k][:, m * 128:(m + 1) * 128],
                                         yb[k][:, half * 512:(half + 1) * 512],
                                         start=(k == 0), stop=(k == 3))
                    if half == 0:
                        nc.vector.tensor_copy(osb[:, 0:512], ps[:])
                    else:
                        nc.scalar.copy(osb[:, 512:1024], ps[:])
                dma(outT[m * 128:(m + 1) * 128, :], osb[:])

    nc.compile()
    return nc


def _prep_inputs(inputs):
    f32 = np.float32
    hs = np.asarray(inputs['hidden_states'], f32)
    tri = np.tril(np.ones((C, C), f32))
    maskM = (1.0 - tri).astype(f32)
    maskG = (1.0 - tri + np.eye(C, dtype=f32)).astype(f32)
    repl = np.zeros((NG, DK), f32)
    for n in range(NG):
        repl[n, n * GG:(n + 1) * GG] = 1.0
    sel8 = np.zeros((NG, NG * 128), f32)
    for n in range(NG):
        sel8[n, n * 128:(n + 1) * 128] = 1.0
    oh8 = np.zeros((DK, 64), f32)
    for i in range(8):
        oh8[:, i * 8 + i] = 1.0
    ident = np.eye(128, dtype=f32)

    maps = []
    for c in range(8):
        b, hg = c // 4, c % 4
        cols = slice(hg * NH * DK, (hg + 1) * NH * DK)
        gcols = slice(hg * NH * NG, (hg + 1) * NH * NG)
        hcols = slice(hg * NH, (hg + 1) * NH)
        nega = -np.exp(np.repeat(np.asarray(inputs['A_log'], f32)[hcols], NG))

        packf = np.zeros((128, NF), f32)
        cw = np.concatenate(
            [np.asarray(inputs['conv_q'], f32)[cols],
             np.asarray(inputs['conv_k'], f32)[cols],
             np.asarray(inputs['conv_v'], f32)[cols]], 1)  # [512, 12]
        for m in range(4):
            packf[:, PF_CW + m * 12:PF_CW + (m + 1) * 12] = cw[m * 128:(m + 1) * 128]
        packf[:, PF_BG:PF_BG + 4] = np.asarray(inputs['bg'], f32)[cols].reshape(NH, DV).T
        packf[:, PF_NW] = np.asarray(inputs['norm_w'], f32)
        packf[0:8, PF_NEGA:PF_NEGA + 4] = nega.reshape(NH, NG).T
        packf[0:8, PF_DTB:PF_DTB + 4] = (
            np.asarray(inputs['dt_bias'], f32)[gcols].reshape(NH, NG).T)
        packf[0:8, PF_SC8] = [1.0 / SCALE ** 2] * 4 + [1.0] * 4
        packf[0:8, PF_EPS8] = [1e-6 / SCALE ** 2] * 4 + [1e-6] * 4
        packf[0:1, PF_EPSN] = EPS
        packf[0:8, PF_REPL:PF_REPL + 128] = repl
        packf[:, PF_IDF:PF_IDF + 128] = ident
        packf[0:8, PF_SEL:PF_SEL + 1024] = sel8
        packf[:, PF_GMC:PF_GMC + 8] = repl.T

        packb = np.zeros((128, NB), f32)
        packb[:, PB_OH8:PB_OH8 + 64] = oh8
        packb[0:8, PB_S8B:PB_S8B + 1024] = sel8
        packb[:, PB_MM:PB_MM + 128] = maskM
        packb[:, PB_MG:PB_MG + 128] = maskG
        packb[:, PB_IDB:PB_IDB + 128] = ident
        packb[:, PB_OCT] = 1.0
        packb[0:1, PB_O1B:PB_O1B + 128] = 1.0

        wallm = np.concatenate(
            [np.asarray(inputs['Wq'], f32)[:, cols],
             np.asarray(inputs['Wk'], f32)[:, cols],
             np.asarray(inputs['Wv'], f32)[:, cols],
             np.asarray(inputs['Wg'], f32)[:, cols],
             np.asarray(inputs['Wf1'], f32),
             np.asarray(inputs['Wb'], f32)[:, hcols]], 1)

        m = {
            'hT': np.ascontiguousarray(hs[b].T).astype(BF),
            'wall': np.ascontiguousarray(wallm).astype(BF),
            'wo': np.ascontiguousarray(np.asarray(inputs['Wo'], f32)[cols, :]).astype(BF),
            'wf2': np.ascontiguousarray(np.asarray(inputs['Wf2'], f32)[:, gcols]).astype(BF),
            'packf': packf,
            'packb': packb.astype(BF),
        }
        maps.append(m)
    return maps


def kernel(**inputs):
    from concourse.bass_utils import run_bass_kernel_spmd
    if 'nc' not in _CACHE:
        _CACHE['nc'] = _build()
    nc = _CACHE['nc']
    maps = _prep_inputs(inputs)
    res = run_bass_kernel_spmd(nc, maps, list(range(8))).results
    out = np.zeros((B, T, D), np.float32)
    for c in range(8):
        out[c // 4] += res[c]['outT'].T.astype(np.float32)
    return out


# revision 21
# speedup vs baseline: 1.6957x; 1.0196x over previous
"""Grouped gated DeltaNet (KDA-style) on 8 TRN2 NeuronCores.

Sharding: core c -> (batch b = c//4, head-group hg = c%4 of 4 heads).
Per core: column-sharded projections (weights resident, loaded once),
short-conv+silu, l2norm, chunked gated delta-rule recurrence (chunk
C=128, group decay via 1-partition f32r broadcast matmuls + fused
sub/clamp, 16-partition group correlation matmuls, transpose-free A/B
doubling with interleaved triangular-solve application), deferred gated
RMSNorm (batched over T), row-shard output projection. Host sums 4
partials per batch.

Self-contained: B=2, T=1024, D=2048, H=16, DK=DV=128 hardcoded.
"""
import sys
sys.path.insert(0, '/opt/trn_rl_repo')
import numpy as np
import ml_dtypes
from contextlib import ExitStack

B, T, D = 2, 1024, 2048
H, DK, DV, GG = 16, 128, 128, 16
NG = DK // GG          # 8 gate groups per head
NH = 4                 # heads per core
C = 128                # chunk length
NCH = T // C
SCALE = DK ** -0.5
EPS = 1e-5

# packf fp32 column offsets
PF_CW = 0        # 4 blocks x 12
PF_BG = 48
PF_NW = 52
PF_NEGA = 53     # [8,4] (n,h)
PF_DTB = 57      # [8,4]
PF_SC8 = 61
PF_EPS8 = 62
PF_EPSN = 63
PF_REPL = 64     # [8,128]
PF_IDF = 192     # [128,128]
PF_SEL = 320     # [8,1024] group-selector
PF_GMC = 1344    # [128,8] group row-mask cols
NF = 1352
# packb bf16 column offsets
PB_OH8 = 0       # [128,64]
PB_S8B = 64      # [8,1024]
PB_MM = 1088     # [128,128]
PB_MG = 1216
PB_IDB = 1344
PB_OCT = 1472    # [128,1]
PB_O1B = 1473    # [1,128]
NB = 1601

WQ0, WK0, WV0, WG0, WF10, WB0 = 0, 512, 1024, 1536, 2048, 2176
WALLC = 2180

BF = ml_dtypes.bfloat16
_CACHE = {}

FP32_CHAIN = False   # fp32 x-chain fallback (precision)


def _build():
    import concourse.tile as tile
    from concourse import bacc, mybir

    fp32 = mybir.dt.float32
    f32r = mybir.dt.float32r
    bf16 = mybir.dt.bfloat16
    Alu = mybir.AluOpType
    Act = mybir.ActivationFunctionType

    nc = bacc.Bacc("TRN2", target_bir_lowering=False, debug=False, num_devices=8)
    dp = lambda n, sh, dt: nc.dram_tensor(n, sh, dt, kind="ExternalInput").ap()
    hT = dp("hT", [D, T], bf16)
    wall = dp("wall", [D, WALLC], bf16)
    wo = dp("wo", [NH * DV, D], bf16)
    wf2 = dp("wf2", [DV, NH * NG], bf16)
    packf = dp("packf", [128, NF], fp32)
    packb = dp("packb", [128, NB], bf16)
    outT = nc.dram_tensor("outT", [D, T], fp32, kind="ExternalOutput").ap()

    with tile.TileContext(nc) as tc, ExitStack() as ctx:
        pool = lambda name, bufs, space="SBUF": ctx.enter_context(
            tc.tile_pool(name=name, bufs=bufs, space=space))

        cons = pool("cons", 1)
        pers = pool("pers", 1)
        stp = pool("st", 1)

        dma = nc.sync.dma_start

        pf = cons.tile([128, NF], fp32, tag="packf", name="packf")
        dma(pf[:], packf[:])
        pb = cons.tile([128, NB], bf16, tag="packb", name="packb")
        dma(pb[:], packb[:])
        wf2t = cons.tile([128, 32], bf16, tag="wf2t", name="wf2t")
        dma(wf2t[:], wf2[:])

        cwt = lambda m: pf[:, PF_CW + m * 12: PF_CW + (m + 1) * 12]
        bgt = pf[:, PF_BG:PF_BG + 4]
        nwt = pf[:, PF_NW:PF_NW + 1]
        negat8 = lambda h: pf[0:8, PF_NEGA + h:PF_NEGA + h + 1]
        dtbt = pf[0:8, PF_DTB:PF_DTB + 4]
        sc8t = pf[0:8, PF_SC8:PF_SC8 + 1]
        eps8t = pf[0:8, PF_EPS8:PF_EPS8 + 1]
        epsnt = pf[0:1, PF_EPSN:PF_EPSN + 1]
        replt = pf[0:8, PF_REPL:PF_REPL + 128]
        idf = pf[:, PF_IDF:PF_IDF + 128]
        sel8 = pf[0:8, PF_SEL:PF_SEL + 1024]
        oh8t = pb[:, PB_OH8:PB_OH8 + 64]
        s8b = pb[0:8, PB_S8B:PB_S8B + 1024]
        mMt = pb[:, PB_MM:PB_MM + 128]
        mGt = pb[:, PB_MG:PB_MG + 128]
        idb = pb[:, PB_IDB:PB_IDB + 128]
        octb = pb[:, PB_OCT:PB_OCT + 1]
        o1b = pb[0:1, PB_O1B:PB_O1B + 128]
        gmct = pf[:, PF_GMC:PF_GMC + 8]

        ones32 = cons.tile([32, C], fp32, tag="ones32", name="ones32")
        nc.vector.memset(ones32[:], 1.0)

        # ---- persistent activations ----
        mk = lambda nm: [pers.tile([128, T], bf16, tag=f"{nm}{m}", name=f"{nm}{m}")
                         for m in range(4)]
        qb, kb, vb = mk("qb"), mk("kb"), mk("vb")
        gateb = mk("gateb")
        f1b = pers.tile([128, T], bf16, tag="f1b", name="f1b")
        gna8 = [pers.tile([8, T], bf16, tag=f"gna{h}", name=f"gna{h}")
                for h in range(4)]
        bsg = pers.tile([4, T], fp32, tag="bsg", name="bsg")

        # ---- state tiles (parity pairs) ----
        Sf = [[stp.tile([128, DV], fp32, tag=f"Sf{h}_{p}", name=f"Sf{h}_{p}")
               for p in range(2)] for h in range(4)]
        Sb = [[stp.tile([128, DV], bf16, tag=f"Sb{h}_{p}", name=f"Sb{h}_{p}")
               for p in range(2)] for h in range(4)]
        for h in range(4):
            nc.vector.memset(Sf[h][0][:], 0.0)
            nc.vector.memset(Sb[h][0][:], 0.0)

        # ================= projections =================
        with tc.tile_pool(name="htp", bufs=1) as htp, \
             tc.tile_pool(name="wallp", bufs=1) as wallp, \
             tc.tile_pool(name="convp", bufs=2) as convp, \
             tc.tile_pool(name="sqp", bufs=2) as sqp, \
             tc.tile_pool(name="smt", bufs=2) as smt, \
             tc.tile_pool(name="pps", bufs=1, space="PSUM") as pr:
            # PSUM tags: proj(2) sqs(1) bps(1) gps(1) nb(2) = 7 banks
            ssqsb = smt.tile([8, T], fp32, tag="ssqsb", name="ssqsb", bufs=1)
            nc.vector.memset(ssqsb[:], 0.0)
            ht, wt = [], []
            for k in range(16):
                t = htp.tile([128, T], bf16, tag=f"ht{k}", name=f"ht{k}")
                dma(t[:], hT[k * 128:(k + 1) * 128, :])
                ht.append(t)
                w = wallp.tile([128, WALLC], bf16, tag=f"wl{k}", name=f"wl{k}")
                dma(w[:], wall[k * 128:(k + 1) * 128, :])
                wt.append(w)

            qs = {}

            def project(col0, m, dst_bf16=None, conv_slot=None, pair=None,
                        gate_bias=None):
                xpad = None
                if conv_slot is not None:
                    xpad = convp.tile([128, T + 3], fp32, tag="xpad", name="xpad")
                    nc.vector.memset(xpad[:, 0:3], 0.0)
                for half in range(2):
                    ps = pr.tile([128, 512], fp32, tag="proj", name="projps", bufs=2)
                    for k in range(16):
                        nc.tensor.matmul(ps[:], wt[k][:, col0 + m * 128:col0 + (m + 1) * 128],
                                         ht[k][:, half * 512:(half + 1) * 512],
                                         start=(k == 0), stop=(k == 15))
                    if xpad is not None:
                        nc.scalar.copy(xpad[:, 3 + half * 512: 3 + (half + 1) * 512], ps[:])
                    elif gate_bias is not None:
                        nc.scalar.activation(dst_bf16[:, half * 512:(half + 1) * 512],
                                             ps[:], Act.Silu, bias=gate_bias)
                    else:
                        nc.scalar.copy(dst_bf16[:, half * 512:(half + 1) * 512], ps[:])
                if xpad is None:
                    return
                cwm = cwt(m)
                s = conv_slot * 4
                a = convp.tile([128, T], fp32, tag="acca", name="acca", bufs=1)
                bt = convp.tile([128, T], fp32, tag="accb", name="accb", bufs=1)
                nc.vector.tensor_scalar(a[:], xpad[:, 3:3 + T], cwm[:, s + 3:s + 4],
                                        None, op0=Alu.mult)
                cur, nxt = a, bt
                for kk in (2, 1, 0):
                    nc.vector.scalar_tensor_tensor(nxt[:], xpad[:, kk:kk + T],
                                                   cwm[:, s + kk:s + kk + 1], cur[:],
                                                   op0=Alu.mult, op1=Alu.add)
                    cur, nxt = nxt, cur
                if pair is None:
                    nc.scalar.activation(dst_bf16[:], cur[:], Act.Silu)
                else:
                    qsil = qb[pair] if pair < 4 else kb[pair - 4]
                    qs[pair] = qsil
                    nc.scalar.activation(qsil[:], cur[:], Act.Silu)
                    sq = sqp.tile([128, T], bf16, tag="sq", name="sq", bufs=1)
                    nc.scalar.activation(sq[:], qsil[:], Act.Square)
                    for half in range(2):
                        pss = pr.tile([8, 512], fp32, tag="sqs", name="sqs")
                        nc.tensor.matmul(pss[:], oh8t[:, pair * 8:pair * 8 + 8],
                                         sq[:, half * 512:(half + 1) * 512],
                                         start=True, stop=True)
                        nc.vector.tensor_tensor(ssqsb[:, half * 512:(half + 1) * 512],
                                                ssqsb[:, half * 512:(half + 1) * 512],
                                                pss[:], op=Alu.add)

            for m in range(4):
                project(WQ0, m, conv_slot=0, pair=m)
            for m in range(4):
                project(WK0, m, conv_slot=1, pair=4 + m)
            for m in range(4):
                project(WV0, m, dst_bf16=vb[m], conv_slot=2)
            for m in range(4):
                project(WG0, m, dst_bf16=gateb[m], gate_bias=bgt[:, m:m + 1])

            # f1 projection
            for half in range(2):
                ps = pr.tile([128, 512], fp32, tag="proj", name="f1ps", bufs=2)
                for k in range(16):
                    nc.tensor.matmul(ps[:], wt[k][:, WF10:WF10 + 128],
                                     ht[k][:, half * 512:(half + 1) * 512],
                                     start=(k == 0), stop=(k == 15))
                nc.scalar.copy(f1b[:, half * 512:(half + 1) * 512], ps[:])

            # beta (sigmoid) then gate-softplus chain, table-load friendly order
            for half in range(2):
                bps = pr.tile([4, 512], fp32, tag="bps", name="bps")
                for k in range(16):
                    nc.tensor.matmul(bps[:], wt[k][:, WB0:WB0 + 4],
                                     ht[k][:, half * 512:(half + 1) * 512],
                                     start=(k == 0), stop=(k == 15))
                nc.scalar.activation(bsg[:, half * 512:(half + 1) * 512], bps[:],
                                     Act.Sigmoid)
            sp1s = []
            for half in range(2):
                for h in range(4):
                    gps = pr.tile([8, 512], fp32, tag="gps", name="gps", bufs=2)
                    nc.tensor.matmul(gps[:], wf2t[:, h * 8:(h + 1) * 8],
                                     f1b[:, half * 512:(half + 1) * 512],
                                     start=True, stop=True)
                    spe = smt.tile([8, 512], bf16, tag="spe", name="spe", bufs=2)
                    nc.scalar.activation(spe[:], gps[:], Act.Exp,
                                         bias=dtbt[:, h:h + 1])
                    sp1 = smt.tile([8, 512], bf16, tag="sp1", name="sp1", bufs=8)
                    nc.vector.tensor_scalar(sp1[:], spe[:], 1.0, None, op0=Alu.add)
                    sp1s.append((half, h, sp1))
            # all Ln together: l2 normalizer + softplus logs
            nrm = smt.tile([8, T], fp32, tag="nrm", name="nrm", bufs=1)
            nc.scalar.activation(nrm[:], ssqsb[:], Act.Ln, scale=sc8t[:, 0:1],
                                 bias=eps8t[:, 0:1])
            for half, h, sp1 in sp1s:
                sp = smt.tile([8, 512], bf16, tag="sp", name="sp", bufs=2)
                nc.scalar.activation(sp[:], sp1[:], Act.Ln)
                nc.vector.tensor_scalar(gna8[h][:, half * 512:(half + 1) * 512],
                                        sp[:], negat8(h), None, op0=Alu.mult)
            recb = smt.tile([8, T], bf16, tag="recb", name="recb", bufs=1)
            nc.scalar.activation(recb[:], nrm[:], Act.Exp, scale=-0.5)
            for pair in range(8):
                dst = qb[pair] if pair < 4 else kb[pair - 4]
                for half in range(2):
                    nb = pr.tile([128, 512], fp32, tag="nb", name="nb", bufs=2)
                    nc.tensor.matmul(nb[:], s8b[:, pair * 128:(pair + 1) * 128],
                                     recb[:, half * 512:(half + 1) * 512],
                                     start=True, stop=True)
                    nc.vector.tensor_tensor(dst[:, half * 512:(half + 1) * 512],
                                            qs[pair][:, half * 512:(half + 1) * 512],
                                            nb[:], op=Alu.mult)

        # weights/hT pools closed; load wo for the tail now (overlaps recurrence)
        pers2 = ctx.enter_context(tc.tile_pool(name="pers2", bufs=1))
        yb = [pers2.tile([128, T], bf16, tag=f"yb{m}", name=f"yb{m}")
              for m in range(4)]
        wotp = ctx.enter_context(tc.tile_pool(name="wotp", bufs=1))
        wot = [wotp.tile([128, D], bf16, tag=f"wo{k}", name=f"wo{k}") for k in range(4)]
        for k in range(4):
            dma(wot[k][:], wo[k * 128:(k + 1) * 128, :])

        # ================= recurrence =================
        rc = ctx.enter_context(tc.tile_pool(name="rc", bufs=2))
        rr = ctx.enter_context(tc.tile_pool(name="rr", bufs=4))
        rctx = ExitStack()
        prc = rctx.enter_context(tc.tile_pool(name="prc", bufs=1, space="PSUM"))
        # PSUM tags: tp(2) big(2) dblx(2) mx(2) = 8 banks

        hdt = lambda nm, h, sh, dt=bf16, bufs=2: rc.tile(
            sh, dt, tag=f"{nm}{h}", name=f"{nm}{h}", bufs=bufs)

        MSKN = {1: 0, 3: 1, 5: 2, 6: 3, 7: 4}

        def prologue(ci):
            ts = slice(ci * C, (ci + 1) * C)
            P = {'ts': ts}
            cn8s = []
            for h in range(4):
                cn8 = hdt("cn8", h, [8, C], fp32)
                nc.vector.tensor_tensor_scan(cn8[:], ones32[0:8, :],
                                             gna8[h][:, ts], 0.0,
                                             op0=Alu.mult, op1=Alu.add)
                cn8s.append(cn8)
            cnt8s = []
            for h in range(4):
                cNtp = prc.tile([128, 8], fp32, tag="tp", name="cNtp", bufs=2)
                nc.tensor.transpose(cNtp[:], cn8s[h][:], idf[0:8, 0:8])
                cnt8 = hdt("cnt8", h, [128, 8], fp32)
                nc.scalar.copy(cnt8[:], cNtp[:])
                cnt8s.append(cnt8)
            b2p = prc.tile([128, 4], fp32, tag="tp", name="b2p", bufs=2)
            nc.tensor.transpose(b2p[:], bsg[:, ts], idf[0:4, 0:4])
            beta2 = rc.tile([128, 4], fp32, tag="beta2", name="beta2")
            nc.scalar.copy(beta2[:], b2p[:])
            kmsks = []
            for h in range(4):
                kmsk = rr.tile([128, 5 * C], bf16, tag="kmsk", name="kmsk")
                for n, j in MSKN.items():
                    dst = kmsk[:, j * C:(j + 1) * C]
                    if j % 2 == 0:
                        nc.scalar.mul(dst, kb[h][:, ts], gmct[:, n:n + 1])
                    else:
                        nc.vector.tensor_scalar(dst, kb[h][:, ts], gmct[:, n:n + 1],
                                                None, op0=Alu.mult)
                kmsks.append(kmsk)
            exp8s, exp8ks = [], []
            for h in range(4):
                e8 = hdt("exp8", h, [8, C], fp32)
                nc.scalar.activation(e8[:], cn8s[h][:], Act.Exp)
                exp8s.append(e8)
            for h in range(4):
                e8k = hdt("exp8k", h, [8, C], fp32)
                nc.scalar.activation(e8k[:], cn8s[h][:], Act.Exp, scale=-1.0,
                                     bias=cn8s[h][:, C - 1:C])
                exp8ks.append(e8k)
            bfks = []
            for h in range(4):
                bfk = prc.tile([128, 256], fp32, tag="tp", name="bfk", bufs=2)
                nc.tensor.matmul(bfk[:, 0:128], replt, exp8s[h][:],
                                 start=True, stop=True)
                nc.tensor.matmul(bfk[:, 128:256], replt, exp8ks[h][:],
                                 start=True, stop=True)
                bfks.append(bfk)
            P['bC'] = []
            for h in range(4):
                bC = hdt("bC", h, [128, 1], fp32)
                nc.scalar.copy(bC[:], bfks[h][:, 127:128])
                P['bC'].append(bC)
            P['Wt'], P['qtT'], kends = [], [], []
            for h in range(4):
                Wth = hdt("Wt", h, [128, C])
                nc.vector.tensor_tensor(Wth[:], kb[h][:, ts], bfks[h][:, 0:128],
                                        op=Alu.mult)
                P['Wt'].append(Wth)
                qtTh = hdt("qtT", h, [128, C])
                nc.vector.tensor_tensor(qtTh[:], qb[h][:, ts], bfks[h][:, 0:128],
                                        op=Alu.mult)
                P['qtT'].append(qtTh)
                kendh = hdt("kend", h, [128, C])
                nc.vector.tensor_tensor(kendh[:], kb[h][:, ts], bfks[h][:, 128:256],
                                        op=Alu.mult)
                kends.append(kendh)
            ealls = []
            for h in range(4):
                bcaL = prc.tile([128, 512], fp32, tag="big", name="bcaL", bufs=2)
                bcaH = prc.tile([128, 512], fp32, tag="big", name="bcaH", bufs=2)
                for n in range(8):
                    dst = bcaL if n < 4 else bcaH
                    nc.tensor.matmul(dst[:, (n % 4) * C:(n % 4 + 1) * C],
                                     sel8[:, n * 128:(n + 1) * 128], cn8s[h][:],
                                     start=True, stop=True)
                eallin = rr.tile([128, 8 * C], bf16, tag="eallin", name="eallin")
                for n in range(8):
                    bsrc = bcaL if n < 4 else bcaH
                    nc.scalar.activation(eallin[:, n * C:(n + 1) * C],
                                         bsrc[:, (n % 4) * C:(n % 4 + 1) * C],
                                         Act.Relu, scale=-1.0,
                                         bias=cnt8s[h][:, n:n + 1])
                eall = rr.tile([128, 8 * C], bf16, tag="eall", name="eall")
                nc.scalar.activation(eall[:], eallin[:], Act.Exp, scale=-1.0)
                ealls.append(eall)
            P['A'] = [[None] * 7 for _ in range(4)]
            P['GtM'] = []
            for h in range(4):
                pls = []
                for csrc in (kb[h], qb[h]):
                    pl = prc.tile([128, 512], fp32, tag="big", name="pall", bufs=2)
                    ph = prc.tile([128, 512], fp32, tag="big", name="pallh", bufs=2)
                    for n in range(8):
                        dst = pl if n < 4 else ph
                        if n in (0, 2, 4):
                            nc.tensor.matmul(dst[:, (n % 4) * C:(n % 4 + 1) * C],
                                             kb[h][16 * n:16 * (n + 1), ts],
                                             csrc[16 * n:16 * (n + 1), ts],
                                             start=True, stop=True)
                        else:
                            j = MSKN[n]
                            nc.tensor.matmul(dst[:, (n % 4) * C:(n % 4 + 1) * C],
                                             kmsks[h][:, j * C:(j + 1) * C],
                                             csrc[:, ts],
                                             start=True, stop=True)
                    pls.append((pl, ph))
                prods = []
                for x, (pl, ph) in enumerate(pls):
                    prod = rr.tile([128, 8 * C], bf16, tag="prod", name=f"prod{x}")
                    nc.vector.tensor_tensor(prod[:, 0:4 * C], ealls[h][:, 0:4 * C],
                                            pl[:], op=Alu.mult)
                    nc.vector.tensor_tensor(prod[:, 4 * C:], ealls[h][:, 4 * C:],
                                            ph[:], op=Alu.mult)
                    prods.append(prod)
                for x, prod in enumerate(prods):
                    t4 = rr.tile([128, 4 * C], bf16, tag="t4", name="t4")
                    nc.vector.tensor_tensor(t4[:], prod[:, :4 * C], prod[:, 4 * C:],
                                            op=Alu.add)
                    t2 = rr.tile([128, 2 * C], bf16, tag="t2", name="t2")
                    nc.vector.tensor_tensor(t2[:], t4[:, :2 * C], t4[:, 2 * C:],
                                            op=Alu.add)
                    t1 = rr.tile([128, C], bf16, tag="t1", name="t1")
                    nc.vector.tensor_tensor(t1[:], t2[:, :C], t2[:, C:], op=Alu.add)
                    if x == 0:
                        A0 = rc.tile([128, C], bf16, tag=f"A{h}", name=f"A{h}_0",
                                     bufs=14)
                        nc.vector.scalar_tensor_tensor(A0[:], t1[:],
                                                       beta2[:, h:h + 1], mMt[:],
                                                       op0=Alu.mult, op1=Alu.mult)
                        P['A'][h][0] = A0
                    else:
                        GtM = hdt("GtM", h, [128, C])
                        nc.vector.scalar_tensor_tensor(GtM[:], t1[:],
                                                       beta2[:, h:h + 1], mGt[:],
                                                       op0=Alu.mult, op1=Alu.mult)
                        P['GtM'].append(GtM)
            P['vt'], P['kts'] = [], []
            for h in range(4):
                vtp = prc.tile([128, C], bf16, tag="tp", name="vtp", bufs=2)
                nc.tensor.transpose(vtp[:], vb[h][:, ts], idb[:])
                vt = hdt("vt", h, [128, C])
                nc.vector.tensor_copy(vt[:], vtp[:])
                P['vt'].append(vt)
                ktp = prc.tile([128, C], bf16, tag="tp", name="ktp", bufs=2)
                nc.tensor.transpose(ktp[:], kends[h][:], idb[:])
                kts = hdt("kts", h, [128, C])
                nc.vector.tensor_scalar(kts[:], ktp[:], beta2[:, h:h + 1], None,
                                        op0=Alu.mult)
                P['kts'].append(kts)
            # A/B doubling chains (state-independent)
            Bs = [[None] * 6 for _ in range(4)]
            for h in range(4):
                b0p = prc.tile([128, C], bf16, tag="tp", name="b0p", bufs=2)
                nc.tensor.transpose(b0p[:], P['A'][h][0][:], idb[:])
                B0 = rc.tile([128, C], bf16, tag=f"B{h}", name=f"B{h}_0", bufs=3)
                nc.scalar.copy(B0[:], b0p[:])
                Bs[h][0] = B0
            for lev in range(1, 7):
                for h in range(4):
                    dbl = prc.tile([128, 256], fp32, tag="dblx", name="dbl", bufs=2)
                    nc.tensor.matmul(dbl[:, 0:128], Bs[h][lev - 1][:],
                                     P['A'][h][lev - 1][:], start=True, stop=True)
                    if lev < 6:
                        nc.tensor.matmul(dbl[:, 128:256], P['A'][h][lev - 1][:],
                                         Bs[h][lev - 1][:], start=True, stop=True)
                    An = rc.tile([128, C], bf16, tag=f"A{h}", name=f"A{h}_{lev}",
                                 bufs=14)
                    nc.scalar.copy(An[:], dbl[:, 0:128])
                    P['A'][h][lev] = An
                    if lev < 6:
                        Bn = rc.tile([128, C], bf16, tag=f"B{h}",
                                     name=f"B{h}_{lev}", bufs=3)
                        nc.vector.tensor_copy(Bn[:], dbl[:, 128:256])
                        Bs[h][lev] = Bn
            return P

        def spine(ci, P):
            ts = P['ts']
            po, pn = ci % 2, (ci + 1) % 2
            xbs = []
            for h in range(4):
                ws0 = prc.tile([128, C], fp32, tag="tp", name="ws0", bufs=2)
                nc.tensor.matmul(ws0[:], P['Wt'][h][:], Sb[h][po][:],
                                 start=True, stop=True)
                xb = rc.tile([128, C], bf16, tag=f"xb{h}", name=f"xb{h}", bufs=3)
                nc.vector.tensor_tensor(xb[:], P['vt'][h][:], ws0[:],
                                        op=Alu.subtract)
                xbs.append(xb)
            for lev in range(7):
                for h in range(4):
                    mx = prc.tile([128, C], fp32, tag="mx", name="mx", bufs=2)
                    nc.tensor.matmul(mx[:], P['A'][h][lev][:], xbs[h][:],
                                     start=True, stop=True)
                    xn = rc.tile([128, C], bf16, tag=f"xb{h}", name=f"xb{h}_{lev}",
                                 bufs=3)
                    nc.vector.tensor_tensor(xn[:], xbs[h][:], mx[:],
                                            op=(Alu.subtract if lev == 0 else Alu.add))
                    xbs[h] = xn
            for h in range(4):
                sup = prc.tile([128, DV], fp32, tag="tp", name="sup", bufs=2)
                nc.tensor.matmul(sup[:], P['kts'][h][:], xbs[h][:],
                                 start=True, stop=True)
                nc.vector.scalar_tensor_tensor(Sf[h][pn][:], Sf[h][po][:],
                                               P['bC'][h][:, 0:1], sup[:],
                                               op0=Alu.mult, op1=Alu.add)
                nc.vector.scalar_tensor_tensor(Sb[h][pn][:], Sf[h][po][:],
                                               P['bC'][h][:, 0:1], sup[:],
                                               op0=Alu.mult, op1=Alu.add)
                otp = prc.tile([128, C], fp32, tag="tp", name="otp", bufs=2)
                nc.tensor.matmul(otp[:], Sb[h][po][:], P['qtT'][h][:],
                                 start=True, stop=False)
                nc.tensor.matmul(otp[:], xbs[h][:], P['GtM'][h][:],
                                 start=False, stop=True)
                nc.vector.tensor_tensor(yb[h][:, ts], gateb[h][:, ts], otp[:],
                                        op=Alu.mult)

        pros = [None, None]
        for ci in range(NCH + 1):
            if ci < NCH:
                pros[ci % 2] = prologue(ci)
            if ci >= 1:
                spine(ci - 1, pros[(ci - 1) % 2])

        rctx.close()
        # ================= deferred RMSNorm + output projection =================
        with tc.tile_pool(name="post", bufs=2) as post, \
             tc.tile_pool(name="ppc", bufs=2, space="PSUM") as ppc:
            # PSUM tags: ssp(2) rbc(2) proj(2) = 6 banks
            ysqs = []
            for h in range(4):
                ysq = post.tile([128, T], bf16, tag="ysq", name="ysq", bufs=4)
                nc.scalar.activation(ysq[:], yb[h][:], Act.Square)
                ysqs.append(ysq)
            nrcs = []
            for h in range(4):
                nrc = post.tile([1, T], fp32, tag="nrc", name="nrc", bufs=4)
                for half in range(2):
                    ssp = ppc.tile([1, 512], fp32, tag="ssp", name="ssp")
                    nc.tensor.matmul(ssp[:], octb[:],
                                     ysqs[h][:, half * 512:(half + 1) * 512],
                                     start=True, stop=True)
                    nc.scalar.activation(nrc[:, half * 512:(half + 1) * 512],
                                         ssp[:], Act.Ln, scale=1.0 / DV,
                                         bias=epsnt[:, 0:1])
                nrcs.append(nrc)
            for h in range(4):
                rcb = post.tile([1, T], bf16, tag="rcb", name="rcb", bufs=4)
                nc.scalar.activation(rcb[:], nrcs[h][:], Act.Exp, scale=-0.5)
                for half in range(2):
                    rbc = ppc.tile([128, 512], fp32, tag="rbc", name="rbc")
                    nc.tensor.matmul(rbc[:], o1b[:], rcb[:, half * 512:(half + 1) * 512],
                                     start=True, stop=True)
                    nc.vector.scalar_tensor_tensor(yb[h][:, half * 512:(half + 1) * 512],
                                                   yb[h][:, half * 512:(half + 1) * 512],
                                                   nwt[:, 0:1], rbc[:],
                                                   op0=Alu.mult, op1=Alu.mult)
            # output projection
            for m in range(16):
                osb = post.tile([128, T], fp32, tag="osb", name="osb")
                for half in range(2):
                    ps = ppc.tile([128, 512], fp32, tag="proj", name="ops")
                    for k in range(4):
                        nc.tensor.matmul(ps[:], wot[k][:, m * 128:(m + 1) * 128],
                                         yb[k][:, half * 512:(half + 1) * 512],
                                         start=(k == 0), stop=(k == 3))
                    if half == 0:
                        nc.vector.tensor_copy(osb[:, 0:512], ps[:])
                    else:
                        nc.scalar.copy(osb[:, 512:1024], ps[:])
                dma(outT[m * 128:(m + 1) * 128, :], osb[:])

    nc.compile()
    return nc


def _prep_inputs(inputs):
    f32 = np.float32
    hs = np.asarray(inputs['hidden_states'], f32)
    tri = np.tril(np.ones((C, C), f32))
    maskM = (1.0 - tri).astype(f32)
    maskG = (1.0 - tri + np.eye(C, dtype=f32)).astype(f32)
    repl = np.zeros((NG, DK), f32)
    for n in range(NG):
        repl[n, n * GG:(n + 1) * GG] = 1.0
    sel8 = np.zeros((NG, NG * 128), f32)
    for n in range(NG):
        sel8[n, n * 128:(n + 1) * 128] = 1.0
    oh8 = np.zeros((DK, 64), f32)
    for i in range(8):
        oh8[:, i * 8 + i] = 1.0
    ident = np.eye(128, dtype=f32)

    maps = []
    for c in range(8):
        b, hg = c // 4, c % 4
        cols = slice(hg * NH * DK, (hg + 1) * NH * DK)
        gcols = slice(hg * NH * NG, (hg + 1) * NH * NG)
        hcols = slice(hg * NH, (hg + 1) * NH)
        nega = -np.exp(np.repeat(np.asarray(inputs['A_log'], f32)[hcols], NG))

        packf = np.zeros((128, NF), f32)
        cw = np.concatenate(
            [np.asarray(inputs['conv_q'], f32)[cols],
             np.asarray(inputs['conv_k'], f32)[cols],
             np.asarray(inputs['conv_v'], f32)[cols]], 1)  # [512, 12]
        for m in range(4):
            packf[:, PF_CW + m * 12:PF_CW + (m + 1) * 12] = cw[m * 128:(m + 1) * 128]
        packf[:, PF_BG:PF_BG + 4] = np.asarray(inputs['bg'], f32)[cols].reshape(NH, DV).T
        packf[:, PF_NW] = np.asarray(inputs['norm_w'], f32)
        packf[0:8, PF_NEGA:PF_NEGA + 4] = nega.reshape(NH, NG).T
        packf[0:8, PF_DTB:PF_DTB + 4] = (
            np.asarray(inputs['dt_bias'], f32)[gcols].reshape(NH, NG).T)
        packf[0:8, PF_SC8] = [1.0 / SCALE ** 2] * 4 + [1.0] * 4
        packf[0:8, PF_EPS8] = [1e-6 / SCALE ** 2] * 4 + [1e-6] * 4
        packf[0:1, PF_EPSN] = EPS
        packf[0:8, PF_REPL:PF_REPL + 128] = repl
        packf[:, PF_IDF:PF_IDF + 128] = ident
        packf[0:8, PF_SEL:PF_SEL + 1024] = sel8
        packf[:, PF_GMC:PF_GMC + 8] = repl.T

        packb = np.zeros((128, NB), f32)
        packb[:, PB_OH8:PB_OH8 + 64] = oh8
        packb[0:8, PB_S8B:PB_S8B + 1024] = sel8
        packb[:, PB_MM:PB_MM + 128] = maskM
        packb[:, PB_MG:PB_MG + 128] = maskG
        packb[:, PB_IDB:PB_IDB + 128] = ident
        packb[:, PB_OCT] = 1.0
        packb[0:1, PB_O1B:PB_O1B + 128] = 1.0

        wallm = np.concatenate(
            [np.asarray(inputs['Wq'], f32)[:, cols],
             np.asarray(inputs['Wk'], f32)[:, cols],
             np.asarray(inputs['Wv'], f32)[:, cols],
             np.asarray(inputs['Wg'], f32)[:, cols],
             np.asarray(inputs['Wf1'], f32),
             np.asarray(inputs['Wb'], f32)[:, hcols]], 1)

        m = {
            'hT': np.ascontiguousarray(hs[b].T).astype(BF),
            'wall': np.ascontiguousarray(wallm).astype(BF),
            'wo': np.ascontiguousarray(np.asarray(inputs['Wo'], f32)[cols, :]).astype(BF),
            'wf2': np.ascontiguousarray(np.asarray(inputs['Wf2'], f32)[:, gcols]).astype(BF),
            'packf': packf,
            'packb': packb.astype(BF),
        }
        maps.append(m)
    return maps


def kernel(**inputs):
    from concourse.bass_utils import run_bass_kernel_spmd
    if 'nc' not in _CACHE:
        _CACHE['nc'] = _build()
    nc = _CACHE['nc']
    maps = _prep_inputs(inputs)
    res = run_bass_kernel_spmd(nc, maps, list(range(8))).results
    out = np.zeros((B, T, D), np.float32)
    for c in range(8):
        out[c // 4] += res[c]['outT'].T.astype(np.float32)
    return out


# revision 22
# speedup vs baseline: 1.7665x; 1.0418x over previous
"""Grouped gated DeltaNet (KDA-style) on 8 TRN2 NeuronCores.

Sharding: core c -> (batch b = c//4, head-group hg = c%4 of 4 heads).
Per core: column-sharded projections (weights resident, loaded once),
short-conv+silu, l2norm, chunked gated delta-rule recurrence (chunk
C=128, group decay via 1-partition f32r broadcast matmuls + fused
sub/clamp, 16-partition group correlation matmuls, transpose-free A/B
doubling with interleaved triangular-solve application), deferred gated
RMSNorm (batched over T), row-shard output projection. Host sums 4
partials per batch.

Self-contained: B=2, T=1024, D=2048, H=16, DK=DV=128 hardcoded.
"""
import sys
sys.path.insert(0, '/opt/trn_rl_repo')
import numpy as np
import ml_dtypes
from contextlib import ExitStack

B, T, D = 2, 1024, 2048
H, DK, DV, GG = 16, 128, 128, 16
NG = DK // GG          # 8 gate groups per head
NH = 4                 # heads per core
C = 128                # chunk length
NCH = T // C
SCALE = DK ** -0.5
EPS = 1e-5

# packf fp32 column offsets
PF_CW = 0        # 4 blocks x 12
PF_BG = 48
PF_NW = 52
PF_NEGA = 53     # [8,4] (n,h)
PF_DTB = 57      # [8,4]
PF_SC8 = 61
PF_EPS8 = 62
PF_EPSN = 63
PF_REPL = 64     # [8,128]
PF_IDF = 192     # [128,128]
PF_SEL = 320     # [8,1024] group-selector
PF_GMC = 1344    # [128,8] group row-mask cols
NF = 1352
# packb bf16 column offsets
PB_OH8 = 0       # [128,64]
PB_S8B = 64      # [8,1024]
PB_MM = 1088     # [128,128]
PB_MG = 1216
PB_IDB = 1344
PB_OCT = 1472    # [128,1]
PB_O1B = 1473    # [1,128]
NB = 1601

WQ0, WK0, WV0, WG0, WF10, WB0 = 0, 512, 1024, 1536, 2048, 2176
WALLC = 2180

BF = ml_dtypes.bfloat16
_CACHE = {}

FP32_CHAIN = False   # fp32 x-chain fallback (precision)


def _build():
    import concourse.tile as tile
    from concourse import bacc, mybir

    fp32 = mybir.dt.float32
    f32r = mybir.dt.float32r
    bf16 = mybir.dt.bfloat16
    Alu = mybir.AluOpType
    Act = mybir.ActivationFunctionType

    nc = bacc.Bacc("TRN2", target_bir_lowering=False, debug=False, num_devices=8)
    dp = lambda n, sh, dt: nc.dram_tensor(n, sh, dt, kind="ExternalInput").ap()
    hT = dp("hT", [D, T], bf16)
    wall = dp("wall", [D, WALLC], bf16)
    wo = dp("wo", [NH * DV, D], bf16)
    wf2 = dp("wf2", [DV, NH * NG], bf16)
    packf = dp("packf", [128, NF], fp32)
    packb = dp("packb", [128, NB], bf16)
    outT = nc.dram_tensor("outT", [D, T], fp32, kind="ExternalOutput").ap()

    with tile.TileContext(nc) as tc, ExitStack() as ctx:
        pool = lambda name, bufs, space="SBUF": ctx.enter_context(
            tc.tile_pool(name=name, bufs=bufs, space=space))

        cons = pool("cons", 1)
        pers = pool("pers", 1)
        stp = pool("st", 1)

        dma = nc.sync.dma_start

        pf = cons.tile([128, NF], fp32, tag="packf", name="packf")
        dma(pf[:], packf[:])
        pb = cons.tile([128, NB], bf16, tag="packb", name="packb")
        dma(pb[:], packb[:])
        wf2t = cons.tile([128, 32], bf16, tag="wf2t", name="wf2t")
        dma(wf2t[:], wf2[:])

        cwt = lambda m: pf[:, PF_CW + m * 12: PF_CW + (m + 1) * 12]
        bgt = pf[:, PF_BG:PF_BG + 4]
        nwt = pf[:, PF_NW:PF_NW + 1]
        negat8 = lambda h: pf[0:8, PF_NEGA + h:PF_NEGA + h + 1]
        dtbt = pf[0:8, PF_DTB:PF_DTB + 4]
        sc8t = pf[0:8, PF_SC8:PF_SC8 + 1]
        eps8t = pf[0:8, PF_EPS8:PF_EPS8 + 1]
        epsnt = pf[0:1, PF_EPSN:PF_EPSN + 1]
        replt = pf[0:8, PF_REPL:PF_REPL + 128]
        idf = pf[:, PF_IDF:PF_IDF + 128]
        sel8 = pf[0:8, PF_SEL:PF_SEL + 1024]
        oh8t = pb[:, PB_OH8:PB_OH8 + 64]
        s8b = pb[0:8, PB_S8B:PB_S8B + 1024]
        mMt = pb[:, PB_MM:PB_MM + 128]
        mGt = pb[:, PB_MG:PB_MG + 128]
        idb = pb[:, PB_IDB:PB_IDB + 128]
        octb = pb[:, PB_OCT:PB_OCT + 1]
        o1b = pb[0:1, PB_O1B:PB_O1B + 128]
        gmct = pf[:, PF_GMC:PF_GMC + 8]

        ones32 = cons.tile([32, C], fp32, tag="ones32", name="ones32")
        nc.vector.memset(ones32[:], 1.0)

        # ---- persistent activations ----
        mk = lambda nm: [pers.tile([128, T], bf16, tag=f"{nm}{m}", name=f"{nm}{m}")
                         for m in range(4)]
        qb, kb, vb = mk("qb"), mk("kb"), mk("vb")
        gateb = mk("gateb")
        f1b = pers.tile([128, T], bf16, tag="f1b", name="f1b")
        gna8 = [pers.tile([8, T], bf16, tag=f"gna{h}", name=f"gna{h}")
                for h in range(4)]
        bsg = pers.tile([4, T], fp32, tag="bsg", name="bsg")

        # ---- state tiles (parity pairs) ----
        Sf = [[stp.tile([128, DV], fp32, tag=f"Sf{h}_{p}", name=f"Sf{h}_{p}")
               for p in range(2)] for h in range(4)]
        Sb = [[stp.tile([128, DV], bf16, tag=f"Sb{h}_{p}", name=f"Sb{h}_{p}")
               for p in range(2)] for h in range(4)]
        for h in range(4):
            nc.vector.memset(Sf[h][0][:], 0.0)
            nc.vector.memset(Sb[h][0][:], 0.0)

        # ================= projections =================
        with tc.tile_pool(name="htp", bufs=1) as htp, \
             tc.tile_pool(name="wallp", bufs=1) as wallp, \
             tc.tile_pool(name="convp", bufs=2) as convp, \
             tc.tile_pool(name="sqp", bufs=2) as sqp, \
             tc.tile_pool(name="smt", bufs=2) as smt, \
             tc.tile_pool(name="pps", bufs=1, space="PSUM") as pr:
            # PSUM tags: proj(2) sqs(1) bps(1) gps(1) nb(2) = 7 banks
            ssqsb = smt.tile([8, T], fp32, tag="ssqsb", name="ssqsb", bufs=1)
            nc.vector.memset(ssqsb[:], 0.0)
            ht, wt = [], []
            for k in range(16):
                t = htp.tile([128, T], bf16, tag=f"ht{k}", name=f"ht{k}")
                dma(t[:], hT[k * 128:(k + 1) * 128, :])
                ht.append(t)
                w = wallp.tile([128, WALLC], bf16, tag=f"wl{k}", name=f"wl{k}")
                dma(w[:], wall[k * 128:(k + 1) * 128, :])
                wt.append(w)

            qs = {}

            def project(col0, m, dst_bf16=None, conv_slot=None, pair=None,
                        gate_bias=None):
                xpad = None
                if conv_slot is not None:
                    xpad = convp.tile([128, T + 3], fp32, tag="xpad", name="xpad")
                    nc.vector.memset(xpad[:, 0:3], 0.0)
                for half in range(2):
                    ps = pr.tile([128, 512], fp32, tag="proj", name="projps", bufs=2)
                    for k in range(16):
                        nc.tensor.matmul(ps[:], wt[k][:, col0 + m * 128:col0 + (m + 1) * 128],
                                         ht[k][:, half * 512:(half + 1) * 512],
                                         start=(k == 0), stop=(k == 15))
                    if xpad is not None:
                        nc.scalar.copy(xpad[:, 3 + half * 512: 3 + (half + 1) * 512], ps[:])
                    elif gate_bias is not None:
                        nc.scalar.activation(dst_bf16[:, half * 512:(half + 1) * 512],
                                             ps[:], Act.Silu, bias=gate_bias)
                    else:
                        nc.scalar.copy(dst_bf16[:, half * 512:(half + 1) * 512], ps[:])
                if xpad is None:
                    return
                cwm = cwt(m)
                s = conv_slot * 4
                a = convp.tile([128, T], fp32, tag="acca", name="acca", bufs=1)
                bt = convp.tile([128, T], fp32, tag="accb", name="accb", bufs=1)
                nc.vector.tensor_scalar(a[:], xpad[:, 3:3 + T], cwm[:, s + 3:s + 4],
                                        None, op0=Alu.mult)
                cur, nxt = a, bt
                for kk in (2, 1, 0):
                    nc.vector.scalar_tensor_tensor(nxt[:], xpad[:, kk:kk + T],
                                                   cwm[:, s + kk:s + kk + 1], cur[:],
                                                   op0=Alu.mult, op1=Alu.add)
                    cur, nxt = nxt, cur
                if pair is None:
                    nc.scalar.activation(dst_bf16[:], cur[:], Act.Silu)
                else:
                    qsil = qb[pair] if pair < 4 else kb[pair - 4]
                    qs[pair] = qsil
                    nc.scalar.activation(qsil[:], cur[:], Act.Silu)
                    sq = sqp.tile([128, T], bf16, tag="sq", name="sq", bufs=1)
                    nc.scalar.activation(sq[:], qsil[:], Act.Square)
                    for half in range(2):
                        pss = pr.tile([8, 512], fp32, tag="sqs", name="sqs")
                        nc.tensor.matmul(pss[:], oh8t[:, pair * 8:pair * 8 + 8],
                                         sq[:, half * 512:(half + 1) * 512],
                                         start=True, stop=True)
                        nc.vector.tensor_tensor(ssqsb[:, half * 512:(half + 1) * 512],
                                                ssqsb[:, half * 512:(half + 1) * 512],
                                                pss[:], op=Alu.add)

            for m in range(4):
                project(WQ0, m, conv_slot=0, pair=m)
            for m in range(4):
                project(WK0, m, conv_slot=1, pair=4 + m)
            for m in range(4):
                project(WV0, m, dst_bf16=vb[m], conv_slot=2)
            for m in range(4):
                project(WG0, m, dst_bf16=gateb[m], gate_bias=bgt[:, m:m + 1])

            # f1 projection
            for half in range(2):
                ps = pr.tile([128, 512], fp32, tag="proj", name="f1ps", bufs=2)
                for k in range(16):
                    nc.tensor.matmul(ps[:], wt[k][:, WF10:WF10 + 128],
                                     ht[k][:, half * 512:(half + 1) * 512],
                                     start=(k == 0), stop=(k == 15))
                nc.scalar.copy(f1b[:, half * 512:(half + 1) * 512], ps[:])

            # beta (sigmoid) then gate-softplus chain, table-load friendly order
            for half in range(2):
                bps = pr.tile([4, 512], fp32, tag="bps", name="bps")
                for k in range(16):
                    nc.tensor.matmul(bps[:], wt[k][:, WB0:WB0 + 4],
                                     ht[k][:, half * 512:(half + 1) * 512],
                                     start=(k == 0), stop=(k == 15))
                nc.scalar.activation(bsg[:, half * 512:(half + 1) * 512], bps[:],
                                     Act.Sigmoid)
            sp1s = []
            for half in range(2):
                for h in range(4):
                    gps = pr.tile([8, 512], fp32, tag="gps", name="gps", bufs=2)
                    nc.tensor.matmul(gps[:], wf2t[:, h * 8:(h + 1) * 8],
                                     f1b[:, half * 512:(half + 1) * 512],
                                     start=True, stop=True)
                    spe = smt.tile([8, 512], bf16, tag="spe", name="spe", bufs=2)
                    nc.scalar.activation(spe[:], gps[:], Act.Exp,
                                         bias=dtbt[:, h:h + 1])
                    sp1 = smt.tile([8, 512], bf16, tag="sp1", name="sp1", bufs=8)
                    nc.vector.tensor_scalar(sp1[:], spe[:], 1.0, None, op0=Alu.add)
                    sp1s.append((half, h, sp1))
            # all Ln together: l2 normalizer + softplus logs
            nrm = smt.tile([8, T], fp32, tag="nrm", name="nrm", bufs=1)
            nc.scalar.activation(nrm[:], ssqsb[:], Act.Ln, scale=sc8t[:, 0:1],
                                 bias=eps8t[:, 0:1])
            for half, h, sp1 in sp1s:
                sp = smt.tile([8, 512], bf16, tag="sp", name="sp", bufs=2)
                nc.scalar.activation(sp[:], sp1[:], Act.Ln)
                nc.vector.tensor_scalar(gna8[h][:, half * 512:(half + 1) * 512],
                                        sp[:], negat8(h), None, op0=Alu.mult)
            recb = smt.tile([8, T], bf16, tag="recb", name="recb", bufs=1)
            nc.scalar.activation(recb[:], nrm[:], Act.Exp, scale=-0.5)
            for pair in range(8):
                dst = qb[pair] if pair < 4 else kb[pair - 4]
                for half in range(2):
                    nb = pr.tile([128, 512], fp32, tag="nb", name="nb", bufs=2)
                    nc.tensor.matmul(nb[:], s8b[:, pair * 128:(pair + 1) * 128],
                                     recb[:, half * 512:(half + 1) * 512],
                                     start=True, stop=True)
                    nc.vector.tensor_tensor(dst[:, half * 512:(half + 1) * 512],
                                            qs[pair][:, half * 512:(half + 1) * 512],
                                            nb[:], op=Alu.mult)

        # weights/hT pools closed; load wo for the tail now (overlaps recurrence)
        pers2 = ctx.enter_context(tc.tile_pool(name="pers2", bufs=1))
        yb = [pers2.tile([128, T], bf16, tag=f"yb{m}", name=f"yb{m}")
              for m in range(4)]
        wotp = ctx.enter_context(tc.tile_pool(name="wotp", bufs=1))
        wot = [wotp.tile([128, D], bf16, tag=f"wo{k}", name=f"wo{k}") for k in range(4)]
        for k in range(4):
            dma(wot[k][:], wo[k * 128:(k + 1) * 128, :])

        # ================= recurrence =================
        rc = ctx.enter_context(tc.tile_pool(name="rc", bufs=2))
        rr = ctx.enter_context(tc.tile_pool(name="rr", bufs=4))
        rctx = ExitStack()
        prc = rctx.enter_context(tc.tile_pool(name="prc", bufs=1, space="PSUM"))
        # PSUM tags: tp(2) big(2) dblx(2) mx(2) = 8 banks

        hdt = lambda nm, h, sh, dt=bf16, bufs=2: rc.tile(
            sh, dt, tag=f"{nm}{h}", name=f"{nm}{h}", bufs=bufs)

        MSKN = {1: 0, 3: 1, 5: 2, 6: 3, 7: 4}

        def prologue(ci):
            ts = slice(ci * C, (ci + 1) * C)
            P = {'ts': ts}
            cn8s = []
            for h in range(4):
                cn8 = hdt("cn8", h, [8, C], fp32)
                nc.vector.tensor_tensor_scan(cn8[:], ones32[0:8, :],
                                             gna8[h][:, ts], 0.0,
                                             op0=Alu.mult, op1=Alu.add)
                cn8s.append(cn8)
            cnt8s = []
            for h in range(4):
                cNtp = prc.tile([128, 8], fp32, tag="tp", name="cNtp", bufs=2)
                nc.tensor.transpose(cNtp[:], cn8s[h][:], idf[0:8, 0:8])
                cnt8 = hdt("cnt8", h, [128, 8], fp32)
                nc.scalar.copy(cnt8[:], cNtp[:])
                cnt8s.append(cnt8)
            b2p = prc.tile([128, 4], fp32, tag="tp", name="b2p", bufs=2)
            nc.tensor.transpose(b2p[:], bsg[:, ts], idf[0:4, 0:4])
            beta2 = rc.tile([128, 4], fp32, tag="beta2", name="beta2")
            nc.scalar.copy(beta2[:], b2p[:])
            kmsks = []
            for h in range(4):
                kmsk = rr.tile([128, 5 * C], bf16, tag="kmsk", name="kmsk")
                for n, j in MSKN.items():
                    dst = kmsk[:, j * C:(j + 1) * C]
                    nc.scalar.mul(dst, kb[h][:, ts], gmct[:, n:n + 1])
                kmsks.append(kmsk)
            exp8s, exp8ks = [], []
            for h in range(4):
                e8 = hdt("exp8", h, [8, C], fp32)
                nc.scalar.activation(e8[:], cn8s[h][:], Act.Exp)
                exp8s.append(e8)
            for h in range(4):
                e8k = hdt("exp8k", h, [8, C], fp32)
                nc.scalar.activation(e8k[:], cn8s[h][:], Act.Exp, scale=-1.0,
                                     bias=cn8s[h][:, C - 1:C])
                exp8ks.append(e8k)
            bfks = []
            for h in range(4):
                bfk = prc.tile([128, 256], fp32, tag="tp", name="bfk", bufs=2)
                nc.tensor.matmul(bfk[:, 0:128], replt, exp8s[h][:],
                                 start=True, stop=True)
                nc.tensor.matmul(bfk[:, 128:256], replt, exp8ks[h][:],
                                 start=True, stop=True)
                bfks.append(bfk)
            P['bC'] = []
            for h in range(4):
                bC = hdt("bC", h, [128, 1], fp32)
                nc.scalar.copy(bC[:], bfks[h][:, 127:128])
                P['bC'].append(bC)
            P['Wt'], P['qtT'], kends = [], [], []
            for h in range(4):
                Wth = hdt("Wt", h, [128, C])
                nc.vector.tensor_tensor(Wth[:], kb[h][:, ts], bfks[h][:, 0:128],
                                        op=Alu.mult)
                P['Wt'].append(Wth)
                qtTh = hdt("qtT", h, [128, C])
                nc.vector.tensor_tensor(qtTh[:], qb[h][:, ts], bfks[h][:, 0:128],
                                        op=Alu.mult)
                P['qtT'].append(qtTh)
                kendh = hdt("kend", h, [128, C])
                nc.vector.tensor_tensor(kendh[:], kb[h][:, ts], bfks[h][:, 128:256],
                                        op=Alu.mult)
                kends.append(kendh)
            ealls = []
            for h in range(4):
                bcaL = prc.tile([128, 512], fp32, tag="big", name="bcaL", bufs=2)
                bcaH = prc.tile([128, 512], fp32, tag="big", name="bcaH", bufs=2)
                for n in range(8):
                    dst = bcaL if n < 4 else bcaH
                    nc.tensor.matmul(dst[:, (n % 4) * C:(n % 4 + 1) * C],
                                     sel8[:, n * 128:(n + 1) * 128], cn8s[h][:],
                                     start=True, stop=True)
                eallin = rr.tile([128, 8 * C], bf16, tag="eallin", name="eallin")
                for n in range(8):
                    bsrc = bcaL if n < 4 else bcaH
                    nc.scalar.activation(eallin[:, n * C:(n + 1) * C],
                                         bsrc[:, (n % 4) * C:(n % 4 + 1) * C],
                                         Act.Relu, scale=-1.0,
                                         bias=cnt8s[h][:, n:n + 1])
                eall = rr.tile([128, 8 * C], bf16, tag="eall", name="eall")
                nc.scalar.activation(eall[:], eallin[:], Act.Exp, scale=-1.0)
                ealls.append(eall)
            P['A'] = [[None] * 7 for _ in range(4)]
            P['GtM'] = []
            for h in range(4):
                pls = []
                for csrc in (kb[h], qb[h]):
                    pl = prc.tile([128, 512], fp32, tag="big", name="pall", bufs=2)
                    ph = prc.tile([128, 512], fp32, tag="big", name="pallh", bufs=2)
                    for n in range(8):
                        dst = pl if n < 4 else ph
                        if n in (0, 2, 4):
                            nc.tensor.matmul(dst[:, (n % 4) * C:(n % 4 + 1) * C],
                                             kb[h][16 * n:16 * (n + 1), ts],
                                             csrc[16 * n:16 * (n + 1), ts],
                                             start=True, stop=True)
                        else:
                            j = MSKN[n]
                            nc.tensor.matmul(dst[:, (n % 4) * C:(n % 4 + 1) * C],
                                             kmsks[h][:, j * C:(j + 1) * C],
                                             csrc[:, ts],
                                             start=True, stop=True)
                    pls.append((pl, ph))
                prods = []
                for x, (pl, ph) in enumerate(pls):
                    prod = rr.tile([128, 8 * C], bf16, tag="prod", name=f"prod{x}")
                    nc.vector.tensor_tensor(prod[:, 0:4 * C], ealls[h][:, 0:4 * C],
                                            pl[:], op=Alu.mult)
                    nc.vector.tensor_tensor(prod[:, 4 * C:], ealls[h][:, 4 * C:],
                                            ph[:], op=Alu.mult)
                    prods.append(prod)
                for x, prod in enumerate(prods):
                    t4 = rr.tile([128, 4 * C], bf16, tag="t4", name="t4")
                    nc.vector.tensor_tensor(t4[:], prod[:, :4 * C], prod[:, 4 * C:],
                                            op=Alu.add)
                    t2 = rr.tile([128, 2 * C], bf16, tag="t2", name="t2")
                    nc.vector.tensor_tensor(t2[:], t4[:, :2 * C], t4[:, 2 * C:],
                                            op=Alu.add)
                    t1 = rr.tile([128, C], bf16, tag="t1", name="t1")
                    nc.vector.tensor_tensor(t1[:], t2[:, :C], t2[:, C:], op=Alu.add)
                    if x == 0:
                        A0 = rc.tile([128, C], bf16, tag=f"A{h}", name=f"A{h}_0",
                                     bufs=14)
                        nc.vector.scalar_tensor_tensor(A0[:], t1[:],
                                                       beta2[:, h:h + 1], mMt[:],
                                                       op0=Alu.mult, op1=Alu.mult)
                        P['A'][h][0] = A0
                    else:
                        GtM = hdt("GtM", h, [128, C])
                        nc.vector.scalar_tensor_tensor(GtM[:], t1[:],
                                                       beta2[:, h:h + 1], mGt[:],
                                                       op0=Alu.mult, op1=Alu.mult)
                        P['GtM'].append(GtM)
            P['vt'], P['kts'] = [], []
            for h in range(4):
                vtp = prc.tile([128, C], bf16, tag="tp", name="vtp", bufs=2)
                nc.tensor.transpose(vtp[:], vb[h][:, ts], idb[:])
                vt = hdt("vt", h, [128, C])
                nc.scalar.copy(vt[:], vtp[:])
                P['vt'].append(vt)
                ktp = prc.tile([128, C], bf16, tag="tp", name="ktp", bufs=2)
                nc.tensor.transpose(ktp[:], kends[h][:], idb[:])
                kts = hdt("kts", h, [128, C])
                nc.vector.tensor_scalar(kts[:], ktp[:], beta2[:, h:h + 1], None,
                                        op0=Alu.mult)
                P['kts'].append(kts)
            # A/B doubling chains (state-independent)
            Bs = [[None] * 6 for _ in range(4)]
            for h in range(4):
                b0p = prc.tile([128, C], bf16, tag="tp", name="b0p", bufs=2)
                nc.tensor.transpose(b0p[:], P['A'][h][0][:], idb[:])
                B0 = rc.tile([128, C], bf16, tag=f"B{h}", name=f"B{h}_0", bufs=3)
                nc.scalar.copy(B0[:], b0p[:])
                Bs[h][0] = B0
            for lev in range(1, 7):
                for h in range(4):
                    dbl = prc.tile([128, 256], fp32, tag="dblx", name="dbl", bufs=2)
                    nc.tensor.matmul(dbl[:, 0:128], Bs[h][lev - 1][:],
                                     P['A'][h][lev - 1][:], start=True, stop=True)
                    if lev < 6:
                        nc.tensor.matmul(dbl[:, 128:256], P['A'][h][lev - 1][:],
                                         Bs[h][lev - 1][:], start=True, stop=True)
                    An = rc.tile([128, C], bf16, tag=f"A{h}", name=f"A{h}_{lev}",
                                 bufs=14)
                    nc.scalar.copy(An[:], dbl[:, 0:128])
                    P['A'][h][lev] = An
                    if lev < 6:
                        Bn = rc.tile([128, C], bf16, tag=f"B{h}",
                                     name=f"B{h}_{lev}", bufs=3)
                        nc.scalar.copy(Bn[:], dbl[:, 128:256])
                        Bs[h][lev] = Bn
            return P

        def spine(ci, P):
            ts = P['ts']
            po, pn = ci % 2, (ci + 1) % 2
            xbs = []
            for h in range(4):
                ws0 = prc.tile([128, C], fp32, tag="tp", name="ws0", bufs=2)
                nc.tensor.matmul(ws0[:], P['Wt'][h][:], Sb[h][po][:],
                                 start=True, stop=True)
                xb = rc.tile([128, C], bf16, tag=f"xb{h}", name=f"xb{h}", bufs=3)
                nc.vector.tensor_tensor(xb[:], P['vt'][h][:], ws0[:],
                                        op=Alu.subtract)
                xbs.append(xb)
            for lev in range(7):
                for h in range(4):
                    mx = prc.tile([128, C], fp32, tag="mx", name="mx", bufs=2)
                    nc.tensor.matmul(mx[:], P['A'][h][lev][:], xbs[h][:],
                                     start=True, stop=True)
                    xn = rc.tile([128, C], bf16, tag=f"xb{h}", name=f"xb{h}_{lev}",
                                 bufs=3)
                    nc.vector.tensor_tensor(xn[:], xbs[h][:], mx[:],
                                            op=(Alu.subtract if lev == 0 else Alu.add))
                    xbs[h] = xn
            for h in range(4):
                sup = prc.tile([128, DV], fp32, tag="tp", name="sup", bufs=2)
                nc.tensor.matmul(sup[:], P['kts'][h][:], xbs[h][:],
                                 start=True, stop=True)
                nc.vector.scalar_tensor_tensor(Sf[h][pn][:], Sf[h][po][:],
                                               P['bC'][h][:, 0:1], sup[:],
                                               op0=Alu.mult, op1=Alu.add)
                nc.vector.scalar_tensor_tensor(Sb[h][pn][:], Sf[h][po][:],
                                               P['bC'][h][:, 0:1], sup[:],
                                               op0=Alu.mult, op1=Alu.add)
                otp = prc.tile([128, C], fp32, tag="tp", name="otp", bufs=2)
                nc.tensor.matmul(otp[:], Sb[h][po][:], P['qtT'][h][:],
                                 start=True, stop=False)
                nc.tensor.matmul(otp[:], xbs[h][:], P['GtM'][h][:],
                                 start=False, stop=True)
                nc.vector.tensor_tensor(yb[h][:, ts], gateb[h][:, ts], otp[:],
                                        op=Alu.mult)

        pros = [None, None]
        for ci in range(NCH + 1):
            if ci < NCH:
                pros[ci % 2] = prologue(ci)
            if ci >= 1:
                spine(ci - 1, pros[(ci - 1) % 2])

        rctx.close()
        # ================= deferred RMSNorm + output projection =================
        with tc.tile_pool(name="post", bufs=2) as post, \
             tc.tile_pool(name="ppc", bufs=2, space="PSUM") as ppc:
            # PSUM tags: ssp(2) rbc(2) proj(2) = 6 banks
            ysqs = []
            for h in range(4):
                ysq = post.tile([128, T], bf16, tag="ysq", name="ysq", bufs=4)
                nc.scalar.activation(ysq[:], yb[h][:], Act.Square)
                ysqs.append(ysq)
            nrcs = []
            for h in range(4):
                nrc = post.tile([1, T], fp32, tag="nrc", name="nrc", bufs=4)
                for half in range(2):
                    ssp = ppc.tile([1, 512], fp32, tag="ssp", name="ssp")
                    nc.tensor.matmul(ssp[:], octb[:],
                                     ysqs[h][:, half * 512:(half + 1) * 512],
                                     start=True, stop=True)
                    nc.scalar.activation(nrc[:, half * 512:(half + 1) * 512],
                                         ssp[:], Act.Ln, scale=1.0 / DV,
                                         bias=epsnt[:, 0:1])
                nrcs.append(nrc)
            for h in range(4):
                rcb = post.tile([1, T], bf16, tag="rcb", name="rcb", bufs=4)
                nc.scalar.activation(rcb[:], nrcs[h][:], Act.Exp, scale=-0.5)
                for half in range(2):
                    rbc = ppc.tile([128, 512], fp32, tag="rbc", name="rbc")
                    nc.tensor.matmul(rbc[:], o1b[:], rcb[:, half * 512:(half + 1) * 512],
                                     start=True, stop=True)
                    nc.vector.scalar_tensor_tensor(yb[h][:, half * 512:(half + 1) * 512],
                                                   yb[h][:, half * 512:(half + 1) * 512],
                                                   nwt[:, 0:1], rbc[:],
                                                   op0=Alu.mult, op1=Alu.mult)
            # output projection
            for m in range(16):
                osb = post.tile([128, T], fp32, tag="osb", name="osb")
                for half in range(2):
                    ps = ppc.tile([128, 512], fp32, tag="proj", name="ops")
                    for k in range(4):
                        nc.tensor.matmul(ps[:], wot[k][:, m * 128:(m + 1) * 128],
                                         yb[k][:, half * 512:(half + 1) * 512],
                                         start=(k == 0), stop=(k == 3))
                    if half == 0:
                        nc.vector.tensor_copy(osb[:, 0:512], ps[:])
                    else:
                        nc.scalar.copy(osb[:, 512:1024], ps[:])
                dma(outT[m * 128:(m + 1) * 128, :], osb[:])

    nc.compile()
    return nc


def _prep_inputs(inputs):
    f32 = np.float32
    hs = np.asarray(inputs['hidden_states'], f32)
    tri = np.tril(np.ones((C, C), f32))
    maskM = (1.0 - tri).astype(f32)
    maskG = (1.0 - tri + np.eye(C, dtype=f32)).astype(f32)
    repl = np.zeros((NG, DK), f32)
    for n in range(NG):
        repl[n, n * GG:(n + 1) * GG] = 1.0
    sel8 = np.zeros((NG, NG * 128), f32)
    for n in range(NG):
        sel8[n, n * 128:(n + 1) * 128] = 1.0
    oh8 = np.zeros((DK, 64), f32)
    for i in range(8):
        oh8[:, i * 8 + i] = 1.0
    ident = np.eye(128, dtype=f32)

    maps = []
    for c in range(8):
        b, hg = c // 4, c % 4
        cols = slice(hg * NH * DK, (hg + 1) * NH * DK)
        gcols = slice(hg * NH * NG, (hg + 1) * NH * NG)
        hcols = slice(hg * NH, (hg + 1) * NH)
        nega = -np.exp(np.repeat(np.asarray(inputs['A_log'], f32)[hcols], NG))

        packf = np.zeros((128, NF), f32)
        cw = np.concatenate(
            [np.asarray(inputs['conv_q'], f32)[cols],
             np.asarray(inputs['conv_k'], f32)[cols],
             np.asarray(inputs['conv_v'], f32)[cols]], 1)  # [512, 12]
        for m in range(4):
            packf[:, PF_CW + m * 12:PF_CW + (m + 1) * 12] = cw[m * 128:(m + 1) * 128]
        packf[:, PF_BG:PF_BG + 4] = np.asarray(inputs['bg'], f32)[cols].reshape(NH, DV).T
        packf[:, PF_NW] = np.asarray(inputs['norm_w'], f32)
        packf[0:8, PF_NEGA:PF_NEGA + 4] = nega.reshape(NH, NG).T
        packf[0:8, PF_DTB:PF_DTB + 4] = (
            np.asarray(inputs['dt_bias'], f32)[gcols].reshape(NH, NG).T)
        packf[0:8, PF_SC8] = [1.0 / SCALE ** 2] * 4 + [1.0] * 4
        packf[0:8, PF_EPS8] = [1e-6 / SCALE ** 2] * 4 + [1e-6] * 4
        packf[0:1, PF_EPSN] = EPS
        packf[0:8, PF_REPL:PF_REPL + 128] = repl
        packf[:, PF_IDF:PF_IDF + 128] = ident
        packf[0:8, PF_SEL:PF_SEL + 1024] = sel8
        packf[:, PF_GMC:PF_GMC + 8] = repl.T

        packb = np.zeros((128, NB), f32)
        packb[:, PB_OH8:PB_OH8 + 64] = oh8
        packb[0:8, PB_S8B:PB_S8B + 1024] = sel8
        packb[:, PB_MM:PB_MM + 128] = maskM
        packb[:, PB_MG:PB_MG + 128] = maskG
        packb[:, PB_IDB:PB_IDB + 128] = ident
        packb[:, PB_OCT] = 1.0
        packb[0:1, PB_O1B:PB_O1B + 128] = 1.0

        wallm = np.concatenate(
            [np.asarray(inputs['Wq'], f32)[:, cols],
             np.asarray(inputs['Wk'], f32)[:, cols],
             np.asarray(inputs['Wv'], f32)[:, cols],
             np.asarray(inputs['Wg'], f32)[:, cols],
             np.asarray(inputs['Wf1'], f32),
             np.asarray(inputs['Wb'], f32)[:, hcols]], 1)

        m = {
            'hT': np.ascontiguousarray(hs[b].T).astype(BF),
            'wall': np.ascontiguousarray(wallm).astype(BF),
            'wo': np.ascontiguousarray(np.asarray(inputs['Wo'], f32)[cols, :]).astype(BF),
            'wf2': np.ascontiguousarray(np.asarray(inputs['Wf2'], f32)[:, gcols]).astype(BF),
            'packf': packf,
            'packb': packb.astype(BF),
        }
        maps.append(m)
    return maps


def kernel(**inputs):
    from concourse.bass_utils import run_bass_kernel_spmd
    if 'nc' not in _CACHE:
        _CACHE['nc'] = _build()
    nc = _CACHE['nc']
    maps = _prep_inputs(inputs)
    res = run_bass_kernel_spmd(nc, maps, list(range(8))).results
    out = np.zeros((B, T, D), np.float32)
    for c in range(8):
        out[c // 4] += res[c]['outT'].T.astype(np.float32)
    return out


# revision 23
# speedup vs baseline: 1.7847x; 1.0103x over previous
"""Grouped gated DeltaNet (KDA-style) on 8 TRN2 NeuronCores.

Sharding: core c -> (batch b = c//4, head-group hg = c%4 of 4 heads).
Per core: column-sharded projections (weights resident, loaded once),
short-conv+silu, l2norm, chunked gated delta-rule recurrence (chunk
C=128, group decay via 1-partition f32r broadcast matmuls + fused
sub/clamp, 16-partition group correlation matmuls, transpose-free A/B
doubling with interleaved triangular-solve application), deferred gated
RMSNorm (batched over T), row-shard output projection. Host sums 4
partials per batch.

Self-contained: B=2, T=1024, D=2048, H=16, DK=DV=128 hardcoded.
"""
import sys
sys.path.insert(0, '/opt/trn_rl_repo')
import numpy as np
import ml_dtypes
from contextlib import ExitStack

B, T, D = 2, 1024, 2048
H, DK, DV, GG = 16, 128, 128, 16
NG = DK // GG          # 8 gate groups per head
NH = 4                 # heads per core
C = 128                # chunk length
NCH = T // C
SCALE = DK ** -0.5
EPS = 1e-5

# packf fp32 column offsets
PF_CW = 0        # 4 blocks x 12
PF_BG = 48
PF_NW = 52
PF_NEGA = 53     # [8,4] (n,h)
PF_DTB = 57      # [8,4]
PF_SC8 = 61
PF_EPS8 = 62
PF_EPSN = 63
PF_REPL = 64     # [8,128]
PF_IDF = 192     # [128,128]
PF_SEL = 320     # [8,1024] group-selector
PF_GMC = 1344    # [128,8] group row-mask cols
NF = 1352
# packb bf16 column offsets
PB_OH8 = 0       # [128,64]
PB_S8B = 64      # [8,1024]
PB_MM = 1088     # [128,128]
PB_MG = 1216
PB_IDB = 1344
PB_OCT = 1472    # [128,1]
PB_O1B = 1473    # [1,128]
NB = 1601

WQ0, WK0, WV0, WG0, WF10, WB0 = 0, 512, 1024, 1536, 2048, 2176
WALLC = 2180

BF = ml_dtypes.bfloat16
_CACHE = {}

FP32_CHAIN = False   # fp32 x-chain fallback (precision)


def _build():
    import concourse.tile as tile
    from concourse import bacc, mybir

    fp32 = mybir.dt.float32
    f32r = mybir.dt.float32r
    bf16 = mybir.dt.bfloat16
    Alu = mybir.AluOpType
    Act = mybir.ActivationFunctionType

    nc = bacc.Bacc("TRN2", target_bir_lowering=False, debug=False, num_devices=8)
    dp = lambda n, sh, dt: nc.dram_tensor(n, sh, dt, kind="ExternalInput").ap()
    hT = dp("hT", [D, T], bf16)
    wall = dp("wall", [D, WALLC], bf16)
    wo = dp("wo", [NH * DV, D], bf16)
    wf2 = dp("wf2", [DV, NH * NG], bf16)
    packf = dp("packf", [128, NF], fp32)
    packb = dp("packb", [128, NB], bf16)
    outT = nc.dram_tensor("outT", [D, T], fp32, kind="ExternalOutput").ap()

    with tile.TileContext(nc) as tc, ExitStack() as ctx:
        pool = lambda name, bufs, space="SBUF": ctx.enter_context(
            tc.tile_pool(name=name, bufs=bufs, space=space))

        cons = pool("cons", 1)
        pers = pool("pers", 1)
        stp = pool("st", 1)

        dma = nc.sync.dma_start

        pf = cons.tile([128, NF], fp32, tag="packf", name="packf")
        dma(pf[:], packf[:])
        pb = cons.tile([128, NB], bf16, tag="packb", name="packb")
        dma(pb[:], packb[:])
        wf2t = cons.tile([128, 32], bf16, tag="wf2t", name="wf2t")
        dma(wf2t[:], wf2[:])

        cwt = lambda m: pf[:, PF_CW + m * 12: PF_CW + (m + 1) * 12]
        bgt = pf[:, PF_BG:PF_BG + 4]
        nwt = pf[:, PF_NW:PF_NW + 1]
        negat8 = lambda h: pf[0:8, PF_NEGA + h:PF_NEGA + h + 1]
        dtbt = pf[0:8, PF_DTB:PF_DTB + 4]
        sc8t = pf[0:8, PF_SC8:PF_SC8 + 1]
        eps8t = pf[0:8, PF_EPS8:PF_EPS8 + 1]
        epsnt = pf[0:1, PF_EPSN:PF_EPSN + 1]
        replt = pf[0:8, PF_REPL:PF_REPL + 128]
        idf = pf[:, PF_IDF:PF_IDF + 128]
        sel8 = pf[0:8, PF_SEL:PF_SEL + 1024]
        oh8t = pb[:, PB_OH8:PB_OH8 + 64]
        s8b = pb[0:8, PB_S8B:PB_S8B + 1024]
        mMt = pb[:, PB_MM:PB_MM + 128]
        mGt = pb[:, PB_MG:PB_MG + 128]
        idb = pb[:, PB_IDB:PB_IDB + 128]
        octb = pb[:, PB_OCT:PB_OCT + 1]
        o1b = pb[0:1, PB_O1B:PB_O1B + 128]
        gmct = pf[:, PF_GMC:PF_GMC + 8]

        ones32 = cons.tile([32, C], fp32, tag="ones32", name="ones32")
        nc.vector.memset(ones32[:], 1.0)

        # ---- persistent activations ----
        mk = lambda nm: [pers.tile([128, T], bf16, tag=f"{nm}{m}", name=f"{nm}{m}")
                         for m in range(4)]
        qb, kb, vb = mk("qb"), mk("kb"), mk("vb")
        gateb = mk("gateb")
        f1b = pers.tile([128, T], bf16, tag="f1b", name="f1b")
        gna8 = [pers.tile([8, T], bf16, tag=f"gna{h}", name=f"gna{h}")
                for h in range(4)]
        bsg = pers.tile([4, T], fp32, tag="bsg", name="bsg")

        # ---- state tiles (parity pairs) ----
        Sf = [[stp.tile([128, DV], fp32, tag=f"Sf{h}_{p}", name=f"Sf{h}_{p}")
               for p in range(2)] for h in range(4)]
        Sb = [[stp.tile([128, DV], bf16, tag=f"Sb{h}_{p}", name=f"Sb{h}_{p}")
               for p in range(2)] for h in range(4)]
        for h in range(4):
            nc.vector.memset(Sf[h][0][:], 0.0)
            nc.vector.memset(Sb[h][0][:], 0.0)

        # ================= projections =================
        with tc.tile_pool(name="htp", bufs=1) as htp, \
             tc.tile_pool(name="wallp", bufs=1) as wallp, \
             tc.tile_pool(name="convp", bufs=2) as convp, \
             tc.tile_pool(name="sqp", bufs=2) as sqp, \
             tc.tile_pool(name="smt", bufs=2) as smt, \
             tc.tile_pool(name="pps", bufs=1, space="PSUM") as pr:
            # PSUM tags: proj(2) sqs(1) bps(1) gps(1) nb(2) = 7 banks
            ssqsb = smt.tile([8, T], fp32, tag="ssqsb", name="ssqsb", bufs=1)
            nc.vector.memset(ssqsb[:], 0.0)
            ht, wt = [], []
            for k in range(16):
                t = htp.tile([128, T], bf16, tag=f"ht{k}", name=f"ht{k}")
                dma(t[:], hT[k * 128:(k + 1) * 128, :])
                ht.append(t)
                w = wallp.tile([128, WALLC], bf16, tag=f"wl{k}", name=f"wl{k}")
                dma(w[:], wall[k * 128:(k + 1) * 128, :])
                wt.append(w)

            qs = {}

            def project(col0, m, dst_bf16=None, conv_slot=None, pair=None,
                        gate_bias=None):
                xpad = None
                if conv_slot is not None:
                    xpad = convp.tile([128, T + 3], fp32, tag="xpad", name="xpad")
                    nc.vector.memset(xpad[:, 0:3], 0.0)
                for half in range(2):
                    ps = pr.tile([128, 512], fp32, tag="proj", name="projps", bufs=2)
                    for k in range(16):
                        nc.tensor.matmul(ps[:], wt[k][:, col0 + m * 128:col0 + (m + 1) * 128],
                                         ht[k][:, half * 512:(half + 1) * 512],
                                         start=(k == 0), stop=(k == 15))
                    if xpad is not None:
                        nc.scalar.copy(xpad[:, 3 + half * 512: 3 + (half + 1) * 512], ps[:])
                    elif gate_bias is not None:
                        nc.scalar.activation(dst_bf16[:, half * 512:(half + 1) * 512],
                                             ps[:], Act.Silu, bias=gate_bias)
                    else:
                        nc.scalar.copy(dst_bf16[:, half * 512:(half + 1) * 512], ps[:])
                if xpad is None:
                    return
                cwm = cwt(m)
                s = conv_slot * 4
                a = convp.tile([128, T], fp32, tag="acca", name="acca", bufs=1)
                bt = convp.tile([128, T], fp32, tag="accb", name="accb", bufs=1)
                nc.vector.tensor_scalar(a[:], xpad[:, 3:3 + T], cwm[:, s + 3:s + 4],
                                        None, op0=Alu.mult)
                cur, nxt = a, bt
                for kk in (2, 1, 0):
                    nc.vector.scalar_tensor_tensor(nxt[:], xpad[:, kk:kk + T],
                                                   cwm[:, s + kk:s + kk + 1], cur[:],
                                                   op0=Alu.mult, op1=Alu.add)
                    cur, nxt = nxt, cur
                if pair is None:
                    nc.scalar.activation(dst_bf16[:], cur[:], Act.Silu)
                else:
                    qsil = qb[pair] if pair < 4 else kb[pair - 4]
                    qs[pair] = qsil
                    nc.scalar.activation(qsil[:], cur[:], Act.Silu)
                    sq = sqp.tile([128, T], bf16, tag="sq", name="sq", bufs=1)
                    nc.scalar.activation(sq[:], qsil[:], Act.Square)
                    for half in range(2):
                        pss = pr.tile([8, 512], fp32, tag="sqs", name="sqs")
                        nc.tensor.matmul(pss[:], oh8t[:, pair * 8:pair * 8 + 8],
                                         sq[:, half * 512:(half + 1) * 512],
                                         start=True, stop=True)
                        nc.vector.tensor_tensor(ssqsb[:, half * 512:(half + 1) * 512],
                                                ssqsb[:, half * 512:(half + 1) * 512],
                                                pss[:], op=Alu.add)

            for m in range(4):
                project(WQ0, m, conv_slot=0, pair=m)
            for m in range(4):
                project(WK0, m, conv_slot=1, pair=4 + m)
            for m in range(4):
                project(WV0, m, dst_bf16=vb[m], conv_slot=2)
            for m in range(4):
                project(WG0, m, dst_bf16=gateb[m], gate_bias=bgt[:, m:m + 1])

            # f1 projection
            for half in range(2):
                ps = pr.tile([128, 512], fp32, tag="proj", name="f1ps", bufs=2)
                for k in range(16):
                    nc.tensor.matmul(ps[:], wt[k][:, WF10:WF10 + 128],
                                     ht[k][:, half * 512:(half + 1) * 512],
                                     start=(k == 0), stop=(k == 15))
                nc.scalar.copy(f1b[:, half * 512:(half + 1) * 512], ps[:])

            # beta (sigmoid) then gate-softplus chain, table-load friendly order
            for half in range(2):
                bps = pr.tile([4, 512], fp32, tag="bps", name="bps")
                for k in range(16):
                    nc.tensor.matmul(bps[:], wt[k][:, WB0:WB0 + 4],
                                     ht[k][:, half * 512:(half + 1) * 512],
                                     start=(k == 0), stop=(k == 15))
                nc.scalar.activation(bsg[:, half * 512:(half + 1) * 512], bps[:],
                                     Act.Sigmoid)
            sp1s = []
            for half in range(2):
                for h in range(4):
                    gps = pr.tile([8, 512], fp32, tag="gps", name="gps", bufs=2)
                    nc.tensor.matmul(gps[:], wf2t[:, h * 8:(h + 1) * 8],
                                     f1b[:, half * 512:(half + 1) * 512],
                                     start=True, stop=True)
                    spe = smt.tile([8, 512], bf16, tag="spe", name="spe", bufs=2)
                    nc.scalar.activation(spe[:], gps[:], Act.Exp,
                                         bias=dtbt[:, h:h + 1])
                    sp1 = smt.tile([8, 512], bf16, tag="sp1", name="sp1", bufs=8)
                    nc.vector.tensor_scalar(sp1[:], spe[:], 1.0, None, op0=Alu.add)
                    sp1s.append((half, h, sp1))
            # all Ln together: l2 normalizer + softplus logs
            nrm = smt.tile([8, T], fp32, tag="nrm", name="nrm", bufs=1)
            nc.scalar.activation(nrm[:], ssqsb[:], Act.Ln, scale=sc8t[:, 0:1],
                                 bias=eps8t[:, 0:1])
            for half, h, sp1 in sp1s:
                sp = smt.tile([8, 512], bf16, tag="sp", name="sp", bufs=2)
                nc.scalar.activation(sp[:], sp1[:], Act.Ln)
                nc.vector.tensor_scalar(gna8[h][:, half * 512:(half + 1) * 512],
                                        sp[:], negat8(h), None, op0=Alu.mult)
            recb = smt.tile([8, T], bf16, tag="recb", name="recb", bufs=1)
            nc.scalar.activation(recb[:], nrm[:], Act.Exp, scale=-0.5)
            for pair in range(8):
                dst = qb[pair] if pair < 4 else kb[pair - 4]
                for half in range(2):
                    nb = pr.tile([128, 512], fp32, tag="nb", name="nb", bufs=2)
                    nc.tensor.matmul(nb[:], s8b[:, pair * 128:(pair + 1) * 128],
                                     recb[:, half * 512:(half + 1) * 512],
                                     start=True, stop=True)
                    nc.vector.tensor_tensor(dst[:, half * 512:(half + 1) * 512],
                                            qs[pair][:, half * 512:(half + 1) * 512],
                                            nb[:], op=Alu.mult)

        # weights/hT pools closed; load wo for the tail now (overlaps recurrence)
        pers2 = ctx.enter_context(tc.tile_pool(name="pers2", bufs=1))
        yb = [pers2.tile([128, T], bf16, tag=f"yb{m}", name=f"yb{m}")
              for m in range(4)]
        wotp = ctx.enter_context(tc.tile_pool(name="wotp", bufs=1))
        wot = [wotp.tile([128, D], bf16, tag=f"wo{k}", name=f"wo{k}") for k in range(4)]
        for k in range(4):
            dma(wot[k][:], wo[k * 128:(k + 1) * 128, :])

        # ================= recurrence =================
        rc = ctx.enter_context(tc.tile_pool(name="rc", bufs=2))
        rr = ctx.enter_context(tc.tile_pool(name="rr", bufs=4))
        rctx = ExitStack()
        prc = rctx.enter_context(tc.tile_pool(name="prc", bufs=1, space="PSUM"))
        # PSUM tags: tp(2) big(2) dblx(2) mx(2) = 8 banks

        hdt = lambda nm, h, sh, dt=bf16, bufs=2: rc.tile(
            sh, dt, tag=f"{nm}{h}", name=f"{nm}{h}", bufs=bufs)

        MSKN = {1: 0, 3: 1, 5: 2, 6: 3, 7: 4}

        def prologue(ci):
            ts = slice(ci * C, (ci + 1) * C)
            P = {'ts': ts}
            cn8s = []
            for h in range(4):
                cn8 = hdt("cn8", h, [8, C], fp32)
                nc.vector.tensor_tensor_scan(cn8[:], ones32[0:8, :],
                                             gna8[h][:, ts], 0.0,
                                             op0=Alu.mult, op1=Alu.add)
                cn8s.append(cn8)
            cnt8s = []
            for h in range(4):
                cNtp = prc.tile([128, 8], fp32, tag="tp", name="cNtp", bufs=2)
                nc.tensor.transpose(cNtp[:], cn8s[h][:], idf[0:8, 0:8])
                cnt8 = hdt("cnt8", h, [128, 8], fp32)
                nc.scalar.copy(cnt8[:], cNtp[:])
                cnt8s.append(cnt8)
            b2p = prc.tile([128, 4], fp32, tag="tp", name="b2p", bufs=2)
            nc.tensor.transpose(b2p[:], bsg[:, ts], idf[0:4, 0:4])
            beta2 = rc.tile([128, 4], fp32, tag="beta2", name="beta2")
            nc.scalar.copy(beta2[:], b2p[:])
            kmsks = []
            for h in range(4):
                kmsk = rr.tile([128, 5 * C], bf16, tag="kmsk", name="kmsk")
                for n, j in MSKN.items():
                    dst = kmsk[:, j * C:(j + 1) * C]
                    if j < 3:
                        nc.scalar.mul(dst, kb[h][:, ts], gmct[:, n:n + 1])
                    else:
                        nc.vector.tensor_scalar(dst, kb[h][:, ts], gmct[:, n:n + 1],
                                                None, op0=Alu.mult)
                kmsks.append(kmsk)
            exp8s, exp8ks = [], []
            for h in range(4):
                e8 = hdt("exp8", h, [8, C], fp32)
                nc.scalar.activation(e8[:], cn8s[h][:], Act.Exp)
                exp8s.append(e8)
            for h in range(4):
                e8k = hdt("exp8k", h, [8, C], fp32)
                nc.scalar.activation(e8k[:], cn8s[h][:], Act.Exp, scale=-1.0,
                                     bias=cn8s[h][:, C - 1:C])
                exp8ks.append(e8k)
            bfks = []
            for h in range(4):
                bfk = prc.tile([128, 256], fp32, tag="tp", name="bfk", bufs=2)
                nc.tensor.matmul(bfk[:, 0:128], replt, exp8s[h][:],
                                 start=True, stop=True)
                nc.tensor.matmul(bfk[:, 128:256], replt, exp8ks[h][:],
                                 start=True, stop=True)
                bfks.append(bfk)
            P['bC'] = []
            for h in range(4):
                bC = hdt("bC", h, [128, 1], fp32)
                nc.scalar.copy(bC[:], bfks[h][:, 127:128])
                P['bC'].append(bC)
            P['Wt'], P['qtT'], kends = [], [], []
            for h in range(4):
                Wth = hdt("Wt", h, [128, C])
                nc.vector.tensor_tensor(Wth[:], kb[h][:, ts], bfks[h][:, 0:128],
                                        op=Alu.mult)
                P['Wt'].append(Wth)
                qtTh = hdt("qtT", h, [128, C])
                nc.vector.tensor_tensor(qtTh[:], qb[h][:, ts], bfks[h][:, 0:128],
                                        op=Alu.mult)
                P['qtT'].append(qtTh)
                kendh = hdt("kend", h, [128, C])
                nc.vector.tensor_tensor(kendh[:], kb[h][:, ts], bfks[h][:, 128:256],
                                        op=Alu.mult)
                kends.append(kendh)
            ealls = []
            for h in range(4):
                bcaL = prc.tile([128, 512], fp32, tag="big", name="bcaL", bufs=2)
                bcaH = prc.tile([128, 512], fp32, tag="big", name="bcaH", bufs=2)
                for n in range(8):
                    dst = bcaL if n < 4 else bcaH
                    nc.tensor.matmul(dst[:, (n % 4) * C:(n % 4 + 1) * C],
                                     sel8[:, n * 128:(n + 1) * 128], cn8s[h][:],
                                     start=True, stop=True)
                eallin = rr.tile([128, 8 * C], bf16, tag="eallin", name="eallin")
                for n in range(4):
                    nc.scalar.activation(eallin[:, n * C:(n + 1) * C],
                                         bcaL[:, n * C:(n + 1) * C],
                                         Act.Relu, scale=-1.0,
                                         bias=cnt8s[h][:, n:n + 1])
                for n in range(4, 8):
                    nc.vector.tensor_scalar(eallin[:, n * C:(n + 1) * C],
                                            bcaH[:, (n - 4) * C:(n - 3) * C],
                                            cnt8s[h][:, n:n + 1], 0.0,
                                            op0=Alu.subtract, op1=Alu.min)
                eall = rr.tile([128, 8 * C], bf16, tag="eall", name="eall")
                nc.scalar.activation(eall[:, 0:4 * C], eallin[:, 0:4 * C],
                                     Act.Exp, scale=-1.0)
                nc.scalar.activation(eall[:, 4 * C:], eallin[:, 4 * C:], Act.Exp)
                ealls.append(eall)
            P['A'] = [[None] * 7 for _ in range(4)]
            P['GtM'] = []
            for h in range(4):
                pls = []
                for csrc in (kb[h], qb[h]):
                    pl = prc.tile([128, 512], fp32, tag="big", name="pall", bufs=2)
                    ph = prc.tile([128, 512], fp32, tag="big", name="pallh", bufs=2)
                    for n in range(8):
                        dst = pl if n < 4 else ph
                        if n in (0, 2, 4):
                            nc.tensor.matmul(dst[:, (n % 4) * C:(n % 4 + 1) * C],
                                             kb[h][16 * n:16 * (n + 1), ts],
                                             csrc[16 * n:16 * (n + 1), ts],
                                             start=True, stop=True)
                        else:
                            j = MSKN[n]
                            nc.tensor.matmul(dst[:, (n % 4) * C:(n % 4 + 1) * C],
                                             kmsks[h][:, j * C:(j + 1) * C],
                                             csrc[:, ts],
                                             start=True, stop=True)
                    pls.append((pl, ph))
                prods = []
                for x, (pl, ph) in enumerate(pls):
                    prod = rr.tile([128, 8 * C], bf16, tag="prod", name=f"prod{x}")
                    nc.vector.tensor_tensor(prod[:, 0:4 * C], ealls[h][:, 0:4 * C],
                                            pl[:], op=Alu.mult)
                    nc.vector.tensor_tensor(prod[:, 4 * C:], ealls[h][:, 4 * C:],
                                            ph[:], op=Alu.mult)
                    prods.append(prod)
                for x, prod in enumerate(prods):
                    t4 = rr.tile([128, 4 * C], bf16, tag="t4", name="t4")
                    nc.vector.tensor_tensor(t4[:], prod[:, :4 * C], prod[:, 4 * C:],
                                            op=Alu.add)
                    t2 = rr.tile([128, 2 * C], bf16, tag="t2", name="t2")
                    nc.vector.tensor_tensor(t2[:], t4[:, :2 * C], t4[:, 2 * C:],
                                            op=Alu.add)
                    t1 = rr.tile([128, C], bf16, tag="t1", name="t1")
                    nc.vector.tensor_tensor(t1[:], t2[:, :C], t2[:, C:], op=Alu.add)
                    if x == 0:
                        A0 = rc.tile([128, C], bf16, tag=f"A{h}", name=f"A{h}_0",
                                     bufs=14)
                        nc.vector.scalar_tensor_tensor(A0[:], t1[:],
                                                       beta2[:, h:h + 1], mMt[:],
                                                       op0=Alu.mult, op1=Alu.mult)
                        P['A'][h][0] = A0
                    else:
                        GtM = hdt("GtM", h, [128, C])
                        nc.vector.scalar_tensor_tensor(GtM[:], t1[:],
                                                       beta2[:, h:h + 1], mGt[:],
                                                       op0=Alu.mult, op1=Alu.mult)
                        P['GtM'].append(GtM)
            P['vt'], P['kts'] = [], []
            for h in range(4):
                vtp = prc.tile([128, C], bf16, tag="tp", name="vtp", bufs=2)
                nc.tensor.transpose(vtp[:], vb[h][:, ts], idb[:])
                vt = hdt("vt", h, [128, C])
                nc.scalar.copy(vt[:], vtp[:])
                P['vt'].append(vt)
                ktp = prc.tile([128, C], bf16, tag="tp", name="ktp", bufs=2)
                nc.tensor.transpose(ktp[:], kends[h][:], idb[:])
                kts = hdt("kts", h, [128, C])
                nc.vector.tensor_scalar(kts[:], ktp[:], beta2[:, h:h + 1], None,
                                        op0=Alu.mult)
                P['kts'].append(kts)
            # A/B doubling chains (state-independent)
            Bs = [[None] * 6 for _ in range(4)]
            for h in range(4):
                b0p = prc.tile([128, C], bf16, tag="tp", name="b0p", bufs=2)
                nc.tensor.transpose(b0p[:], P['A'][h][0][:], idb[:])
                B0 = rc.tile([128, C], bf16, tag=f"B{h}", name=f"B{h}_0", bufs=3)
                nc.scalar.copy(B0[:], b0p[:])
                Bs[h][0] = B0
            for lev in range(1, 7):
                for h in range(4):
                    dbl = prc.tile([128, 256], fp32, tag="dblx", name="dbl", bufs=2)
                    nc.tensor.matmul(dbl[:, 0:128], Bs[h][lev - 1][:],
                                     P['A'][h][lev - 1][:], start=True, stop=True)
                    if lev < 6:
                        nc.tensor.matmul(dbl[:, 128:256], P['A'][h][lev - 1][:],
                                         Bs[h][lev - 1][:], start=True, stop=True)
                    An = rc.tile([128, C], bf16, tag=f"A{h}", name=f"A{h}_{lev}",
                                 bufs=14)
                    nc.scalar.copy(An[:], dbl[:, 0:128])
                    P['A'][h][lev] = An
                    if lev < 6:
                        Bn = rc.tile([128, C], bf16, tag=f"B{h}",
                                     name=f"B{h}_{lev}", bufs=3)
                        if lev % 2 == 0:
                            nc.scalar.copy(Bn[:], dbl[:, 128:256])
                        else:
                            nc.vector.tensor_copy(Bn[:], dbl[:, 128:256])
                        Bs[h][lev] = Bn
            return P

        def spine(ci, P):
            ts = P['ts']
            po, pn = ci % 2, (ci + 1) % 2
            xbs = []
            for h in range(4):
                ws0 = prc.tile([128, C], fp32, tag="tp", name="ws0", bufs=2)
                nc.tensor.matmul(ws0[:], P['Wt'][h][:], Sb[h][po][:],
                                 start=True, stop=True)
                xb = rc.tile([128, C], bf16, tag=f"xb{h}", name=f"xb{h}", bufs=3)
                nc.vector.tensor_tensor(xb[:], P['vt'][h][:], ws0[:],
                                        op=Alu.subtract)
                xbs.append(xb)
            for lev in range(7):
                for h in range(4):
                    mx = prc.tile([128, C], fp32, tag="mx", name="mx", bufs=2)
                    nc.tensor.matmul(mx[:], P['A'][h][lev][:], xbs[h][:],
                                     start=True, stop=True)
                    xn = rc.tile([128, C], bf16, tag=f"xb{h}", name=f"xb{h}_{lev}",
                                 bufs=3)
                    nc.vector.tensor_tensor(xn[:], xbs[h][:], mx[:],
                                            op=(Alu.subtract if lev == 0 else Alu.add))
                    xbs[h] = xn
            for h in range(4):
                sup = prc.tile([128, DV], fp32, tag="tp", name="sup", bufs=2)
                nc.tensor.matmul(sup[:], P['kts'][h][:], xbs[h][:],
                                 start=True, stop=True)
                nc.vector.scalar_tensor_tensor(Sf[h][pn][:], Sf[h][po][:],
                                               P['bC'][h][:, 0:1], sup[:],
                                               op0=Alu.mult, op1=Alu.add)
                nc.vector.scalar_tensor_tensor(Sb[h][pn][:], Sf[h][po][:],
                                               P['bC'][h][:, 0:1], sup[:],
                                               op0=Alu.mult, op1=Alu.add)
                otp = prc.tile([128, C], fp32, tag="tp", name="otp", bufs=2)
                nc.tensor.matmul(otp[:], Sb[h][po][:], P['qtT'][h][:],
                                 start=True, stop=False)
                nc.tensor.matmul(otp[:], xbs[h][:], P['GtM'][h][:],
                                 start=False, stop=True)
                nc.vector.tensor_tensor(yb[h][:, ts], gateb[h][:, ts], otp[:],
                                        op=Alu.mult)

        pros = [None, None]
        for ci in range(NCH + 1):
            if ci < NCH:
                pros[ci % 2] = prologue(ci)
            if ci >= 1:
                spine(ci - 1, pros[(ci - 1) % 2])

        rctx.close()
        # ================= deferred RMSNorm + output projection =================
        with tc.tile_pool(name="post", bufs=2) as post, \
             tc.tile_pool(name="ppc", bufs=2, space="PSUM") as ppc:
            # PSUM tags: ssp(2) rbc(2) proj(2) = 6 banks
            ysqs = []
            for h in range(4):
                ysq = post.tile([128, T], bf16, tag="ysq", name="ysq", bufs=4)
                nc.scalar.activation(ysq[:], yb[h][:], Act.Square)
                ysqs.append(ysq)
            nrcs = []
            for h in range(4):
                nrc = post.tile([1, T], fp32, tag="nrc", name="nrc", bufs=4)
                for half in range(2):
                    ssp = ppc.tile([1, 512], fp32, tag="ssp", name="ssp")
                    nc.tensor.matmul(ssp[:], octb[:],
                                     ysqs[h][:, half * 512:(half + 1) * 512],
                                     start=True, stop=True)
                    nc.scalar.activation(nrc[:, half * 512:(half + 1) * 512],
                                         ssp[:], Act.Ln, scale=1.0 / DV,
                                         bias=epsnt[:, 0:1])
                nrcs.append(nrc)
            for h in range(4):
                rcb = post.tile([1, T], bf16, tag="rcb", name="rcb", bufs=4)
                nc.scalar.activation(rcb[:], nrcs[h][:], Act.Exp, scale=-0.5)
                for half in range(2):
                    rbc = ppc.tile([128, 512], fp32, tag="rbc", name="rbc")
                    nc.tensor.matmul(rbc[:], o1b[:], rcb[:, half * 512:(half + 1) * 512],
                                     start=True, stop=True)
                    nc.vector.scalar_tensor_tensor(yb[h][:, half * 512:(half + 1) * 512],
                                                   yb[h][:, half * 512:(half + 1) * 512],
                                                   nwt[:, 0:1], rbc[:],
                                                   op0=Alu.mult, op1=Alu.mult)
            # output projection
            for m in range(16):
                osb = post.tile([128, T], fp32, tag="osb", name="osb")
                for half in range(2):
                    ps = ppc.tile([128, 512], fp32, tag="proj", name="ops")
                    for k in range(4):
                        nc.tensor.matmul(ps[:], wot[k][:, m * 128:(m + 1) * 128],
                                         yb[k][:, half * 512:(half + 1) * 512],
                                         start=(k == 0), stop=(k == 3))
                    if half == 0:
                        nc.vector.tensor_copy(osb[:, 0:512], ps[:])
                    else:
                        nc.scalar.copy(osb[:, 512:1024], ps[:])
                dma(outT[m * 128:(m + 1) * 128, :], osb[:])

    nc.compile()
    return nc


def _prep_inputs(inputs):
    f32 = np.float32
    hs = np.asarray(inputs['hidden_states'], f32)
    tri = np.tril(np.ones((C, C), f32))
    maskM = (1.0 - tri).astype(f32)
    maskG = (1.0 - tri + np.eye(C, dtype=f32)).astype(f32)
    repl = np.zeros((NG, DK), f32)
    for n in range(NG):
        repl[n, n * GG:(n + 1) * GG] = 1.0
    sel8 = np.zeros((NG, NG * 128), f32)
    for n in range(NG):
        sel8[n, n * 128:(n + 1) * 128] = 1.0
    oh8 = np.zeros((DK, 64), f32)
    for i in range(8):
        oh8[:, i * 8 + i] = 1.0
    ident = np.eye(128, dtype=f32)

    maps = []
    for c in range(8):
        b, hg = c // 4, c % 4
        cols = slice(hg * NH * DK, (hg + 1) * NH * DK)
        gcols = slice(hg * NH * NG, (hg + 1) * NH * NG)
        hcols = slice(hg * NH, (hg + 1) * NH)
        nega = -np.exp(np.repeat(np.asarray(inputs['A_log'], f32)[hcols], NG))

        packf = np.zeros((128, NF), f32)
        cw = np.concatenate(
            [np.asarray(inputs['conv_q'], f32)[cols],
             np.asarray(inputs['conv_k'], f32)[cols],
             np.asarray(inputs['conv_v'], f32)[cols]], 1)  # [512, 12]
        for m in range(4):
            packf[:, PF_CW + m * 12:PF_CW + (m + 1) * 12] = cw[m * 128:(m + 1) * 128]
        packf[:, PF_BG:PF_BG + 4] = np.asarray(inputs['bg'], f32)[cols].reshape(NH, DV).T
        packf[:, PF_NW] = np.asarray(inputs['norm_w'], f32)
        packf[0:8, PF_NEGA:PF_NEGA + 4] = nega.reshape(NH, NG).T
        packf[0:8, PF_DTB:PF_DTB + 4] = (
            np.asarray(inputs['dt_bias'], f32)[gcols].reshape(NH, NG).T)
        packf[0:8, PF_SC8] = [1.0 / SCALE ** 2] * 4 + [1.0] * 4
        packf[0:8, PF_EPS8] = [1e-6 / SCALE ** 2] * 4 + [1e-6] * 4
        packf[0:1, PF_EPSN] = EPS
        packf[0:8, PF_REPL:PF_REPL + 128] = repl
        packf[:, PF_IDF:PF_IDF + 128] = ident
        packf[0:8, PF_SEL:PF_SEL + 1024] = sel8
        packf[:, PF_GMC:PF_GMC + 8] = repl.T

        packb = np.zeros((128, NB), f32)
        packb[:, PB_OH8:PB_OH8 + 64] = oh8
        packb[0:8, PB_S8B:PB_S8B + 1024] = sel8
        packb[:, PB_MM:PB_MM + 128] = maskM
        packb[:, PB_MG:PB_MG + 128] = maskG
        packb[:, PB_IDB:PB_IDB + 128] = ident
        packb[:, PB_OCT] = 1.0
        packb[0:1, PB_O1B:PB_O1B + 128] = 1.0

        wallm = np.concatenate(
            [np.asarray(inputs['Wq'], f32)[:, cols],
             np.asarray(inputs['Wk'], f32)[:, cols],
             np.asarray(inputs['Wv'], f32)[:, cols],
             np.asarray(inputs['Wg'], f32)[:, cols],
             np.asarray(inputs['Wf1'], f32),
             np.asarray(inputs['Wb'], f32)[:, hcols]], 1)

        m = {
            'hT': np.ascontiguousarray(hs[b].T).astype(BF),
            'wall': np.ascontiguousarray(wallm).astype(BF),
            'wo': np.ascontiguousarray(np.asarray(inputs['Wo'], f32)[cols, :]).astype(BF),
            'wf2': np.ascontiguousarray(np.asarray(inputs['Wf2'], f32)[:, gcols]).astype(BF),
            'packf': packf,
            'packb': packb.astype(BF),
        }
        maps.append(m)
    return maps


def kernel(**inputs):
    from concourse.bass_utils import run_bass_kernel_spmd
    if 'nc' not in _CACHE:
        _CACHE['nc'] = _build()
    nc = _CACHE['nc']
    maps = _prep_inputs(inputs)
    res = run_bass_kernel_spmd(nc, maps, list(range(8))).results
    out = np.zeros((B, T, D), np.float32)
    for c in range(8):
        out[c // 4] += res[c]['outT'].T.astype(np.float32)
    return out
